# revision 70
# baseline (speedup 1.0000x reference)
"""Trainium2 Bass kernel for nn_BasicBlock (EfficientViT-style block), v3.

Data-parallel over 8 NeuronCores: batch 64 -> 8 images/core.
SBUF-resident bf16 spatial trunk, no DRAM intermediates.
Depthwise convs via valid-region shifted views, split across V/S/G engines.
Per-core program: dw0 -> MLP0 -> cascaded window attention -> proj -> dw1 -> MLP1.
"""
import itertools
import functools
from contextlib import ExitStack
import numpy as np
import ml_dtypes

import concourse.bass as bass
import concourse.mybir as mybir
import concourse.tile as tile
from concourse import bacc
from concourse import bass_utils

f32 = mybir.dt.float32
bf16 = mybir.dt.bfloat16
AO = mybir.AluOpType
AF = mybir.ActivationFunctionType

ED, KD, NH, AR = 512, 16, 8, 4
D = AR * KD            # 64
DH = D * NH            # 512
RES, WS = 28, 7
SCALE = KD ** -0.5
KS = [7, 5, 3, 3, 3, 3, 3, 3]
NI = 8                 # images per core
NCORES = 8
POS = RES * RES        # 784
NW = 16                # windows per image
WN = WS * WS           # 49


def _bias_idx(ws):
    pts = list(itertools.product(range(ws), range(ws)))
    offs, idxs = {}, []
    for p1 in pts:
        for p2 in pts:
            o = (abs(p1[0] - p2[0]), abs(p1[1] - p2[1]))
            if o not in offs:
                offs[o] = len(offs)
            idxs.append(offs[o])
    return np.array(idxs, dtype=np.int32).reshape(ws * ws, ws * ws), len(offs)


BIAS_IDX, N_OFFS = _bias_idx(WS)


# ---------------------------------------------------------------------------
# conv planning (engine split)
# ---------------------------------------------------------------------------

def _region(sz, d):
    """1D dst range [y0,y1) for shift d (src index = dst + d)."""
    return max(0, -d), sz - max(0, d)


def _plan_taps1(k, cols, fold_engs, desc, g0):
    """Greedy engine split for k*k taps. cols(t) -> per-tap column count.

    V taps: STT accumulate in place. S taps: ACT into a cycling tmp slot,
    folded into the accumulator by a V tensor_tensor (@2x) or a G
    tensor_tensor (slow but off the critical engines).
    Returns ((v_taps, s_taps), wall); s_taps entries are (tap, fold_engine).
    """
    p = k // 2
    center = (p, p)
    ccols = cols(center)
    busy = {"V": ccols * 0.26 + 105 + ccols * 0.52 + 105, "S": 0.0, "G": g0}
    v_taps, s_taps = [], []
    order = sorted([t for t in itertools.product(range(k), range(k))
                    if t != center], key=lambda t: (-cols(t) if desc else cols(t)))
    for t in order:
        c = cols(t)
        cand = {}
        nb = dict(busy)
        nb["V"] = busy["V"] + c * 1.042 + 105
        cand["V"] = max(nb.values())
        if "V" in fold_engs:
            nb = dict(busy)
            nb["S"] = busy["S"] + c * 0.833 + 217
            nb["V"] = busy["V"] + c * 0.52 + 105
            cand["SV"] = max(nb.values())
        if "G" in fold_engs:
            nb = dict(busy)
            nb["S"] = busy["S"] + c * 0.833 + 217
            nb["G"] = busy["G"] + c * 1.98 + 156
            cand["SG"] = max(nb.values())
        eng = min(cand, key=lambda e: cand[e])
        if eng == "V":
            v_taps.append(t)
            busy["V"] += c * 1.042 + 105
        elif eng == "SV":
            s_taps.append((t, "V"))
            busy["S"] += c * 0.833 + 217
            busy["V"] += c * 0.52 + 105
        else:
            s_taps.append((t, "G"))
            busy["S"] += c * 0.833 + 217
            busy["G"] += c * 1.98 + 156
    return (v_taps, s_taps), max(busy.values())


def plan_taps(k, cols, g0=0.0):
    best = None
    for folds in (("V",), ("G",), ("V", "G")):
        for desc in (True, False):
            plan, wall = _plan_taps1(k, cols, folds, desc, g0)
            if best is None or wall < best[1]:
                best = (plan, wall)
    return best[0]


def _cols_sp(t, k):
    dy, dx = t
    p = k // 2
    return (RES - abs(dy - p)) * (RES - abs(dx - p))


PLAN_DW = plan_taps(3, lambda t: _cols_sp(t, 3), g0=1000.0)

# shared 1x16 guttered macro-grid for the per-head q convs: 7 rows,
# 16 windows of 7 cols separated by 3-col gutters (max pad of any head)
GUT = 3
GCOLS = 16 * 7 + 15 * GUT       # 157
GROWS = 7


def _cols_gq(t, k):
    dy, dx = t
    p = k // 2
    return (GROWS - abs(dy - p)) * (GCOLS - abs(dx - p))


PLAN_DWQ = {_k: plan_taps(_k, lambda t: _cols_gq(t, _k), g0=1000.0)
            for _k in (3, 5, 7)}
NSLOT = 4


# dw conv engine split: V taps (flat STT), S taps (flat ACT tmps),
# folds mostly on V (flat TT @2x), FOLD_G set folded on GpSimd
SPLIT_DW0 = ([(1, 0), (1, 2), (0, 1), (0, 0)],
             [(0, 2), (2, 0), (2, 1), (2, 2)], {(2, 1)})
SPLIT_DW1 = ([(1, 0), (1, 2), (0, 1), (0, 0), (2, 2)],
             [(0, 2), (2, 0), (2, 1)], {(2, 1)})


def emit_conv_dw(nc, wt, bt, g, g2, acc, tmp_fn, final_fn, split):
    """3x3 depthwise conv, flat shifted taps over a [128,30,32] padded grid.

    g2 is g shifted left one column (keeps even element offsets for the
    odd-dx taps). acc is [128, 28, 32]; flat cols 0..891 hold the interior.
    """
    DW_V_TAPS, DW_S_TAPS, DW_FOLD_G = split
    gf = g[:].rearrange("p h w -> p (h w)")
    g2f = g2[:].rearrange("p h w -> p (h w)") if g2 is not None else None
    af = acc[:].rearrange("p h w -> p (h w)")

    def src(t):
        dy, dx = t
        if g2f is not None and dx == 1:
            return g2f[:, 32 * dy:32 * dy + 892]
        return gf[:, 32 * dy + dx:32 * dy + dx + 892]

    def w_(t):
        return wt[:, (t[0] * 3 + t[1]):(t[0] * 3 + t[1]) + 1]

    nc.vector.tensor_scalar(af[:, 0:892], src((1, 1)), w_((1, 1)),
                            bt[:, 0:1], AO.mult, AO.add)
    vq = list(DW_V_TAPS)
    for i, t in enumerate(DW_S_TAPS):
        tm = tmp_fn(i % NSLOT)
        tf = tm[:].rearrange("p h w -> p (h w)") if len(tm.shape) == 3 else tm[:]
        nc.scalar.activation(tf[:, 0:892], src(t), AF.Identity, scale=w_(t))
        if t in DW_FOLD_G:
            nc.gpsimd.tensor_tensor(af[:, 0:892], tf[:, 0:892], af[:, 0:892],
                                    AO.add)
        else:
            nc.vector.tensor_tensor(af[:, 0:892], tf[:, 0:892], af[:, 0:892],
                                    AO.add)
        if vq:
            t2 = vq.pop(0)
            nc.vector.scalar_tensor_tensor(af[:, 0:892], src(t2), w_(t2),
                                           af[:, 0:892], AO.mult, AO.add)
    for t2 in vq:
        nc.vector.scalar_tensor_tensor(af[:, 0:892], src(t2), w_(t2),
                                       af[:, 0:892], AO.mult, AO.add)
    final_fn()


def emit_conv_grid(nc, k, wt, bt, G, GA, tmp_fn, warm_fn):
    """k*k depthwise conv on the shared guttered macro-grid [128, 7, 157]."""
    p = k // 2
    v_taps, s_taps = PLAN_DWQ[k]

    def dst_v(base, t):
        dy, dx = t
        y0, y1 = _region(GROWS, dy - p)
        x0, x1 = _region(GCOLS, dx - p)
        return base[:, y0:y1, x0:x1]

    def src_v(t):
        dy, dx = t
        y0, y1 = _region(GROWS, dy - p)
        x0, x1 = _region(GCOLS, dx - p)
        return G[:, y0 + dy - p:y1 + dy - p, x0 + dx - p:x1 + dx - p]

    def wcol(t):
        return t[0] * k + t[1]

    nc.vector.tensor_scalar(dst_v(GA[:], (p, p)), src_v((p, p)),
                            wt[:, wcol((p, p)):wcol((p, p)) + 1],
                            bt[:, 0:1], AO.mult, AO.add)
    vq = list(v_taps)
    for i, (t, feng) in enumerate(s_taps):
        tm = tmp_fn(i % NSLOT)
        nc.scalar.activation(dst_v(tm[:], t), src_v(t), AF.Identity,
                             scale=wt[:, wcol(t):wcol(t) + 1])
        av, tv = dst_v(GA[:], t), dst_v(tm[:], t)
        if feng == "G":
            nc.gpsimd.tensor_tensor(av, tv, av, AO.add)
        else:
            nc.vector.tensor_tensor(av, tv, av, AO.add)
        if warm_fn is not None and i % 2 == 0:
            warm_fn(tm, t)
        if vq:
            t2 = vq.pop(0)
            nc.vector.scalar_tensor_tensor(dst_v(GA[:], t2), src_v(t2),
                                           wt[:, wcol(t2):wcol(t2) + 1],
                                           dst_v(GA[:], t2), AO.mult, AO.add)
    for t2 in vq:
        nc.vector.scalar_tensor_tensor(dst_v(GA[:], t2), src_v(t2),
                                       wt[:, wcol(t2):wcol(t2) + 1],
                                       dst_v(GA[:], t2), AO.mult, AO.add)


# ---------------------------------------------------------------------------
# program builder
# ---------------------------------------------------------------------------

def build_program():
    nc = bacc.Bacc("TRN2", target_bir_lowering=False, debug=False,
                   enable_asserts=False, num_devices=NCORES)

    def din(name, shape, dt=f32):
        return nc.dram_tensor(name, list(shape), dt, kind="ExternalInput").ap()

    x_d = din("x", [NI, ED, POS], bf16)
    dwpk_d = din("dwpk", [128, 80])
    w1T0_d = din("w1T0", [ED, 2 * ED], bf16)
    b1f0_d = din("b1f0", [2 * ED], bf16)
    w2T0_d = din("w2T0", [2 * ED, ED], bf16)
    attkq_d = din("attkq", [128, NH * 2 * KD], bf16)
    attwv_d = din("attwv", [128, NH * D], bf16)
    attbias_d = din("attbias", [128, 2 * NH])
    dwqw_d = din("dwqw", [128, NH * 50])
    ab_d = din("ab", [WN, NH * 392], bf16)
    iab_d = din("iab", [WN, 128], bf16)
    ones2_d = din("ones2", [128, 2], bf16)
    sel2_d = din("sel2", [2, 128])
    projT_d = din("projT", [DH, ED], bf16)
    bpk_d = din("bpk", [128, 16])
    w1T1_d = din("w1T1", [ED, 2 * ED], bf16)
    b1f1_d = din("b1f1", [2 * ED], bf16)
    w2T1_d = din("w2T1", [2 * ED, ED], bf16)

    out_d = nc.dram_tensor("out", [NI, ED, POS], f32, kind="ExternalOutput").ap()

    with tile.TileContext(nc) as tc:
        _body(tc, nc, x_d, dwpk_d, w1T0_d, b1f0_d, w2T0_d,
              attkq_d, attwv_d, attbias_d, dwqw_d, ab_d,
              iab_d, ones2_d, sel2_d,
              projT_d, bpk_d,
              w1T1_d, b1f1_d, w2T1_d, out_d)

    nc.compile()
    return nc


def win_ap(ap392, n2, w, spatial):
    """Per-window [*, 49] AP from a 392-col half. spatial: 3D 7x7 slice of
    the 14x28 spatial half; else dense 49-block (window-block layout)."""
    if spatial:
        al, b = w // 4, w % 4
        v = ap392.rearrange("p (h x) -> p h x", h=14)
        return v[:, 7 * al:7 * al + 7, 7 * b:7 * b + 7]
    return ap392[:, WN * w:WN * (w + 1)]


def _body(tc, nc, x_d, dwpk_d, w1T0_d, b1f0_d, w2T0_d,
          attkq_d, attwv_d, attbias_d, dwqw_d, ab_d,
          iab_d, ones2_d, sel2_d,
          projT_d, bpk_d,
          w1T1_d, b1f1_d, w2T1_d, out_d):

    # ---------------- persistent pools -------------------------------------
    wp_cm = tc.tile_pool(name="wp", bufs=1)
    wp = wp_cm.__enter__()
    big_cm = tc.tile_pool(name="big", bufs=1)
    big = big_cm.__enter__()
    xw_cm = tc.tile_pool(name="xw", bufs=1)
    xwp = xw_cm.__enter__()

    def load_mlp_w(w1T_dram, b1_dram, w2T_dram, b2_dram, pool):
        w1sb = []
        for k in range(4):
            w = pool.tile([128, 2 * ED], bf16, tag=f"w1_{k}")
            nc.sync.dma_start(out=w, in_=w1T_dram[128 * k:128 * (k + 1), :])
            w1sb.append(w)
        w2sb = []
        for k in range(8):
            w = pool.tile([128, ED], bf16, tag=f"w2_{k}")
            nc.sync.dma_start(out=w, in_=w2T_dram[128 * k:128 * (k + 1), :])
            w2sb.append(w)
        b1row = pool.tile([1, 2 * ED], bf16, tag="b1row")
        nc.sync.dma_start(out=b1row, in_=b1_dram.unsqueeze(0))
        b2sb = []
        for m in range(4):
            b = pool.tile([128, 1], f32, tag=f"b2_{m}")
            nc.sync.dma_start(out=b, in_=b2_dram[128 * m:128 * (m + 1)].unsqueeze(1))
            b2sb.append(b)
        return w1sb, w2sb, b1row, b2sb

    # dw weights
    dw_w, dw_b = {}, {}
    for nm, wd, bd in (("dw0", dw0w_d, dw0b_d), ("dw1", dw1w_d, dw1b_d)):
        ws_, bs_ = [], []
        for c in range(4):
            w = wp.tile([128, 9], f32, tag=f"{nm}w{c}")
            nc.sync.dma_start(out=w, in_=wd[c])
            b = wp.tile([128, 1], f32, tag=f"{nm}b{c}")
            nc.sync.dma_start(out=b, in_=bd[c].unsqueeze(1))
            ws_.append(w)
            bs_.append(b)
        dw_w[nm], dw_b[nm] = ws_, bs_

    ones392 = wp.tile([1, 392], bf16, tag="ones392")
    nc.vector.memset(ones392, 1.0)
    acth = wp.tile([128, 1], f32, tag="acth")
    nc.vector.memset(acth, 0.5)
    acts = wp.tile([128, 1], f32, tag="acts")
    nc.vector.memset(acts, 1.0 / 6.0)

    # attention weights
    wkq_sb, bkq_sb, wv_sb, bv_sb, dq_w, dq_b, ab_sb = [], [], [], [], [], [], []
    for h in range(NH):
        t = wp.tile([128, 2 * KD], bf16, tag=f"wkq{h}")
        nc.sync.dma_start(out=t[0:64, :], in_=wkqT_d[h])
        nc.sync.dma_start(out=t[64:128, :], in_=wkqT_d[h])
        wkq_sb.append(t)
        t = wp.tile([128, 1], f32, tag=f"bkq{h}")
        nc.sync.dma_start(out=t[0:32, :], in_=bkq_d[h].unsqueeze(1))
        nc.sync.dma_start(out=t[64:96, :], in_=bkq_d[h].unsqueeze(1))
        bkq_sb.append(t)
        t = wp.tile([128, D], bf16, tag=f"wv{h}")
        nc.sync.dma_start(out=t[0:64, :], in_=wvT_d[h])
        nc.sync.dma_start(out=t[64:128, :], in_=wvT_d[h])
        wv_sb.append(t)
        t = wp.tile([128, 1], f32, tag=f"bv{h}")
        nc.sync.dma_start(out=t[0:64, :], in_=bv_d[h].unsqueeze(1))
        nc.sync.dma_start(out=t[64:128, :], in_=bv_d[h].unsqueeze(1))
        bv_sb.append(t)

        t = wp.tile([128, 49], f32, tag=f"dqw{h}")
        nc.sync.dma_start(out=t, in_=dwqw_d[h])
        dq_w.append(t)
        t = wp.tile([128, 1], f32, tag=f"dqb{h}")
        nc.sync.dma_start(out=t, in_=dwqb_d[h].unsqueeze(1))
        dq_b.append(t)
        t = wp.tile([WN, 392], bf16, tag=f"ab{h}")
        nc.sync.dma_start(out=t, in_=ab_d[h])
        ab_sb.append(t)
    iab_sb = wp.tile([WN, 128], bf16, tag="iab")
    nc.sync.dma_start(out=iab_sb, in_=iab_d)
    ones2_sb = wp.tile([128, 2], bf16, tag="ones2")
    nc.sync.dma_start(out=ones2_sb, in_=ones2_d)
    sel2_sb = wp.tile([2, 128], f32, tag="sel2")
    nc.sync.dma_start(out=sel2_sb, in_=sel2_d)

    # proj
    pj_sb = []
    for k in range(4):
        w = wp.tile([128, ED], bf16, tag=f"pj{k}")
        nc.sync.dma_start(out=w, in_=projT_d[128 * k:128 * (k + 1), :])
        pj_sb.append(w)
    pjb_sb, yb_sb = [], []
    for m in range(4):
        b = wp.tile([128, 1], f32, tag=f"pjb{m}")
        nc.sync.dma_start(out=b, in_=projb_d[128 * m:128 * (m + 1)].unsqueeze(1))
        pjb_sb.append(b)
        b = wp.tile([128, 1], f32, tag=f"ybt{m}")
        nc.sync.dma_start(out=b, in_=yb_d[128 * m:128 * (m + 1)].unsqueeze(1))
        yb_sb.append(b)

    w1sb0, w2sb0, b1row0, b2sb0 = load_mlp_w(w1T0_d, b1f0_d, w2T0_d, b2f0_d, wp)

    # ---------------- MLP per-image emitter --------------------------------
    def mlp_img(pools, img, w1sb, w2sb, b1row, b2sb, rhs_getter, out_writer, name):
        hp, rp, psp, pop = pools
        hs = []
        for m in range(8):
            h = hp.tile([128, POS], bf16, tag=f"h{m}", name=f"{name}h{m}_{img}")
            hs.append(h)
        for m in range(8):
            for n2 in range(2):
                ph = psp.tile([128, 392], f32, tag="ph")
                for k in range(4):
                    nc.tensor.matmul(
                        ph[:], w1sb[k][:, 128 * m:128 * (m + 1)],
                        rhs_getter(k, img, n2),
                        start=(k == 0), stop=False)
                nc.tensor.matmul(
                    ph[:], b1row[:, 128 * m:128 * (m + 1)],
                    ones392[:], start=False, stop=True)
                r = rp.tile([128, 392], bf16, tag="relu")
                nc.scalar.activation(r[:], ph[:], AF.Relu,
                                     scale=acts[:, 0:1], bias=acth[:, 0:1])
                nc.vector.scalar_tensor_tensor(
                    hs[m][:, 392 * n2:392 * (n2 + 1)], r[:], 1.0,
                    ph[:], AO.min, AO.mult)
        for mo in range(4):
            for n2 in range(2):
                po = pop.tile([128, 392], f32, tag="po")
                for k in range(8):
                    nc.tensor.matmul(
                        po[:], w2sb[k][:, 128 * mo:128 * (mo + 1)],
                        hs[k][:, 392 * n2:392 * (n2 + 1)],
                        start=(k == 0), stop=(k == 7))
                out_writer(mo, img, n2, po, b2sb[mo])

    # ---------------- P0+P1: input DMA + dw0 + residual -> x1 (spatial) -----
    x1fl = [big.tile([128, NI, POS], bf16, tag=f"fl{c}", name=f"x1_{c}")
            for c in range(4)]

    def rhs0(k, img, n2):
        return x1fl[k][:, img, 392 * n2:392 * (n2 + 1)]

    def outw0(mo, img, n2, po, b2):
        ov = x1fl[mo][:, img, 392 * n2:392 * (n2 + 1)]
        nc.vector.scalar_tensor_tensor(ov, po[:], b2[:, 0:1], ov, AO.add, AO.add)

    GN = 3
    with tc.tile_pool(name="grd", bufs=1) as grdp, \
         tc.tile_pool(name="dac", bufs=3) as dacp, \
         tc.tile_pool(name="dwt0", bufs=2) as tmp0, \
         tc.tile_pool(name="m0h", bufs=2) as hp0, \
         tc.tile_pool(name="m0r", bufs=4) as rp0, \
         tc.tile_pool(name="m0ps", bufs=4, space="PSUM") as psp0, \
         tc.tile_pool(name="m0po", bufs=2, space="PSUM") as pop0:
        g_set, g2_set = [], []
        for i in range(GN):
            g = grdp.tile([128, 30, 32], bf16, tag=f"g{i}", name=f"g{i}")
            nc.gpsimd.memset(g[:], 0.0)
            g_set.append(g)
            g2 = grdp.tile([128, 30, 32], bf16, tag=f"g2{i}", name=f"g2{i}")
            nc.gpsimd.memset(g2[:], 0.0)
            g2_set.append(g2)
        for img in range(NI):
            for c in range(4):
                u = (4 * img + c) % GN
                g, g2 = g_set[u], g2_set[u]
                nc.scalar.dma_start(
                    out=g[:, 1:29, 1:29],
                    in_=x_d[img, 128 * c:128 * (c + 1), :]
                    .rearrange("p (h w) -> p h w", h=RES))
                nc.scalar.activation(
                    g2[:].rearrange("p h w -> p (h w)")[:, 0:959],
                    g[:].rearrange("p h w -> p (h w)")[:, 1:960], AF.Copy)
                acc = dacp.tile([128, 28, 32], bf16, tag="acc",
                                name=f"a0_{c}_{img}")

                def tmp_fn(i):
                    return tmp0.tile([128, 896], bf16, tag=f"tm{i}",
                                     name=f"tm0_{c}_{img}_{i}")

                def final_fn():
                    nc.vector.tensor_tensor(
                        x1fl[c][:, img, :].rearrange("p (h w) -> p h w", h=RES),
                        acc[:, :, 0:28], g[:, 1:29, 1:29], AO.add)

                emit_conv_dw(nc, dw_w["dw0"][c], dw_b["dw0"][c], g, g2,
                             acc, tmp_fn, final_fn, SPLIT_DW0)
            mlp_img((hp0, rp0, psp0, pop0), img, w1sb0, w2sb0, b1row0, b2sb0,
                    rhs0, outw0, "m0")

    x2fl = x1fl   # trunk now holds x2 (spatial, bf16)

    # ---------------- P3: cascaded attention -> y_sb ------------------------
    # y in window-block layout: y_sb[c][64*h2+d, img, 49*w + pos]
    y_sb = [xwp.tile([128, NI, POS], bf16, tag=f"wm{c}", name=f"y_{c}")
            for c in range(4)]

    def prow(i):
        return 64 * (i % 2)

    with ExitStack() as es:
        spkp = es.enter_context(tc.tile_pool(name="spk", bufs=1))
        spxp = es.enter_context(tc.tile_pool(name="spx", bufs=2))
        spp = es.enter_context(tc.tile_pool(name="sp", bufs=1))
        kqtp = es.enter_context(tc.tile_pool(name="kqt", bufs=1))
        kpkp = es.enter_context(tc.tile_pool(name="kpk", bufs=1))
        vtp = es.enter_context(tc.tile_pool(name="vt", bufs=1))
        qgp = es.enter_context(tc.tile_pool(name="qg", bufs=1))
        qgrp = es.enter_context(tc.tile_pool(name="qgr", bufs=1))
        qgap = es.enter_context(tc.tile_pool(name="qga", bufs=1))
        qtmp = es.enter_context(tc.tile_pool(name="qtm", bufs=1))
        attp = es.enter_context(tc.tile_pool(name="att", bufs=2))
        rsp = es.enter_context(tc.tile_pool(name="rsp", bufs=1))
        spop = es.enter_context(tc.tile_pool(name="spo", bufs=1))
        pkqp = es.enter_context(tc.tile_pool(name="pkq", bufs=1, space="PSUM"))
        pvtp = es.enter_context(tc.tile_pool(name="pvt", bufs=1, space="PSUM"))
        pap = es.enter_context(tc.tile_pool(name="pa", bufs=2, space="PSUM"))
        ps1p = es.enter_context(tc.tile_pool(name="ps1", bufs=1, space="PSUM"))
        pbcp = es.enter_context(tc.tile_pool(name="pbc", bufs=1, space="PSUM"))
        pavp = es.enter_context(tc.tile_pool(name="pav", bufs=2, space="PSUM"))

        spx_tiles = {}

        def fetch_spx(h):
            """Stage pair-packed spatial slice via DMA, then repack to
            window-block with V copies (@4x)."""
            c, h2 = h // 2, h % 2
            t = spkp.tile([128, 4, POS], bf16, tag="spk", name=f"spk{h}")
            xv = x2fl[c][64 * h2:64 * h2 + 64, :, :] \
                .rearrange("p (j t) x -> p t j x", t=2)
            for par in range(2):
                nc.gpsimd.dma_start(out=t[64 * par:64 * par + 64, :, :],
                                    in_=xv[:, par, :, :])
            twb = spxp.tile([128, 4, POS], bf16, tag="spxwb", name=f"spxwb{h}")
            for j in range(4):
                for n2 in range(2):
                    co = 392 * n2
                    for w in range(8):
                        nc.vector.tensor_copy(
                            twb[:, j, co + WN * w:co + WN * (w + 1)]
                            .rearrange("p (x y) -> p x y", x=7),
                            win_ap(t[:, j, co:co + 392], n2, w, spatial=True))
            spx_tiles[h] = twb

        # shared q-conv macro-grid: gutters zeroed once, window interiors
        # overwritten per head by the pack copies
        Gq = qgrp.tile([128, GROWS, GCOLS], bf16, tag="qpad", name="qpad")
        nc.gpsimd.memset(Gq[:], 0.0)

        fetch_spx(0)
        sp_all = spx_tiles[0]
        for h in range(NH):
            c, h2 = h // 2, h % 2
            if h + 1 < NH:
                fetch_spx(h + 1)

            kqt = kqtp.tile([128, 4, POS], bf16, tag="kqt", name=f"kqt{h}")
            k_pk = kpkp.tile([128, 2, POS], bf16, tag="k", name=f"k{h}")
            qstack = qgp.tile([128, POS], bf16, tag="qstack", name=f"qstack{h}")
            qp_pk = kpkp.tile([128, 2, POS], bf16, tag="qp", name=f"qp{h}")
            vt_pk = vtp.tile([128, 4 * 1024], bf16, tag="vt", name=f"vt{h}")

            # ---- A/B: kqv matmuls + evict + repack DMAs ----
            for j in range(4):          # image pairs (2j, 2j+1)
                for n2 in range(2):
                    pkq = pkqp.tile([128, 392], f32, tag="pkq",
                                    name=f"pkq{h}_{j}_{n2}")
                    pvt = pvtp.tile([128, 512], f32, tag="pvt",
                                    name=f"pvt{h}_{j}_{n2}")
                    for t_ in range(2):
                        img = 2 * j + t_
                        ob = 64 * t_
                        rhs_base = prow(img)
                        spi = sp_all[rhs_base:rhs_base + 64, img // 2,
                                     392 * n2:392 * (n2 + 1)]
                        nc.tensor.matmul(
                            pkq[ob:ob + 2 * KD, :],
                            wkq_sb[h][rhs_base:rhs_base + 64, :],
                            spi, start=True, stop=True,
                            tile_position=(rhs_base, ob))
                        for w in range(8):
                            nc.tensor.matmul(
                                pvt[ob:ob + WN, 64 * w:64 * (w + 1)],
                                spi[:, WN * w:WN * (w + 1)],
                                wv_sb[h][rhs_base:rhs_base + 64, :],
                                start=True, stop=True,
                                tile_position=(rhs_base, ob))
                    nc.scalar.activation(kqt[:, j, 392 * n2:392 * (n2 + 1)],
                                         pkq[:], AF.Identity,
                                         bias=bkq_sb[h][:, 0:1])
                    nc.scalar.activation(
                        vt_pk[:, 1024 * j + 512 * n2:1024 * j + 512 * (n2 + 1)],
                        pvt[:], AF.Copy)
                for t_ in range(2):
                    img = 2 * j + t_
                    rb = 64 * t_
                    nc.sync.dma_start(
                        out=k_pk[32 * (img % 4):32 * (img % 4) + KD, img // 4, :],
                        in_=kqt[rb:rb + KD, j, :])
                    nc.sync.dma_start(
                        out=qstack[KD * img:KD * (img + 1), :],
                        in_=kqt[rb + KD:rb + 2 * KD, j, :])

            # ---- C: depthwise conv on stacked q (shared guttered grid) ----
            kk = KS[h]
            qsv = qstack[:].rearrange("p (n s) -> p n s", n=NW)
            for w in range(NW):
                nc.vector.tensor_copy(
                    Gq[:, :, (7 + GUT) * w:(7 + GUT) * w + 7],
                    qsv[:, w, :].rearrange("p (x y) -> p x y", x=7))
            GA = qgap.tile([128, GROWS, GCOLS], bf16, tag="qacc",
                           name=f"qacc{h}")
            warm_ps = pbcp.tile([128, 392], f32, tag="pbc", name=f"warm{h}")

            def tmp_q(i):
                return qtmp.tile([128, GROWS, GCOLS], bf16, tag=f"qtm{i}",
                                 name=f"qtm{h}_{i}")

            def warm_fn(tm, t):
                p_ = kk // 2
                y0, _ = _region(GROWS, t[0] - p_)
                x0, _ = _region(GCOLS, t[1] - p_)
                nc.tensor.matmul(
                    warm_ps[0:2, 0:WN], ones2_sb[:],
                    tm[:, y0, x0:x0 + WN],
                    start=True, stop=True, tile_position=(0, 0))

            emit_conv_grid(nc, kk, dq_w[h], dq_b[h], Gq[:], GA, tmp_q, warm_fn)
            # unpack to window-block layout
            qflat = qgp.tile([128, NW, WN], bf16, tag="qflat", name=f"qflat{h}")
            for w in range(NW):
                nc.vector.tensor_copy(
                    qflat[:, w, :].rearrange("p (x y) -> p x y", x=7),
                    GA[:, :, (7 + GUT) * w:(7 + GUT) * w + 7])
            for img in range(NI):
                nc.sync.dma_start(
                    out=qp_pk[32 * (img % 4):32 * (img % 4) + KD, img // 4, :],
                    in_=qflat[KD * img:KD * (img + 1)].rearrange("q n s -> q (n s)"))

            # ---- D: attention per pair ----
            spn = None
            if h + 1 < NH:
                spn = spp.tile([128, 4, POS], bf16, tag="sp", name=f"sp{h + 1}")
            spo_all = spop.tile([128, 4, POS], bf16, tag="spo", name=f"spo{h}")
            for j in range(4):
                for n2 in range(2):
                    pa = pap.tile([128, 392], f32, tag="pa", name=f"pa{h}_{j}_{n2}")
                    for t_ in range(2):
                        img = 2 * j + t_
                        ob = 64 * t_
                        q0 = 32 * (img % 4)
                        kh = k_pk[q0:q0 + KD, img // 4, :]
                        qh = qp_pk[q0:q0 + KD, img // 4, :]
                        for w in range(8):
                            co_ = 392 * n2 + WN * w
                            nc.tensor.matmul(
                                pa[ob:ob + WN, WN * w:WN * (w + 1)],
                                kh[:, co_:co_ + WN], qh[:, co_:co_ + WN],
                                start=True, stop=False,
                                tile_position=(q0, ob))
                    nc.tensor.matmul(pa[:], iab_sb[:], ab_sb[h][:],
                                     start=False, stop=True,
                                     tile_position=(0, 0))
                    ein = attp.tile([128, 392], bf16, tag="ein",
                                    name=f"ein{h}_{j}_{n2}")
                    nc.scalar.activation(ein[:], pa[:], AF.Exp)
                    ps1 = ps1p.tile([2, 392], f32, tag="ps1",
                                    name=f"ps1{h}_{j}_{n2}")
                    nc.tensor.matmul(ps1[:], ones2_sb[:], ein[:],
                                     start=True, stop=True,
                                     tile_position=(0, 0))
                    rs = rsp.tile([2, 392], f32, tag="rs", name=f"rs{h}_{j}_{n2}")
                    nc.vector.reciprocal_approx_fast(rs[:], ps1[:])
                    pbc = pbcp.tile([128, 392], f32, tag="pbc",
                                    name=f"pbc{h}_{j}_{n2}")
                    nc.tensor.matmul(pbc[:], sel2_sb[:], rs[:],
                                     start=True, stop=True,
                                     tile_position=(0, 0))
                    bc = attp.tile([128, 392], bf16, tag="bc",
                                   name=f"bc{h}_{j}_{n2}")
                    nc.scalar.activation(bc[:], pbc[:], AF.Copy)
                    pav = pavp.tile([128, 392], f32, tag="pav",
                                    name=f"pav{h}_{j}_{n2}")
                    for t_ in range(2):
                        img = 2 * j + t_
                        ob = 64 * t_
                        for w in range(8):
                            wg = 8 * n2 + w
                            nc.tensor.matmul(
                                pav[ob:ob + D, WN * w:WN * (w + 1)],
                                vt_pk[ob:ob + WN,
                                      1024 * j + 64 * wg:1024 * j + 64 * (wg + 1)],
                                ein[ob:ob + WN, WN * w:WN * (w + 1)],
                                start=True, stop=True,
                                tile_position=(ob, ob))
                    co = 392 * n2
                    nc.vector.tensor_tensor(spo_all[:, j, co:co + 392], pav[:],
                                            bc[:], AO.mult)
                    if spn is not None:
                        nc.vector.scalar_tensor_tensor(
                            spn[:, j, co:co + 392],
                            spo_all[:, j, co:co + 392],
                            bv_sb[h][:, 0:1],
                            spx_tiles[h + 1][:, j, co:co + 392],
                            AO.add, AO.add)
            yv = y_sb[c][64 * h2:64 * h2 + 64, :, :] \
                .rearrange("p (j t) x -> p t j x", t=2)
            for t_ in range(2):
                nc.gpsimd.dma_start(out=yv[:, t_, :, :],
                                    in_=spo_all[64 * t_:64 * t_ + 64, :, :])
            sp_all = spn

    # ---------------- P4+P5+P6 fused per image ------------------------------
    # y is window-block; proj output window-block; x3 written spatially (trunk)
    x3fl = x2fl
    x4fl = [xwp.tile([128, NI, POS], bf16, tag=f"wm{c}", name=f"x4_{c}")
            for c in range(4)]
    w1sb1, w2sb1, b1row1 = load_mlp_w(w1T1_d, b1f1_d, w2T1_d, wp)

    def rhs1(k, img, n2):
        return x4fl[k][:, img, 392 * n2:392 * (n2 + 1)]

    with ExitStack() as es:
        hyp = es.enter_context(tc.tile_pool(name="hyp", bufs=2))
        pjrp = es.enter_context(tc.tile_pool(name="pjr", bufs=2))
        dacp = es.enter_context(tc.tile_pool(name="dac1", bufs=3))
        grdp1 = es.enter_context(tc.tile_pool(name="grd1", bufs=1))
        tmp1 = es.enter_context(tc.tile_pool(name="dwt1", bufs=2))
        o5p = es.enter_context(tc.tile_pool(name="o5", bufs=2))
        hp1 = es.enter_context(tc.tile_pool(name="m1h", bufs=2))
        rp1 = es.enter_context(tc.tile_pool(name="m1r", bufs=4))
        ppp = es.enter_context(tc.tile_pool(name="ppp", bufs=2, space="PSUM"))
        psp1 = es.enter_context(tc.tile_pool(name="m1ps", bufs=4, space="PSUM"))
        pop1 = es.enter_context(tc.tile_pool(name="m1po", bufs=2, space="PSUM"))

        g1_set, g12_set = [], []
        for i in range(GN):
            g = grdp1.tile([128, 30, 32], bf16, tag=f"g{i}", name=f"h{i}")
            nc.gpsimd.memset(g[:], 0.0)
            g1_set.append(g)
            g2 = grdp1.tile([128, 30, 32], bf16, tag=f"g2{i}", name=f"h2{i}")
            nc.gpsimd.memset(g2[:], 0.0)
            g12_set.append(g2)

        def outw1(mo, img, n2, po, b2):
            x5 = o5p.tile([128, 392], f32, tag="x5", name=f"x5_{mo}_{img}_{n2}")
            nc.vector.scalar_tensor_tensor(
                x5[:], po[:], b2[:, 0:1],
                x4fl[mo][:, img, 392 * n2:392 * (n2 + 1)], AO.add, AO.add)
            nc.sync.dma_start(
                out=out_d[img, 128 * mo:128 * (mo + 1), 392 * n2:392 * (n2 + 1)],
                in_=x5[:])

        for img in range(NI):
            # P4: hswish(y + yb), proj, x3 = x2 + proj + pjb
            hys = []
            for cb in range(4):
                yv = y_sb[cb][:, img, :]
                nc.vector.tensor_scalar(yv, yv, yb_sb[cb][:, 0:1], None,
                                        AO.add)
                r = pjrp.tile([128, POS], bf16, tag="pr")
                nc.scalar.activation(r[:], yv, AF.Relu,
                                     scale=acts[:, 0:1], bias=acth[:, 0:1])
                hy = hyp.tile([128, POS], bf16, tag=f"hy{cb}", name=f"hy{cb}_{img}")
                nc.vector.scalar_tensor_tensor(hy[:], r[:], 1.0, yv,
                                               AO.min, AO.mult)
                hys.append(hy)
            for mo in range(4):
                for n2 in range(2):
                    pp = ppp.tile([128, 392], f32, tag="pp")
                    for k in range(4):
                        nc.tensor.matmul(pp[:], pj_sb[k][:, 128 * mo:128 * (mo + 1)],
                                         hys[k][:, 392 * n2:392 * (n2 + 1)],
                                         start=(k == 0), stop=(k == 3))
                    ov = x2fl[mo][:, img, 392 * n2:392 * (n2 + 1)]
                    for w in range(8):
                        nc.vector.scalar_tensor_tensor(
                            win_ap(ov, n2, w, spatial=True),
                            pp[:, WN * w:WN * (w + 1)]
                            .rearrange("p (x y) -> p x y", x=7),
                            pjb_sb[mo][:, 0:1],
                            win_ap(ov, n2, w, spatial=True),
                            AO.add, AO.add)
            # P5: dw1 units for this image (spatial trunk -> x4 spatial)
            for cb in range(4):
                u = (4 * img + cb) % GN
                g, g2 = g1_set[u], g12_set[u]
                nc.scalar.activation(
                    g[:, 1:29, 1:29],
                    x3fl[cb][:, img, :].rearrange("p (h w) -> p h w", h=RES),
                    AF.Copy)
                nc.scalar.activation(
                    g2[:].rearrange("p h w -> p (h w)")[:, 0:959],
                    g[:].rearrange("p h w -> p (h w)")[:, 1:960], AF.Copy)
                acc = dacp.tile([128, 28, 32], bf16, tag="acc",
                                name=f"a1_{cb}_{img}")

                def tmp_fn(i):
                    return tmp1.tile([128, 896], bf16, tag=f"tm{i}",
                                     name=f"tm1_{cb}_{img}_{i}")

                def final_fn():
                    nc.vector.tensor_tensor(
                        x4fl[cb][:, img, :].rearrange("p (h w) -> p h w", h=RES),
                        acc[:, :, 0:28], g[:, 1:29, 1:29], AO.add)

                emit_conv_dw(nc, dw_w["dw1"][cb], dw_b["dw1"][cb], g, g2,
                             acc, tmp_fn, final_fn, SPLIT_DW1)
            # P6: MLP1 for this image
            mlp_img((hp1, rp1, psp1, pop1), img, w1sb1, w2sb1, b1row1, b2sb1,
                    rhs1, outw1, "m1")

    xw_cm.__exit__(None, None, None)
    big_cm.__exit__(None, None, None)
    wp_cm.__exit__(None, None, None)


# ---------------------------------------------------------------------------
# host-side input preprocessing
# ---------------------------------------------------------------------------

def prep_weights(inp):
    def taps(w):  # [C,1,k,k] -> [C, k*k]
        return w.reshape(w.shape[0], -1).astype(np.float32)

    m = {}
    dwpk = np.zeros((128, 80), np.float32)
    for ci in range(4):
        dwpk[:, 10 * ci:10 * ci + 9] = taps(inp["dw0_w"])[128 * ci:128 * (ci + 1)]
        dwpk[:, 10 * ci + 9] = inp["dw0_b"][128 * ci:128 * (ci + 1)]
        dwpk[:, 40 + 10 * ci:40 + 10 * ci + 9] = \
            taps(inp["dw1_w"])[128 * ci:128 * (ci + 1)]
        dwpk[:, 40 + 10 * ci + 9] = inp["dw1_b"][128 * ci:128 * (ci + 1)]
    m["dwpk"] = dwpk
    m["w1T0"] = np.ascontiguousarray(inp["ffn0_w1"].T).astype(ml_dtypes.bfloat16)
    m["b1f0"] = inp["ffn0_b1"].astype(ml_dtypes.bfloat16)
    m["w2T0"] = np.ascontiguousarray(inp["ffn0_w2"].T).astype(ml_dtypes.bfloat16)

    qkv_w, qkv_b = inp["qkv_w"], inp["qkv_b"]
    wkqT = np.empty((NH, D, 2 * KD), np.float32)
    bkq = np.empty((NH, 2 * KD), np.float32)
    wvT = np.empty((NH, D, D), np.float32)
    bv = np.empty((NH, D), np.float32)
    for h in range(NH):
        W = qkv_w[h]  # [96, 64]
        wkqT[h, :, 0:KD] = W[KD:2 * KD].T       # k
        wkqT[h, :, KD:2 * KD] = W[0:KD].T       # q
        bkq[h, 0:KD] = qkv_b[h, KD:2 * KD]
        bkq[h, KD:2 * KD] = qkv_b[h, 0:KD]
        wvT[h] = W[2 * KD:].T
        bv[h] = qkv_b[h, 2 * KD:]
    # packed: [128, NH*32] kq weights (row halves duplicated)
    akq = np.empty((128, NH * 2 * KD), np.float32)
    awv = np.empty((128, NH * D), np.float32)
    abias = np.zeros((128, 2 * NH), np.float32)
    for h in range(NH):
        akq[0:64, 32 * h:32 * h + 32] = wkqT[h]
        akq[64:128, 32 * h:32 * h + 32] = wkqT[h]
        awv[0:64, 64 * h:64 * h + 64] = wvT[h]
        awv[64:128, 64 * h:64 * h + 64] = wvT[h]
        abias[0:32, 2 * h] = bkq[h]
        abias[64:96, 2 * h] = bkq[h]
        abias[0:64, 2 * h + 1] = bv[h]
        abias[64:128, 2 * h + 1] = bv[h]
    m["attkq"] = akq.astype(ml_dtypes.bfloat16)
    m["attwv"] = awv.astype(ml_dtypes.bfloat16)
    m["attbias"] = abias

    dwq_ws = [inp["dwq_w7"], inp["dwq_w5"]] + [inp["dwq_w3"][i] for i in range(6)]
    dwq_bs = [inp["dwq_b7"], inp["dwq_b5"]] + [inp["dwq_b3"][i] for i in range(6)]
    dwqw = np.zeros((128, NH * 50), np.float32)
    for h in range(NH):
        t = taps(dwq_ws[h]) * SCALE
        nt = t.shape[1]
        for i in range(NI):
            dwqw[KD * i:KD * (i + 1), 50 * h:50 * h + nt] = t
            dwqw[KD * i:KD * (i + 1), 50 * h + 49] = dwq_bs[h] * SCALE
    m["dwqw"] = dwqw

    ab = inp["attn_bias"][:, BIAS_IDX]       # [NH, 49, 49]
    ab = np.tile(ab, (1, 1, 8))              # [NH, 49, 392]
    m["ab"] = ab.transpose(1, 0, 2).reshape(WN, NH * 392).copy() \
        .astype(ml_dtypes.bfloat16)

    iab = np.zeros((WN, 128), np.float32)
    for i in range(WN):
        iab[i, i] = 1.0
        iab[i, 64 + i] = 1.0
    m["iab"] = iab.astype(ml_dtypes.bfloat16)
    ones2 = np.zeros((128, 2), np.float32)
    ones2[0:WN, 0] = 1.0
    ones2[64:64 + WN, 1] = 1.0
    m["ones2"] = ones2.astype(ml_dtypes.bfloat16)
    sel2 = np.zeros((2, 128), np.float32)
    sel2[0, 0:64] = 1.0
    sel2[1, 64:128] = 1.0
    m["sel2"] = sel2

    m["projT"] = np.ascontiguousarray(inp["proj_w"].T).astype(ml_dtypes.bfloat16)
    bpk = np.zeros((128, 16), np.float32)
    for ci in range(4):
        bpk[:, ci] = inp["proj_b"][128 * ci:128 * (ci + 1)]
        bpk[:, 4 + ci] = inp["ffn0_b2"][128 * ci:128 * (ci + 1)]
        bpk[:, 8 + ci] = inp["ffn1_b2"][128 * ci:128 * (ci + 1)]
        bpk[:, 12 + ci] = bv.reshape(ED)[128 * ci:128 * (ci + 1)]
    m["bpk"] = bpk

    m["w1T1"] = np.ascontiguousarray(inp["ffn1_w1"].T).astype(ml_dtypes.bfloat16)
    m["b1f1"] = inp["ffn1_b1"].astype(ml_dtypes.bfloat16)
    m["w2T1"] = np.ascontiguousarray(inp["ffn1_w2"].T).astype(ml_dtypes.bfloat16)
    return m


@functools.lru_cache(maxsize=1)
def _cached_program():
    return build_program()


def _run(inputs, trace=False, **kw):
    nc = _cached_program()
    wm = prep_weights(inputs)
    x = np.asarray(inputs["x"], dtype=np.float32).reshape(64, ED, POS)
    x = x.astype(ml_dtypes.bfloat16)
    in_maps = []
    for core in range(NCORES):
        im = dict(wm)
        im["x"] = np.ascontiguousarray(x[NI * core:NI * (core + 1)])
        in_maps.append(im)
    res = bass_utils.run_bass_kernel_spmd(nc, in_maps, list(range(NCORES)),
                                          trace=trace, **kw)
    out = np.concatenate([r["out"] for r in res.results], axis=0)
    return out.reshape(64, ED, RES, RES).astype(np.float32), res


def kernel(**inputs):
    out, _ = _run(inputs)
    return out


# revision 72
# speedup vs baseline: 1.0080x; 1.0080x over previous
"""Trainium2 Bass kernel for nn_BasicBlock (EfficientViT-style block), v3.

Data-parallel over 8 NeuronCores: batch 64 -> 8 images/core.
SBUF-resident bf16 spatial trunk, no DRAM intermediates.
Depthwise convs via valid-region shifted views, split across V/S/G engines.
Per-core program: dw0 -> MLP0 -> cascaded window attention -> proj -> dw1 -> MLP1.
"""
import itertools
import functools
from contextlib import ExitStack
import numpy as np
import ml_dtypes

import concourse.bass as bass
import concourse.mybir as mybir
import concourse.tile as tile
from concourse import bacc
from concourse import bass_utils

f32 = mybir.dt.float32
bf16 = mybir.dt.bfloat16
AO = mybir.AluOpType
AF = mybir.ActivationFunctionType

ED, KD, NH, AR = 512, 16, 8, 4
D = AR * KD            # 64
DH = D * NH            # 512
RES, WS = 28, 7
SCALE = KD ** -0.5
KS = [7, 5, 3, 3, 3, 3, 3, 3]
NI = 8                 # images per core
NCORES = 8
POS = RES * RES        # 784
NW = 16                # windows per image
WN = WS * WS           # 49


def _bias_idx(ws):
    pts = list(itertools.product(range(ws), range(ws)))
    offs, idxs = {}, []
    for p1 in pts:
        for p2 in pts:
            o = (abs(p1[0] - p2[0]), abs(p1[1] - p2[1]))
            if o not in offs:
                offs[o] = len(offs)
            idxs.append(offs[o])
    return np.array(idxs, dtype=np.int32).reshape(ws * ws, ws * ws), len(offs)


BIAS_IDX, N_OFFS = _bias_idx(WS)


# ---------------------------------------------------------------------------
# conv planning (engine split)
# ---------------------------------------------------------------------------

def _region(sz, d):
    """1D dst range [y0,y1) for shift d (src index = dst + d)."""
    return max(0, -d), sz - max(0, d)


def _plan_taps1(k, cols, fold_engs, desc, g0):
    """Greedy engine split for k*k taps. cols(t) -> per-tap column count.

    V taps: STT accumulate in place. S taps: ACT into a cycling tmp slot,
    folded into the accumulator by a V tensor_tensor (@2x) or a G
    tensor_tensor (slow but off the critical engines).
    Returns ((v_taps, s_taps), wall); s_taps entries are (tap, fold_engine).
    """
    p = k // 2
    center = (p, p)
    ccols = cols(center)
    busy = {"V": ccols * 0.26 + 105 + ccols * 0.52 + 105, "S": 0.0, "G": g0}
    v_taps, s_taps = [], []
    order = sorted([t for t in itertools.product(range(k), range(k))
                    if t != center], key=lambda t: (-cols(t) if desc else cols(t)))
    for t in order:
        c = cols(t)
        cand = {}
        nb = dict(busy)
        nb["V"] = busy["V"] + c * 1.042 + 105
        cand["V"] = max(nb.values())
        if "V" in fold_engs:
            nb = dict(busy)
            nb["S"] = busy["S"] + c * 0.833 + 217
            nb["V"] = busy["V"] + c * 0.52 + 105
            cand["SV"] = max(nb.values())
        if "G" in fold_engs:
            nb = dict(busy)
            nb["S"] = busy["S"] + c * 0.833 + 217
            nb["G"] = busy["G"] + c * 1.98 + 156
            cand["SG"] = max(nb.values())
        eng = min(cand, key=lambda e: cand[e])
        if eng == "V":
            v_taps.append(t)
            busy["V"] += c * 1.042 + 105
        elif eng == "SV":
            s_taps.append((t, "V"))
            busy["S"] += c * 0.833 + 217
            busy["V"] += c * 0.52 + 105
        else:
            s_taps.append((t, "G"))
            busy["S"] += c * 0.833 + 217
            busy["G"] += c * 1.98 + 156
    return (v_taps, s_taps), max(busy.values())


def plan_taps(k, cols, g0=0.0):
    best = None
    for folds in (("V",), ("G",), ("V", "G")):
        for desc in (True, False):
            plan, wall = _plan_taps1(k, cols, folds, desc, g0)
            if best is None or wall < best[1]:
                best = (plan, wall)
    return best[0]


def _cols_sp(t, k):
    dy, dx = t
    p = k // 2
    return (RES - abs(dy - p)) * (RES - abs(dx - p))


PLAN_DW = plan_taps(3, lambda t: _cols_sp(t, 3), g0=1000.0)

# shared 1x16 guttered macro-grid for the per-head q convs: 7 rows,
# 16 windows of 7 cols separated by 3-col gutters (max pad of any head)
GUT = 3
GCOLS = 16 * 7 + 15 * GUT       # 157
GROWS = 7


def _cols_gq(t, k):
    dy, dx = t
    p = k // 2
    return (GROWS - abs(dy - p)) * (GCOLS - abs(dx - p))


PLAN_DWQ = {_k: plan_taps(_k, lambda t: _cols_gq(t, _k), g0=1000.0)
            for _k in (3, 5, 7)}
NSLOT = 4


# dw conv engine split: V taps (flat STT), S taps (flat ACT tmps),
# folds mostly on V (flat TT @2x), FOLD_G set folded on GpSimd
SPLIT_DW0 = ([(0, 0), (0, 1), (0, 2), (1, 0), (1, 2)],
             [(2, 0), (2, 1), (2, 2)], set())
SPLIT_DW1 = SPLIT_DW0


def emit_conv_dw(nc, wt, bt, g, g2, acc, tmp_fn, final_fn, split):
    """3x3 depthwise conv, flat shifted taps over a [128,30,32] padded grid.

    g2 is g shifted left one column (keeps even element offsets for the
    odd-dx taps). acc is [128, 28, 32]; flat cols 0..891 hold the interior.
    """
    DW_V_TAPS, DW_S_TAPS, DW_FOLD_G = split
    gf = g[:].rearrange("p h w -> p (h w)")
    g2f = g2[:].rearrange("p h w -> p (h w)") if g2 is not None else None
    af = acc[:].rearrange("p h w -> p (h w)")

    def src(t):
        dy, dx = t
        if g2f is not None and dx == 1:
            return g2f[:, 32 * dy:32 * dy + 892]
        return gf[:, 32 * dy + dx:32 * dy + dx + 892]

    def w_(t):
        return wt[:, (t[0] * 3 + t[1]):(t[0] * 3 + t[1]) + 1]

    nc.vector.tensor_scalar(af[:, 0:892], src((1, 1)), w_((1, 1)),
                            bt[:, 0:1], AO.mult, AO.add)
    vq = list(DW_V_TAPS)
    for i, t in enumerate(DW_S_TAPS):
        tm = tmp_fn(i % NSLOT)
        tf = tm[:].rearrange("p h w -> p (h w)") if len(tm.shape) == 3 else tm[:]
        nc.scalar.activation(tf[:, 0:892], src(t), AF.Identity, scale=w_(t))
        if t in DW_FOLD_G:
            nc.gpsimd.tensor_tensor(af[:, 0:892], tf[:, 0:892], af[:, 0:892],
                                    AO.add)
        else:
            nc.vector.tensor_tensor(af[:, 0:892], tf[:, 0:892], af[:, 0:892],
                                    AO.add)
        if vq:
            t2 = vq.pop(0)
            nc.vector.scalar_tensor_tensor(af[:, 0:892], src(t2), w_(t2),
                                           af[:, 0:892], AO.mult, AO.add)
    for t2 in vq:
        nc.vector.scalar_tensor_tensor(af[:, 0:892], src(t2), w_(t2),
                                       af[:, 0:892], AO.mult, AO.add)
    final_fn()


def emit_conv_grid(nc, k, wt, bt, G, GA, tmp_fn, warm_fn):
    """k*k depthwise conv on the shared guttered macro-grid [128, 7, 157]."""
    p = k // 2
    v_taps, s_taps = PLAN_DWQ[k]

    def dst_v(base, t):
        dy, dx = t
        y0, y1 = _region(GROWS, dy - p)
        x0, x1 = _region(GCOLS, dx - p)
        return base[:, y0:y1, x0:x1]

    def src_v(t):
        dy, dx = t
        y0, y1 = _region(GROWS, dy - p)
        x0, x1 = _region(GCOLS, dx - p)
        return G[:, y0 + dy - p:y1 + dy - p, x0 + dx - p:x1 + dx - p]

    def wcol(t):
        return t[0] * k + t[1]

    nc.vector.tensor_scalar(dst_v(GA[:], (p, p)), src_v((p, p)),
                            wt[:, wcol((p, p)):wcol((p, p)) + 1],
                            bt[:, 0:1], AO.mult, AO.add)
    vq = list(v_taps)
    for i, (t, feng) in enumerate(s_taps):
        tm = tmp_fn(i % NSLOT)
        nc.scalar.activation(dst_v(tm[:], t), src_v(t), AF.Identity,
                             scale=wt[:, wcol(t):wcol(t) + 1])
        av, tv = dst_v(GA[:], t), dst_v(tm[:], t)
        if feng == "G":
            nc.gpsimd.tensor_tensor(av, tv, av, AO.add)
        else:
            nc.vector.tensor_tensor(av, tv, av, AO.add)
        if warm_fn is not None and i % 2 == 0:
            warm_fn(tm, t)
        if vq:
            t2 = vq.pop(0)
            nc.vector.scalar_tensor_tensor(dst_v(GA[:], t2), src_v(t2),
                                           wt[:, wcol(t2):wcol(t2) + 1],
                                           dst_v(GA[:], t2), AO.mult, AO.add)
    for t2 in vq:
        nc.vector.scalar_tensor_tensor(dst_v(GA[:], t2), src_v(t2),
                                       wt[:, wcol(t2):wcol(t2) + 1],
                                       dst_v(GA[:], t2), AO.mult, AO.add)


# ---------------------------------------------------------------------------
# program builder
# ---------------------------------------------------------------------------

def build_program():
    nc = bacc.Bacc("TRN2", target_bir_lowering=False, debug=False,
                   enable_asserts=False, num_devices=NCORES)

    def din(name, shape, dt=f32):
        return nc.dram_tensor(name, list(shape), dt, kind="ExternalInput").ap()

    x_d = din("x", [NI, ED, POS], bf16)
    dwpk_d = din("dwpk", [128, 80])
    w1T0_d = din("w1T0", [ED, 2 * ED], bf16)
    b1f0_d = din("b1f0", [2 * ED], bf16)
    w2T0_d = din("w2T0", [2 * ED, ED], bf16)
    attkq_d = din("attkq", [128, NH * 2 * KD], bf16)
    attwv_d = din("attwv", [128, NH * D], bf16)
    attbias_d = din("attbias", [128, 2 * NH])
    dwqw_d = din("dwqw", [128, NH * 50])
    ab_d = din("ab", [WN, NH * 392], bf16)
    iab_d = din("iab", [WN, 128], bf16)
    ones2_d = din("ones2", [128, 2], bf16)
    sel2_d = din("sel2", [2, 128])
    projT_d = din("projT", [DH, ED], bf16)
    bpk_d = din("bpk", [128, 16])
    w1T1_d = din("w1T1", [ED, 2 * ED], bf16)
    b1f1_d = din("b1f1", [2 * ED], bf16)
    w2T1_d = din("w2T1", [2 * ED, ED], bf16)

    out_d = nc.dram_tensor("out", [NI, ED, POS], f32, kind="ExternalOutput").ap()

    with tile.TileContext(nc) as tc:
        _body(tc, nc, x_d, dwpk_d, w1T0_d, b1f0_d, w2T0_d,
              attkq_d, attwv_d, attbias_d, dwqw_d, ab_d,
              iab_d, ones2_d, sel2_d,
              projT_d, bpk_d,
              w1T1_d, b1f1_d, w2T1_d, out_d)

    nc.compile()
    return nc


def win_ap(ap392, n2, w, spatial):
    """Per-window [*, 49] AP from a 392-col half. spatial: 3D 7x7 slice of
    the 14x28 spatial half; else dense 49-block (window-block layout)."""
    if spatial:
        al, b = w // 4, w % 4
        v = ap392.rearrange("p (h x) -> p h x", h=14)
        return v[:, 7 * al:7 * al + 7, 7 * b:7 * b + 7]
    return ap392[:, WN * w:WN * (w + 1)]


def _body(tc, nc, x_d, dwpk_d, w1T0_d, b1f0_d, w2T0_d,
          attkq_d, attwv_d, attbias_d, dwqw_d, ab_d,
          iab_d, ones2_d, sel2_d,
          projT_d, bpk_d,
          w1T1_d, b1f1_d, w2T1_d, out_d):

    # ---------------- persistent pools -------------------------------------
    wp_cm = tc.tile_pool(name="wp", bufs=1)
    wp = wp_cm.__enter__()
    big_cm = tc.tile_pool(name="big", bufs=1)
    big = big_cm.__enter__()
    xw_cm = tc.tile_pool(name="xw", bufs=1)
    xwp = xw_cm.__enter__()

    def load_mlp_w(w1T_dram, b1_dram, w2T_dram, b2_dram, pool):
        w1sb = []
        for k in range(4):
            w = pool.tile([128, 2 * ED], bf16, tag=f"w1_{k}")
            nc.sync.dma_start(out=w, in_=w1T_dram[128 * k:128 * (k + 1), :])
            w1sb.append(w)
        w2sb = []
        for k in range(8):
            w = pool.tile([128, ED], bf16, tag=f"w2_{k}")
            nc.sync.dma_start(out=w, in_=w2T_dram[128 * k:128 * (k + 1), :])
            w2sb.append(w)
        b1row = pool.tile([1, 2 * ED], bf16, tag="b1row")
        nc.sync.dma_start(out=b1row, in_=b1_dram.unsqueeze(0))
        b2sb = []
        for m in range(4):
            b = pool.tile([128, 1], f32, tag=f"b2_{m}")
            nc.sync.dma_start(out=b, in_=b2_dram[128 * m:128 * (m + 1)].unsqueeze(1))
            b2sb.append(b)
        return w1sb, w2sb, b1row, b2sb

    # dw weights
    dw_w, dw_b = {}, {}
    for nm, wd, bd in (("dw0", dw0w_d, dw0b_d), ("dw1", dw1w_d, dw1b_d)):
        ws_, bs_ = [], []
        for c in range(4):
            w = wp.tile([128, 9], f32, tag=f"{nm}w{c}")
            nc.sync.dma_start(out=w, in_=wd[c])
            b = wp.tile([128, 1], f32, tag=f"{nm}b{c}")
            nc.sync.dma_start(out=b, in_=bd[c].unsqueeze(1))
            ws_.append(w)
            bs_.append(b)
        dw_w[nm], dw_b[nm] = ws_, bs_

    ones392 = wp.tile([1, 392], bf16, tag="ones392")
    nc.vector.memset(ones392, 1.0)
    acth = wp.tile([128, 1], f32, tag="acth")
    nc.vector.memset(acth, 0.5)
    acts = wp.tile([128, 1], f32, tag="acts")
    nc.vector.memset(acts, 1.0 / 6.0)

    # attention weights
    wkq_sb, bkq_sb, wv_sb, bv_sb, dq_w, dq_b, ab_sb = [], [], [], [], [], [], []
    for h in range(NH):
        t = wp.tile([128, 2 * KD], bf16, tag=f"wkq{h}")
        nc.sync.dma_start(out=t[0:64, :], in_=wkqT_d[h])
        nc.sync.dma_start(out=t[64:128, :], in_=wkqT_d[h])
        wkq_sb.append(t)
        t = wp.tile([128, 1], f32, tag=f"bkq{h}")
        nc.sync.dma_start(out=t[0:32, :], in_=bkq_d[h].unsqueeze(1))
        nc.sync.dma_start(out=t[64:96, :], in_=bkq_d[h].unsqueeze(1))
        bkq_sb.append(t)
        t = wp.tile([128, D], bf16, tag=f"wv{h}")
        nc.sync.dma_start(out=t[0:64, :], in_=wvT_d[h])
        nc.sync.dma_start(out=t[64:128, :], in_=wvT_d[h])
        wv_sb.append(t)
        t = wp.tile([128, 1], f32, tag=f"bv{h}")
        nc.sync.dma_start(out=t[0:64, :], in_=bv_d[h].unsqueeze(1))
        nc.sync.dma_start(out=t[64:128, :], in_=bv_d[h].unsqueeze(1))
        bv_sb.append(t)

        t = wp.tile([128, 49], f32, tag=f"dqw{h}")
        nc.sync.dma_start(out=t, in_=dwqw_d[h])
        dq_w.append(t)
        t = wp.tile([128, 1], f32, tag=f"dqb{h}")
        nc.sync.dma_start(out=t, in_=dwqb_d[h].unsqueeze(1))
        dq_b.append(t)
        t = wp.tile([WN, 392], bf16, tag=f"ab{h}")
        nc.sync.dma_start(out=t, in_=ab_d[h])
        ab_sb.append(t)
    iab_sb = wp.tile([WN, 128], bf16, tag="iab")
    nc.sync.dma_start(out=iab_sb, in_=iab_d)
    ones2_sb = wp.tile([128, 2], bf16, tag="ones2")
    nc.sync.dma_start(out=ones2_sb, in_=ones2_d)
    sel2_sb = wp.tile([2, 128], f32, tag="sel2")
    nc.sync.dma_start(out=sel2_sb, in_=sel2_d)

    # proj
    pj_sb = []
    for k in range(4):
        w = wp.tile([128, ED], bf16, tag=f"pj{k}")
        nc.sync.dma_start(out=w, in_=projT_d[128 * k:128 * (k + 1), :])
        pj_sb.append(w)
    pjb_sb, yb_sb = [], []
    for m in range(4):
        b = wp.tile([128, 1], f32, tag=f"pjb{m}")
        nc.sync.dma_start(out=b, in_=projb_d[128 * m:128 * (m + 1)].unsqueeze(1))
        pjb_sb.append(b)
        b = wp.tile([128, 1], f32, tag=f"ybt{m}")
        nc.sync.dma_start(out=b, in_=yb_d[128 * m:128 * (m + 1)].unsqueeze(1))
        yb_sb.append(b)

    w1sb0, w2sb0, b1row0, b2sb0 = load_mlp_w(w1T0_d, b1f0_d, w2T0_d, b2f0_d, wp)

    # ---------------- MLP per-image emitter --------------------------------
    def mlp_img(pools, img, w1sb, w2sb, b1row, b2sb, rhs_getter, out_writer, name):
        hp, rp, psp, pop = pools
        hs = []
        for m in range(8):
            h = hp.tile([128, POS], bf16, tag=f"h{m}", name=f"{name}h{m}_{img}")
            hs.append(h)
        for m in range(8):
            for n2 in range(2):
                ph = psp.tile([128, 392], f32, tag="ph")
                for k in range(4):
                    nc.tensor.matmul(
                        ph[:], w1sb[k][:, 128 * m:128 * (m + 1)],
                        rhs_getter(k, img, n2),
                        start=(k == 0), stop=False)
                nc.tensor.matmul(
                    ph[:], b1row[:, 128 * m:128 * (m + 1)],
                    ones392[:], start=False, stop=True)
                r = rp.tile([128, 392], bf16, tag="relu")
                nc.scalar.activation(r[:], ph[:], AF.Relu,
                                     scale=acts[:, 0:1], bias=acth[:, 0:1])
                nc.vector.scalar_tensor_tensor(
                    hs[m][:, 392 * n2:392 * (n2 + 1)], r[:], 1.0,
                    ph[:], AO.min, AO.mult)
        for mo in range(4):
            for n2 in range(2):
                po = pop.tile([128, 392], f32, tag="po")
                for k in range(8):
                    nc.tensor.matmul(
                        po[:], w2sb[k][:, 128 * mo:128 * (mo + 1)],
                        hs[k][:, 392 * n2:392 * (n2 + 1)],
                        start=(k == 0), stop=(k == 7))
                out_writer(mo, img, n2, po, b2sb[mo])

    # ---------------- P0+P1: input DMA + dw0 + residual -> x1 (spatial) -----
    x1fl = [big.tile([128, NI, POS], bf16, tag=f"fl{c}", name=f"x1_{c}")
            for c in range(4)]

    def rhs0(k, img, n2):
        return x1fl[k][:, img, 392 * n2:392 * (n2 + 1)]

    def outw0(mo, img, n2, po, b2):
        ov = x1fl[mo][:, img, 392 * n2:392 * (n2 + 1)]
        nc.vector.scalar_tensor_tensor(ov, po[:], b2[:, 0:1], ov, AO.add, AO.add)

    GN = 3
    with tc.tile_pool(name="grd", bufs=1) as grdp, \
         tc.tile_pool(name="dac", bufs=3) as dacp, \
         tc.tile_pool(name="dwt0", bufs=2) as tmp0, \
         tc.tile_pool(name="m0h", bufs=2) as hp0, \
         tc.tile_pool(name="m0r", bufs=4) as rp0, \
         tc.tile_pool(name="m0ps", bufs=4, space="PSUM") as psp0, \
         tc.tile_pool(name="m0po", bufs=2, space="PSUM") as pop0:
        g_set, g2_set = [], []
        for i in range(GN):
            g = grdp.tile([128, 30, 32], bf16, tag=f"g{i}", name=f"g{i}")
            nc.gpsimd.memset(g[:], 0.0)
            g_set.append(g)
            g2 = grdp.tile([128, 30, 32], bf16, tag=f"g2{i}", name=f"g2{i}")
            nc.gpsimd.memset(g2[:], 0.0)
            g2_set.append(g2)
        for img in range(NI):
            for c in range(4):
                u = (4 * img + c) % GN
                g, g2 = g_set[u], g2_set[u]
                nc.scalar.dma_start(
                    out=g[:, 1:29, 1:29],
                    in_=x_d[img, 128 * c:128 * (c + 1), :]
                    .rearrange("p (h w) -> p h w", h=RES))
                nc.scalar.activation(
                    g2[:].rearrange("p h w -> p (h w)")[:, 0:959],
                    g[:].rearrange("p h w -> p (h w)")[:, 1:960], AF.Copy)
                acc = dacp.tile([128, 28, 32], bf16, tag="acc",
                                name=f"a0_{c}_{img}")

                def tmp_fn(i):
                    return tmp0.tile([128, 896], bf16, tag=f"tm{i}",
                                     name=f"tm0_{c}_{img}_{i}")

                def final_fn():
                    nc.vector.tensor_tensor(
                        x1fl[c][:, img, :].rearrange("p (h w) -> p h w", h=RES),
                        acc[:, :, 0:28], g[:, 1:29, 1:29], AO.add)

                emit_conv_dw(nc, dw_w["dw0"][c], dw_b["dw0"][c], g, g2,
                             acc, tmp_fn, final_fn, SPLIT_DW0)
            mlp_img((hp0, rp0, psp0, pop0), img, w1sb0, w2sb0, b1row0, b2sb0,
                    rhs0, outw0, "m0")

    x2fl = x1fl   # trunk now holds x2 (spatial, bf16)

    # ---------------- P3: cascaded attention -> y_sb ------------------------
    # y in window-block layout: y_sb[c][64*h2+d, img, 49*w + pos]
    y_sb = [xwp.tile([128, NI, POS], bf16, tag=f"wm{c}", name=f"y_{c}")
            for c in range(4)]

    def prow(i):
        return 64 * (i % 2)

    with ExitStack() as es:
        spkp = es.enter_context(tc.tile_pool(name="spk", bufs=1))
        spxp = es.enter_context(tc.tile_pool(name="spx", bufs=2))
        spp = es.enter_context(tc.tile_pool(name="sp", bufs=1))
        kqtp = es.enter_context(tc.tile_pool(name="kqt", bufs=1))
        kpkp = es.enter_context(tc.tile_pool(name="kpk", bufs=1))
        vtp = es.enter_context(tc.tile_pool(name="vt", bufs=1))
        qgp = es.enter_context(tc.tile_pool(name="qg", bufs=1))
        qgrp = es.enter_context(tc.tile_pool(name="qgr", bufs=1))
        qgap = es.enter_context(tc.tile_pool(name="qga", bufs=1))
        qtmp = es.enter_context(tc.tile_pool(name="qtm", bufs=1))
        attp = es.enter_context(tc.tile_pool(name="att", bufs=2))
        rsp = es.enter_context(tc.tile_pool(name="rsp", bufs=1))
        spop = es.enter_context(tc.tile_pool(name="spo", bufs=1))
        pkqp = es.enter_context(tc.tile_pool(name="pkq", bufs=1, space="PSUM"))
        pvtp = es.enter_context(tc.tile_pool(name="pvt", bufs=1, space="PSUM"))
        pap = es.enter_context(tc.tile_pool(name="pa", bufs=2, space="PSUM"))
        ps1p = es.enter_context(tc.tile_pool(name="ps1", bufs=1, space="PSUM"))
        pbcp = es.enter_context(tc.tile_pool(name="pbc", bufs=1, space="PSUM"))
        pavp = es.enter_context(tc.tile_pool(name="pav", bufs=2, space="PSUM"))

        spx_tiles = {}

        def fetch_spx(h):
            """Stage pair-packed spatial slice via DMA, then repack to
            window-block with V copies (@4x)."""
            c, h2 = h // 2, h % 2
            t = spkp.tile([128, 4, POS], bf16, tag="spk", name=f"spk{h}")
            xv = x2fl[c][64 * h2:64 * h2 + 64, :, :] \
                .rearrange("p (j t) x -> p t j x", t=2)
            for par in range(2):
                nc.gpsimd.dma_start(out=t[64 * par:64 * par + 64, :, :],
                                    in_=xv[:, par, :, :])
            twb = spxp.tile([128, 4, POS], bf16, tag="spxwb", name=f"spxwb{h}")
            for j in range(4):
                for n2 in range(2):
                    co = 392 * n2
                    for w in range(8):
                        nc.vector.tensor_copy(
                            twb[:, j, co + WN * w:co + WN * (w + 1)]
                            .rearrange("p (x y) -> p x y", x=7),
                            win_ap(t[:, j, co:co + 392], n2, w, spatial=True))
            spx_tiles[h] = twb

        # shared q-conv macro-grid: gutters zeroed once, window interiors
        # overwritten per head by the pack copies
        Gq = qgrp.tile([128, GROWS, GCOLS], bf16, tag="qpad", name="qpad")
        nc.gpsimd.memset(Gq[:], 0.0)

        fetch_spx(0)
        sp_all = spx_tiles[0]
        for h in range(NH):
            c, h2 = h // 2, h % 2
            if h + 1 < NH:
                fetch_spx(h + 1)

            kqt = kqtp.tile([128, 4, POS], bf16, tag="kqt", name=f"kqt{h}")
            k_pk = kpkp.tile([128, 2, POS], bf16, tag="k", name=f"k{h}")
            qstack = qgp.tile([128, POS], bf16, tag="qstack", name=f"qstack{h}")
            qp_pk = kpkp.tile([128, 2, POS], bf16, tag="qp", name=f"qp{h}")
            vt_pk = vtp.tile([128, 4 * 1024], bf16, tag="vt", name=f"vt{h}")

            # ---- A/B: kqv matmuls + evict + repack DMAs ----
            for j in range(4):          # image pairs (2j, 2j+1)
                for n2 in range(2):
                    pkq = pkqp.tile([128, 392], f32, tag="pkq",
                                    name=f"pkq{h}_{j}_{n2}")
                    pvt = pvtp.tile([128, 512], f32, tag="pvt",
                                    name=f"pvt{h}_{j}_{n2}")
                    for t_ in range(2):
                        img = 2 * j + t_
                        ob = 64 * t_
                        rhs_base = prow(img)
                        spi = sp_all[rhs_base:rhs_base + 64, img // 2,
                                     392 * n2:392 * (n2 + 1)]
                        nc.tensor.matmul(
                            pkq[ob:ob + 2 * KD, :],
                            wkq_sb[h][rhs_base:rhs_base + 64, :],
                            spi, start=True, stop=True,
                            tile_position=(rhs_base, ob))
                        for w in range(8):
                            nc.tensor.matmul(
                                pvt[ob:ob + WN, 64 * w:64 * (w + 1)],
                                spi[:, WN * w:WN * (w + 1)],
                                wv_sb[h][rhs_base:rhs_base + 64, :],
                                start=True, stop=True,
                                tile_position=(rhs_base, ob))
                    nc.scalar.activation(kqt[:, j, 392 * n2:392 * (n2 + 1)],
                                         pkq[:], AF.Identity,
                                         bias=bkq_sb[h][:, 0:1])
                    nc.scalar.activation(
                        vt_pk[:, 1024 * j + 512 * n2:1024 * j + 512 * (n2 + 1)],
                        pvt[:], AF.Copy)
                for t_ in range(2):
                    img = 2 * j + t_
                    rb = 64 * t_
                    nc.sync.dma_start(
                        out=k_pk[32 * (img % 4):32 * (img % 4) + KD, img // 4, :],
                        in_=kqt[rb:rb + KD, j, :])
                    nc.sync.dma_start(
                        out=qstack[KD * img:KD * (img + 1), :],
                        in_=kqt[rb + KD:rb + 2 * KD, j, :])

            # ---- C: depthwise conv on stacked q (shared guttered grid) ----
            kk = KS[h]
            qsv = qstack[:].rearrange("p (n s) -> p n s", n=NW)
            for w in range(NW):
                nc.vector.tensor_copy(
                    Gq[:, :, (7 + GUT) * w:(7 + GUT) * w + 7],
                    qsv[:, w, :].rearrange("p (x y) -> p x y", x=7))
            GA = qgap.tile([128, GROWS, GCOLS], bf16, tag="qacc",
                           name=f"qacc{h}")
            warm_ps = pbcp.tile([128, 392], f32, tag="pbc", name=f"warm{h}")

            def tmp_q(i):
                return qtmp.tile([128, GROWS, GCOLS], bf16, tag=f"qtm{i}",
                                 name=f"qtm{h}_{i}")

            def warm_fn(tm, t):
                p_ = kk // 2
                y0, _ = _region(GROWS, t[0] - p_)
                x0, _ = _region(GCOLS, t[1] - p_)
                nc.tensor.matmul(
                    warm_ps[0:2, 0:WN], ones2_sb[:],
                    tm[:, y0, x0:x0 + WN],
                    start=True, stop=True, tile_position=(0, 0))

            emit_conv_grid(nc, kk, dq_w[h], dq_b[h], Gq[:], GA, tmp_q, warm_fn)
            # unpack to window-block layout
            qflat = qgp.tile([128, NW, WN], bf16, tag="qflat", name=f"qflat{h}")
            for w in range(NW):
                nc.vector.tensor_copy(
                    qflat[:, w, :].rearrange("p (x y) -> p x y", x=7),
                    GA[:, :, (7 + GUT) * w:(7 + GUT) * w + 7])
            for img in range(NI):
                nc.sync.dma_start(
                    out=qp_pk[32 * (img % 4):32 * (img % 4) + KD, img // 4, :],
                    in_=qflat[KD * img:KD * (img + 1)].rearrange("q n s -> q (n s)"))

            # ---- D: attention per pair ----
            spn = None
            if h + 1 < NH:
                spn = spp.tile([128, 4, POS], bf16, tag="sp", name=f"sp{h + 1}")
            spo_all = spop.tile([128, 4, POS], bf16, tag="spo", name=f"spo{h}")
            for j in range(4):
                for n2 in range(2):
                    pa = pap.tile([128, 392], f32, tag="pa", name=f"pa{h}_{j}_{n2}")
                    for t_ in range(2):
                        img = 2 * j + t_
                        ob = 64 * t_
                        q0 = 32 * (img % 4)
                        kh = k_pk[q0:q0 + KD, img // 4, :]
                        qh = qp_pk[q0:q0 + KD, img // 4, :]
                        for w in range(8):
                            co_ = 392 * n2 + WN * w
                            nc.tensor.matmul(
                                pa[ob:ob + WN, WN * w:WN * (w + 1)],
                                kh[:, co_:co_ + WN], qh[:, co_:co_ + WN],
                                start=True, stop=False,
                                tile_position=(q0, ob))
                    nc.tensor.matmul(pa[:], iab_sb[:], ab_sb[h][:],
                                     start=False, stop=True,
                                     tile_position=(0, 0))
                    ein = attp.tile([128, 392], bf16, tag="ein",
                                    name=f"ein{h}_{j}_{n2}")
                    nc.scalar.activation(ein[:], pa[:], AF.Exp)
                    ps1 = ps1p.tile([2, 392], f32, tag="ps1",
                                    name=f"ps1{h}_{j}_{n2}")
                    nc.tensor.matmul(ps1[:], ones2_sb[:], ein[:],
                                     start=True, stop=True,
                                     tile_position=(0, 0))
                    rs = rsp.tile([2, 392], f32, tag="rs", name=f"rs{h}_{j}_{n2}")
                    nc.vector.reciprocal_approx_fast(rs[:], ps1[:])
                    pbc = pbcp.tile([128, 392], f32, tag="pbc",
                                    name=f"pbc{h}_{j}_{n2}")
                    nc.tensor.matmul(pbc[:], sel2_sb[:], rs[:],
                                     start=True, stop=True,
                                     tile_position=(0, 0))
                    bc = attp.tile([128, 392], bf16, tag="bc",
                                   name=f"bc{h}_{j}_{n2}")
                    nc.scalar.activation(bc[:], pbc[:], AF.Copy)
                    pav = pavp.tile([128, 392], f32, tag="pav",
                                    name=f"pav{h}_{j}_{n2}")
                    for t_ in range(2):
                        img = 2 * j + t_
                        ob = 64 * t_
                        for w in range(8):
                            wg = 8 * n2 + w
                            nc.tensor.matmul(
                                pav[ob:ob + D, WN * w:WN * (w + 1)],
                                vt_pk[ob:ob + WN,
                                      1024 * j + 64 * wg:1024 * j + 64 * (wg + 1)],
                                ein[ob:ob + WN, WN * w:WN * (w + 1)],
                                start=True, stop=True,
                                tile_position=(ob, ob))
                    co = 392 * n2
                    nc.vector.tensor_tensor(spo_all[:, j, co:co + 392], pav[:],
                                            bc[:], AO.mult)
                    if spn is not None:
                        nc.vector.scalar_tensor_tensor(
                            spn[:, j, co:co + 392],
                            spo_all[:, j, co:co + 392],
                            bv_sb[h][:, 0:1],
                            spx_tiles[h + 1][:, j, co:co + 392],
                            AO.add, AO.add)
            yv = y_sb[c][64 * h2:64 * h2 + 64, :, :] \
                .rearrange("p (j t) x -> p t j x", t=2)
            for t_ in range(2):
                nc.gpsimd.dma_start(out=yv[:, t_, :, :],
                                    in_=spo_all[64 * t_:64 * t_ + 64, :, :])
            sp_all = spn

    # ---------------- P4+P5+P6 fused per image ------------------------------
    # y is window-block; proj output window-block; x3 written spatially (trunk)
    x3fl = x2fl
    x4fl = [xwp.tile([128, NI, POS], bf16, tag=f"wm{c}", name=f"x4_{c}")
            for c in range(4)]
    w1sb1, w2sb1, b1row1 = load_mlp_w(w1T1_d, b1f1_d, w2T1_d, wp)

    def rhs1(k, img, n2):
        return x4fl[k][:, img, 392 * n2:392 * (n2 + 1)]

    with ExitStack() as es:
        hyp = es.enter_context(tc.tile_pool(name="hyp", bufs=2))
        pjrp = es.enter_context(tc.tile_pool(name="pjr", bufs=2))
        dacp = es.enter_context(tc.tile_pool(name="dac1", bufs=3))
        grdp1 = es.enter_context(tc.tile_pool(name="grd1", bufs=1))
        tmp1 = es.enter_context(tc.tile_pool(name="dwt1", bufs=2))
        o5p = es.enter_context(tc.tile_pool(name="o5", bufs=2))
        hp1 = es.enter_context(tc.tile_pool(name="m1h", bufs=2))
        rp1 = es.enter_context(tc.tile_pool(name="m1r", bufs=4))
        ppp = es.enter_context(tc.tile_pool(name="ppp", bufs=2, space="PSUM"))
        psp1 = es.enter_context(tc.tile_pool(name="m1ps", bufs=4, space="PSUM"))
        pop1 = es.enter_context(tc.tile_pool(name="m1po", bufs=2, space="PSUM"))

        g1_set, g12_set = [], []
        for i in range(GN):
            g = grdp1.tile([128, 30, 32], bf16, tag=f"g{i}", name=f"h{i}")
            nc.gpsimd.memset(g[:], 0.0)
            g1_set.append(g)
            g2 = grdp1.tile([128, 30, 32], bf16, tag=f"g2{i}", name=f"h2{i}")
            nc.gpsimd.memset(g2[:], 0.0)
            g12_set.append(g2)

        def outw1(mo, img, n2, po, b2):
            x5 = o5p.tile([128, 392], f32, tag="x5", name=f"x5_{mo}_{img}_{n2}")
            nc.vector.scalar_tensor_tensor(
                x5[:], po[:], b2[:, 0:1],
                x4fl[mo][:, img, 392 * n2:392 * (n2 + 1)], AO.add, AO.add)
            nc.sync.dma_start(
                out=out_d[img, 128 * mo:128 * (mo + 1), 392 * n2:392 * (n2 + 1)],
                in_=x5[:])

        for img in range(NI):
            # P4: hswish(y + yb), proj, x3 = x2 + proj + pjb
            hys = []
            for cb in range(4):
                yv = y_sb[cb][:, img, :]
                nc.vector.tensor_scalar(yv, yv, yb_sb[cb][:, 0:1], None,
                                        AO.add)
                r = pjrp.tile([128, POS], bf16, tag="pr")
                nc.scalar.activation(r[:], yv, AF.Relu,
                                     scale=acts[:, 0:1], bias=acth[:, 0:1])
                hy = hyp.tile([128, POS], bf16, tag=f"hy{cb}", name=f"hy{cb}_{img}")
                nc.vector.scalar_tensor_tensor(hy[:], r[:], 1.0, yv,
                                               AO.min, AO.mult)
                hys.append(hy)
            for mo in range(4):
                for n2 in range(2):
                    pp = ppp.tile([128, 392], f32, tag="pp")
                    for k in range(4):
                        nc.tensor.matmul(pp[:], pj_sb[k][:, 128 * mo:128 * (mo + 1)],
                                         hys[k][:, 392 * n2:392 * (n2 + 1)],
                                         start=(k == 0), stop=(k == 3))
                    ov = x2fl[mo][:, img, 392 * n2:392 * (n2 + 1)]
                    for w in range(8):
                        nc.vector.scalar_tensor_tensor(
                            win_ap(ov, n2, w, spatial=True),
                            pp[:, WN * w:WN * (w + 1)]
                            .rearrange("p (x y) -> p x y", x=7),
                            pjb_sb[mo][:, 0:1],
                            win_ap(ov, n2, w, spatial=True),
                            AO.add, AO.add)
            # P5: dw1 units for this image (spatial trunk -> x4 spatial)
            for cb in range(4):
                u = (4 * img + cb) % GN
                g, g2 = g1_set[u], g12_set[u]
                nc.scalar.activation(
                    g[:, 1:29, 1:29],
                    x3fl[cb][:, img, :].rearrange("p (h w) -> p h w", h=RES),
                    AF.Copy)
                nc.scalar.activation(
                    g2[:].rearrange("p h w -> p (h w)")[:, 0:959],
                    g[:].rearrange("p h w -> p (h w)")[:, 1:960], AF.Copy)
                acc = dacp.tile([128, 28, 32], bf16, tag="acc",
                                name=f"a1_{cb}_{img}")

                def tmp_fn(i):
                    return tmp1.tile([128, 896], bf16, tag=f"tm{i}",
                                     name=f"tm1_{cb}_{img}_{i}")

                def final_fn():
                    nc.vector.tensor_tensor(
                        x4fl[cb][:, img, :].rearrange("p (h w) -> p h w", h=RES),
                        acc[:, :, 0:28], g[:, 1:29, 1:29], AO.add)

                emit_conv_dw(nc, dw_w["dw1"][cb], dw_b["dw1"][cb], g, g2,
                             acc, tmp_fn, final_fn, SPLIT_DW1)
            # P6: MLP1 for this image
            mlp_img((hp1, rp1, psp1, pop1), img, w1sb1, w2sb1, b1row1, b2sb1,
                    rhs1, outw1, "m1")

    xw_cm.__exit__(None, None, None)
    big_cm.__exit__(None, None, None)
    wp_cm.__exit__(None, None, None)


# ---------------------------------------------------------------------------
# host-side input preprocessing
# ---------------------------------------------------------------------------

def prep_weights(inp):
    def taps(w):  # [C,1,k,k] -> [C, k*k]
        return w.reshape(w.shape[0], -1).astype(np.float32)

    m = {}
    dwpk = np.zeros((128, 80), np.float32)
    for ci in range(4):
        dwpk[:, 10 * ci:10 * ci + 9] = taps(inp["dw0_w"])[128 * ci:128 * (ci + 1)]
        dwpk[:, 10 * ci + 9] = inp["dw0_b"][128 * ci:128 * (ci + 1)]
        dwpk[:, 40 + 10 * ci:40 + 10 * ci + 9] = \
            taps(inp["dw1_w"])[128 * ci:128 * (ci + 1)]
        dwpk[:, 40 + 10 * ci + 9] = inp["dw1_b"][128 * ci:128 * (ci + 1)]
    m["dwpk"] = dwpk
    m["w1T0"] = np.ascontiguousarray(inp["ffn0_w1"].T).astype(ml_dtypes.bfloat16)
    m["b1f0"] = inp["ffn0_b1"].astype(ml_dtypes.bfloat16)
    m["w2T0"] = np.ascontiguousarray(inp["ffn0_w2"].T).astype(ml_dtypes.bfloat16)

    qkv_w, qkv_b = inp["qkv_w"], inp["qkv_b"]
    wkqT = np.empty((NH, D, 2 * KD), np.float32)
    bkq = np.empty((NH, 2 * KD), np.float32)
    wvT = np.empty((NH, D, D), np.float32)
    bv = np.empty((NH, D), np.float32)
    for h in range(NH):
        W = qkv_w[h]  # [96, 64]
        wkqT[h, :, 0:KD] = W[KD:2 * KD].T       # k
        wkqT[h, :, KD:2 * KD] = W[0:KD].T       # q
        bkq[h, 0:KD] = qkv_b[h, KD:2 * KD]
        bkq[h, KD:2 * KD] = qkv_b[h, 0:KD]
        wvT[h] = W[2 * KD:].T
        bv[h] = qkv_b[h, 2 * KD:]
    # packed: [128, NH*32] kq weights (row halves duplicated)
    akq = np.empty((128, NH * 2 * KD), np.float32)
    awv = np.empty((128, NH * D), np.float32)
    abias = np.zeros((128, 2 * NH), np.float32)
    for h in range(NH):
        akq[0:64, 32 * h:32 * h + 32] = wkqT[h]
        akq[64:128, 32 * h:32 * h + 32] = wkqT[h]
        awv[0:64, 64 * h:64 * h + 64] = wvT[h]
        awv[64:128, 64 * h:64 * h + 64] = wvT[h]
        abias[0:32, 2 * h] = bkq[h]
        abias[64:96, 2 * h] = bkq[h]
        abias[0:64, 2 * h + 1] = bv[h]
        abias[64:128, 2 * h + 1] = bv[h]
    m["attkq"] = akq.astype(ml_dtypes.bfloat16)
    m["attwv"] = awv.astype(ml_dtypes.bfloat16)
    m["attbias"] = abias

    dwq_ws = [inp["dwq_w7"], inp["dwq_w5"]] + [inp["dwq_w3"][i] for i in range(6)]
    dwq_bs = [inp["dwq_b7"], inp["dwq_b5"]] + [inp["dwq_b3"][i] for i in range(6)]
    dwqw = np.zeros((128, NH * 50), np.float32)
    for h in range(NH):
        t = taps(dwq_ws[h]) * SCALE
        nt = t.shape[1]
        for i in range(NI):
            dwqw[KD * i:KD * (i + 1), 50 * h:50 * h + nt] = t
            dwqw[KD * i:KD * (i + 1), 50 * h + 49] = dwq_bs[h] * SCALE
    m["dwqw"] = dwqw

    ab = inp["attn_bias"][:, BIAS_IDX]       # [NH, 49, 49]
    ab = np.tile(ab, (1, 1, 8))              # [NH, 49, 392]
    m["ab"] = ab.transpose(1, 0, 2).reshape(WN, NH * 392).copy() \
        .astype(ml_dtypes.bfloat16)

    iab = np.zeros((WN, 128), np.float32)
    for i in range(WN):
        iab[i, i] = 1.0
        iab[i, 64 + i] = 1.0
    m["iab"] = iab.astype(ml_dtypes.bfloat16)
    ones2 = np.zeros((128, 2), np.float32)
    ones2[0:WN, 0] = 1.0
    ones2[64:64 + WN, 1] = 1.0
    m["ones2"] = ones2.astype(ml_dtypes.bfloat16)
    sel2 = np.zeros((2, 128), np.float32)
    sel2[0, 0:64] = 1.0
    sel2[1, 64:128] = 1.0
    m["sel2"] = sel2

    m["projT"] = np.ascontiguousarray(inp["proj_w"].T).astype(ml_dtypes.bfloat16)
    bpk = np.zeros((128, 16), np.float32)
    for ci in range(4):
        bpk[:, ci] = inp["proj_b"][128 * ci:128 * (ci + 1)]
        bpk[:, 4 + ci] = inp["ffn0_b2"][128 * ci:128 * (ci + 1)]
        bpk[:, 8 + ci] = inp["ffn1_b2"][128 * ci:128 * (ci + 1)]
        bpk[:, 12 + ci] = bv.reshape(ED)[128 * ci:128 * (ci + 1)]
    m["bpk"] = bpk

    m["w1T1"] = np.ascontiguousarray(inp["ffn1_w1"].T).astype(ml_dtypes.bfloat16)
    m["b1f1"] = inp["ffn1_b1"].astype(ml_dtypes.bfloat16)
    m["w2T1"] = np.ascontiguousarray(inp["ffn1_w2"].T).astype(ml_dtypes.bfloat16)
    return m


@functools.lru_cache(maxsize=1)
def _cached_program():
    return build_program()


def _run(inputs, trace=False, **kw):
    nc = _cached_program()
    wm = prep_weights(inputs)
    x = np.asarray(inputs["x"], dtype=np.float32).reshape(64, ED, POS)
    x = x.astype(ml_dtypes.bfloat16)
    in_maps = []
    for core in range(NCORES):
        im = dict(wm)
        im["x"] = np.ascontiguousarray(x[NI * core:NI * (core + 1)])
        in_maps.append(im)
    res = bass_utils.run_bass_kernel_spmd(nc, in_maps, list(range(NCORES)),
                                          trace=trace, **kw)
    out = np.concatenate([r["out"] for r in res.results], axis=0)
    return out.reshape(64, ED, RES, RES).astype(np.float32), res


def kernel(**inputs):
    out, _ = _run(inputs)
    return out


# revision 73
# speedup vs baseline: 1.0396x; 1.0313x over previous
"""Trainium2 Bass kernel for nn_BasicBlock (EfficientViT-style block), v3.

Data-parallel over 8 NeuronCores: batch 64 -> 8 images/core.
SBUF-resident bf16 spatial trunk, no DRAM intermediates.
Depthwise convs via valid-region shifted views, split across V/S/G engines.
Per-core program: dw0 -> MLP0 -> cascaded window attention -> proj -> dw1 -> MLP1.
"""
import itertools
import functools
from contextlib import ExitStack
import numpy as np
import ml_dtypes

import concourse.bass as bass
import concourse.mybir as mybir
import concourse.tile as tile
from concourse import bacc
from concourse import bass_utils

f32 = mybir.dt.float32
bf16 = mybir.dt.bfloat16
AO = mybir.AluOpType
AF = mybir.ActivationFunctionType

ED, KD, NH, AR = 512, 16, 8, 4
D = AR * KD            # 64
DH = D * NH            # 512
RES, WS = 28, 7
SCALE = KD ** -0.5
KS = [7, 5, 3, 3, 3, 3, 3, 3]
NI = 8                 # images per core
NCORES = 8
POS = RES * RES        # 784
NW = 16                # windows per image
WN = WS * WS           # 49


def _bias_idx(ws):
    pts = list(itertools.product(range(ws), range(ws)))
    offs, idxs = {}, []
    for p1 in pts:
        for p2 in pts:
            o = (abs(p1[0] - p2[0]), abs(p1[1] - p2[1]))
            if o not in offs:
                offs[o] = len(offs)
            idxs.append(offs[o])
    return np.array(idxs, dtype=np.int32).reshape(ws * ws, ws * ws), len(offs)


BIAS_IDX, N_OFFS = _bias_idx(WS)


# ---------------------------------------------------------------------------
# conv planning (engine split)
# ---------------------------------------------------------------------------

def _region(sz, d):
    """1D dst range [y0,y1) for shift d (src index = dst + d)."""
    return max(0, -d), sz - max(0, d)


def _plan_taps1(k, cols, fold_engs, desc, g0):
    """Greedy engine split for k*k taps. cols(t) -> per-tap column count.

    V taps: STT accumulate in place. S taps: ACT into a cycling tmp slot,
    folded into the accumulator by a V tensor_tensor (@2x) or a G
    tensor_tensor (slow but off the critical engines).
    Returns ((v_taps, s_taps), wall); s_taps entries are (tap, fold_engine).
    """
    p = k // 2
    center = (p, p)
    ccols = cols(center)
    busy = {"V": ccols * 0.26 + 105 + ccols * 0.52 + 105, "S": 0.0, "G": g0}
    v_taps, s_taps = [], []
    order = sorted([t for t in itertools.product(range(k), range(k))
                    if t != center], key=lambda t: (-cols(t) if desc else cols(t)))
    for t in order:
        c = cols(t)
        cand = {}
        nb = dict(busy)
        nb["V"] = busy["V"] + c * 1.042 + 105
        cand["V"] = max(nb.values())
        if "V" in fold_engs:
            nb = dict(busy)
            nb["S"] = busy["S"] + c * 0.833 + 217
            nb["V"] = busy["V"] + c * 0.52 + 105
            cand["SV"] = max(nb.values())
        if "G" in fold_engs:
            nb = dict(busy)
            nb["S"] = busy["S"] + c * 0.833 + 217
            nb["G"] = busy["G"] + c * 1.98 + 156
            cand["SG"] = max(nb.values())
        eng = min(cand, key=lambda e: cand[e])
        if eng == "V":
            v_taps.append(t)
            busy["V"] += c * 1.042 + 105
        elif eng == "SV":
            s_taps.append((t, "V"))
            busy["S"] += c * 0.833 + 217
            busy["V"] += c * 0.52 + 105
        else:
            s_taps.append((t, "G"))
            busy["S"] += c * 0.833 + 217
            busy["G"] += c * 1.98 + 156
    return (v_taps, s_taps), max(busy.values())


def plan_taps(k, cols, g0=0.0):
    best = None
    for folds in (("V",), ("G",), ("V", "G")):
        for desc in (True, False):
            plan, wall = _plan_taps1(k, cols, folds, desc, g0)
            if best is None or wall < best[1]:
                best = (plan, wall)
    return best[0]


def _cols_sp(t, k):
    dy, dx = t
    p = k // 2
    return (RES - abs(dy - p)) * (RES - abs(dx - p))


PLAN_DW = plan_taps(3, lambda t: _cols_sp(t, 3), g0=1000.0)

# shared 1x16 guttered macro-grid for the per-head q convs: 7 rows,
# 16 windows of 7 cols separated by 3-col gutters (max pad of any head)
GUT = 3
GCOLS = 16 * 7 + 15 * GUT       # 157
GROWS = 7


def _cols_gq(t, k):
    dy, dx = t
    p = k // 2
    return (GROWS - abs(dy - p)) * (GCOLS - abs(dx - p))


PLAN_DWQ = {_k: plan_taps(_k, lambda t: _cols_gq(t, _k), g0=1000.0)
            for _k in (3, 5, 7)}
NSLOT = 4


# dw conv engine split: V taps (flat STT), S taps (flat ACT tmps),
# folds mostly on V (flat TT @2x), FOLD_G set folded on GpSimd
SPLIT_DW0 = ([(1, 0), (1, 2), (0, 1)],
             [(0, 0), (0, 2), (2, 0), (2, 1), (2, 2)], {(2, 1)})
SPLIT_DW1 = SPLIT_DW0


def emit_conv_dw(nc, wt, bt, g, g2, acc, tmp_fn, final_fn, split):
    """3x3 depthwise conv, flat shifted taps over a [128,30,32] padded grid.

    g2 is g shifted left one column (keeps even element offsets for the
    odd-dx taps). acc is [128, 28, 32]; flat cols 0..891 hold the interior.
    """
    DW_V_TAPS, DW_S_TAPS, DW_FOLD_G = split
    gf = g[:].rearrange("p h w -> p (h w)")
    g2f = g2[:].rearrange("p h w -> p (h w)") if g2 is not None else None
    af = acc[:].rearrange("p h w -> p (h w)")

    def src(t):
        dy, dx = t
        if g2f is not None and dx == 1:
            return g2f[:, 32 * dy:32 * dy + 892]
        return gf[:, 32 * dy + dx:32 * dy + dx + 892]

    def w_(t):
        return wt[:, (t[0] * 3 + t[1]):(t[0] * 3 + t[1]) + 1]

    nc.vector.tensor_scalar(af[:, 0:892], src((1, 1)), w_((1, 1)),
                            bt[:, 0:1], AO.mult, AO.add)
    vq = list(DW_V_TAPS)
    for i, t in enumerate(DW_S_TAPS):
        tm = tmp_fn(i % NSLOT)
        tf = tm[:].rearrange("p h w -> p (h w)") if len(tm.shape) == 3 else tm[:]
        nc.scalar.activation(tf[:, 0:892], src(t), AF.Identity, scale=w_(t))
        if t in DW_FOLD_G:
            nc.gpsimd.tensor_tensor(af[:, 0:892], tf[:, 0:892], af[:, 0:892],
                                    AO.add)
        else:
            nc.vector.tensor_tensor(af[:, 0:892], tf[:, 0:892], af[:, 0:892],
                                    AO.add)
        if vq:
            t2 = vq.pop(0)
            nc.vector.scalar_tensor_tensor(af[:, 0:892], src(t2), w_(t2),
                                           af[:, 0:892], AO.mult, AO.add)
    for t2 in vq:
        nc.vector.scalar_tensor_tensor(af[:, 0:892], src(t2), w_(t2),
                                       af[:, 0:892], AO.mult, AO.add)
    final_fn()


def emit_conv_grid(nc, k, wt, bt, G, GA, tmp_fn, warm_fn):
    """k*k depthwise conv on the shared guttered macro-grid [128, 7, 157]."""
    p = k // 2
    v_taps, s_taps = PLAN_DWQ[k]

    def dst_v(base, t):
        dy, dx = t
        y0, y1 = _region(GROWS, dy - p)
        x0, x1 = _region(GCOLS, dx - p)
        return base[:, y0:y1, x0:x1]

    def src_v(t):
        dy, dx = t
        y0, y1 = _region(GROWS, dy - p)
        x0, x1 = _region(GCOLS, dx - p)
        return G[:, y0 + dy - p:y1 + dy - p, x0 + dx - p:x1 + dx - p]

    def wcol(t):
        return t[0] * k + t[1]

    nc.vector.tensor_scalar(dst_v(GA[:], (p, p)), src_v((p, p)),
                            wt[:, wcol((p, p)):wcol((p, p)) + 1],
                            bt[:, 0:1], AO.mult, AO.add)
    vq = list(v_taps)
    for i, (t, feng) in enumerate(s_taps):
        tm = tmp_fn(i % NSLOT)
        nc.scalar.activation(dst_v(tm[:], t), src_v(t), AF.Identity,
                             scale=wt[:, wcol(t):wcol(t) + 1])
        av, tv = dst_v(GA[:], t), dst_v(tm[:], t)
        if feng == "G":
            nc.gpsimd.tensor_tensor(av, tv, av, AO.add)
        else:
            nc.vector.tensor_tensor(av, tv, av, AO.add)
        if warm_fn is not None and i % 2 == 0:
            warm_fn(tm, t)
        if vq:
            t2 = vq.pop(0)
            nc.vector.scalar_tensor_tensor(dst_v(GA[:], t2), src_v(t2),
                                           wt[:, wcol(t2):wcol(t2) + 1],
                                           dst_v(GA[:], t2), AO.mult, AO.add)
    for t2 in vq:
        nc.vector.scalar_tensor_tensor(dst_v(GA[:], t2), src_v(t2),
                                       wt[:, wcol(t2):wcol(t2) + 1],
                                       dst_v(GA[:], t2), AO.mult, AO.add)


# ---------------------------------------------------------------------------
# program builder
# ---------------------------------------------------------------------------

def build_program():
    nc = bacc.Bacc("TRN2", target_bir_lowering=False, debug=False,
                   enable_asserts=False, num_devices=NCORES)

    def din(name, shape, dt=f32):
        return nc.dram_tensor(name, list(shape), dt, kind="ExternalInput").ap()

    x_d = din("x", [NI, ED, POS], bf16)
    dwpk_d = din("dwpk", [128, 80])
    w1T0_d = din("w1T0", [ED, 2 * ED], bf16)
    b1f0_d = din("b1f0", [2 * ED], bf16)
    w2T0_d = din("w2T0", [2 * ED, ED], bf16)
    attkq_d = din("attkq", [128, NH * 2 * KD], bf16)
    attwv_d = din("attwv", [128, NH * D], bf16)
    attbias_d = din("attbias", [128, 2 * NH])
    dwqw_d = din("dwqw", [128, NH * 50])
    ab_d = din("ab", [WN, NH * 392], bf16)
    iab_d = din("iab", [WN, 128], bf16)
    ones2_d = din("ones2", [128, 2], bf16)
    sel2_d = din("sel2", [2, 128])
    projT_d = din("projT", [DH, ED], bf16)
    bpk_d = din("bpk", [128, 16])
    w1T1_d = din("w1T1", [ED, 2 * ED], bf16)
    b1f1_d = din("b1f1", [2 * ED], bf16)
    w2T1_d = din("w2T1", [2 * ED, ED], bf16)

    out_d = nc.dram_tensor("out", [NI, ED, POS], f32, kind="ExternalOutput").ap()

    with tile.TileContext(nc) as tc:
        _body(tc, nc, x_d, dwpk_d, w1T0_d, b1f0_d, w2T0_d,
              attkq_d, attwv_d, attbias_d, dwqw_d, ab_d,
              iab_d, ones2_d, sel2_d,
              projT_d, bpk_d,
              w1T1_d, b1f1_d, w2T1_d, out_d)

    nc.compile()
    return nc


def win_ap(ap392, n2, w, spatial):
    """Per-window [*, 49] AP from a 392-col half. spatial: 3D 7x7 slice of
    the 14x28 spatial half; else dense 49-block (window-block layout)."""
    if spatial:
        al, b = w // 4, w % 4
        v = ap392.rearrange("p (h x) -> p h x", h=14)
        return v[:, 7 * al:7 * al + 7, 7 * b:7 * b + 7]
    return ap392[:, WN * w:WN * (w + 1)]


def _body(tc, nc, x_d, dwpk_d, w1T0_d, b1f0_d, w2T0_d,
          attkq_d, attwv_d, attbias_d, dwqw_d, ab_d,
          iab_d, ones2_d, sel2_d,
          projT_d, bpk_d,
          w1T1_d, b1f1_d, w2T1_d, out_d):

    # ---------------- persistent pools -------------------------------------
    wp_cm = tc.tile_pool(name="wp", bufs=1)
    wp = wp_cm.__enter__()
    big_cm = tc.tile_pool(name="big", bufs=1)
    big = big_cm.__enter__()
    xw_cm = tc.tile_pool(name="xw", bufs=1)
    xwp = xw_cm.__enter__()

    def load_mlp_w(w1T_dram, b1_dram, w2T_dram, b2_dram, pool):
        w1sb = []
        for k in range(4):
            w = pool.tile([128, 2 * ED], bf16, tag=f"w1_{k}")
            nc.sync.dma_start(out=w, in_=w1T_dram[128 * k:128 * (k + 1), :])
            w1sb.append(w)
        w2sb = []
        for k in range(8):
            w = pool.tile([128, ED], bf16, tag=f"w2_{k}")
            nc.sync.dma_start(out=w, in_=w2T_dram[128 * k:128 * (k + 1), :])
            w2sb.append(w)
        b1row = pool.tile([1, 2 * ED], bf16, tag="b1row")
        nc.sync.dma_start(out=b1row, in_=b1_dram.unsqueeze(0))
        b2sb = []
        for m in range(4):
            b = pool.tile([128, 1], f32, tag=f"b2_{m}")
            nc.sync.dma_start(out=b, in_=b2_dram[128 * m:128 * (m + 1)].unsqueeze(1))
            b2sb.append(b)
        return w1sb, w2sb, b1row, b2sb

    # dw weights
    dw_w, dw_b = {}, {}
    for nm, wd, bd in (("dw0", dw0w_d, dw0b_d), ("dw1", dw1w_d, dw1b_d)):
        ws_, bs_ = [], []
        for c in range(4):
            w = wp.tile([128, 9], f32, tag=f"{nm}w{c}")
            nc.sync.dma_start(out=w, in_=wd[c])
            b = wp.tile([128, 1], f32, tag=f"{nm}b{c}")
            nc.sync.dma_start(out=b, in_=bd[c].unsqueeze(1))
            ws_.append(w)
            bs_.append(b)
        dw_w[nm], dw_b[nm] = ws_, bs_

    ones392 = wp.tile([1, 392], bf16, tag="ones392")
    nc.vector.memset(ones392, 1.0)
    acth = wp.tile([128, 1], f32, tag="acth")
    nc.vector.memset(acth, 0.5)
    acts = wp.tile([128, 1], f32, tag="acts")
    nc.vector.memset(acts, 1.0 / 6.0)

    # attention weights
    wkq_sb, bkq_sb, wv_sb, bv_sb, dq_w, dq_b, ab_sb = [], [], [], [], [], [], []
    for h in range(NH):
        t = wp.tile([128, 2 * KD], bf16, tag=f"wkq{h}")
        nc.sync.dma_start(out=t[0:64, :], in_=wkqT_d[h])
        nc.sync.dma_start(out=t[64:128, :], in_=wkqT_d[h])
        wkq_sb.append(t)
        t = wp.tile([128, 1], f32, tag=f"bkq{h}")
        nc.sync.dma_start(out=t[0:32, :], in_=bkq_d[h].unsqueeze(1))
        nc.sync.dma_start(out=t[64:96, :], in_=bkq_d[h].unsqueeze(1))
        bkq_sb.append(t)
        t = wp.tile([128, D], bf16, tag=f"wv{h}")
        nc.sync.dma_start(out=t[0:64, :], in_=wvT_d[h])
        nc.sync.dma_start(out=t[64:128, :], in_=wvT_d[h])
        wv_sb.append(t)
        t = wp.tile([128, 1], f32, tag=f"bv{h}")
        nc.sync.dma_start(out=t[0:64, :], in_=bv_d[h].unsqueeze(1))
        nc.sync.dma_start(out=t[64:128, :], in_=bv_d[h].unsqueeze(1))
        bv_sb.append(t)

        t = wp.tile([128, 49], f32, tag=f"dqw{h}")
        nc.sync.dma_start(out=t, in_=dwqw_d[h])
        dq_w.append(t)
        t = wp.tile([128, 1], f32, tag=f"dqb{h}")
        nc.sync.dma_start(out=t, in_=dwqb_d[h].unsqueeze(1))
        dq_b.append(t)
        t = wp.tile([WN, 392], bf16, tag=f"ab{h}")
        nc.sync.dma_start(out=t, in_=ab_d[h])
        ab_sb.append(t)
    iab_sb = wp.tile([WN, 128], bf16, tag="iab")
    nc.sync.dma_start(out=iab_sb, in_=iab_d)
    ones2_sb = wp.tile([128, 2], bf16, tag="ones2")
    nc.sync.dma_start(out=ones2_sb, in_=ones2_d)
    sel2_sb = wp.tile([2, 128], f32, tag="sel2")
    nc.sync.dma_start(out=sel2_sb, in_=sel2_d)

    # proj
    pj_sb = []
    for k in range(4):
        w = wp.tile([128, ED], bf16, tag=f"pj{k}")
        nc.sync.dma_start(out=w, in_=projT_d[128 * k:128 * (k + 1), :])
        pj_sb.append(w)
    pjb_sb, yb_sb = [], []
    for m in range(4):
        b = wp.tile([128, 1], f32, tag=f"pjb{m}")
        nc.sync.dma_start(out=b, in_=projb_d[128 * m:128 * (m + 1)].unsqueeze(1))
        pjb_sb.append(b)
        b = wp.tile([128, 1], f32, tag=f"ybt{m}")
        nc.sync.dma_start(out=b, in_=yb_d[128 * m:128 * (m + 1)].unsqueeze(1))
        yb_sb.append(b)

    w1sb0, w2sb0, b1row0, b2sb0 = load_mlp_w(w1T0_d, b1f0_d, w2T0_d, b2f0_d, wp)

    # ---------------- MLP per-image emitter --------------------------------
    def mlp_img(pools, img, w1sb, w2sb, b1row, b2sb, rhs_getter, out_writer, name):
        hp, rp, psp, pop = pools
        hs = []
        for m in range(8):
            h = hp.tile([128, POS], bf16, tag=f"h{m}", name=f"{name}h{m}_{img}")
            hs.append(h)
        for m in range(8):
            for n2 in range(2):
                ph = psp.tile([128, 392], f32, tag="ph")
                for k in range(4):
                    nc.tensor.matmul(
                        ph[:], w1sb[k][:, 128 * m:128 * (m + 1)],
                        rhs_getter(k, img, n2),
                        start=(k == 0), stop=False)
                nc.tensor.matmul(
                    ph[:], b1row[:, 128 * m:128 * (m + 1)],
                    ones392[:], start=False, stop=True)
                r = rp.tile([128, 392], bf16, tag="relu")
                nc.scalar.activation(r[:], ph[:], AF.Relu,
                                     scale=acts[:, 0:1], bias=acth[:, 0:1])
                nc.vector.scalar_tensor_tensor(
                    hs[m][:, 392 * n2:392 * (n2 + 1)], r[:], 1.0,
                    ph[:], AO.min, AO.mult)
        for mo in range(4):
            for n2 in range(2):
                po = pop.tile([128, 392], f32, tag="po")
                for k in range(8):
                    nc.tensor.matmul(
                        po[:], w2sb[k][:, 128 * mo:128 * (mo + 1)],
                        hs[k][:, 392 * n2:392 * (n2 + 1)],
                        start=(k == 0), stop=(k == 7))
                out_writer(mo, img, n2, po, b2sb[mo])

    # ---------------- P0+P1: input DMA + dw0 + residual -> x1 (spatial) -----
    x1fl = [big.tile([128, NI, POS], bf16, tag=f"fl{c}", name=f"x1_{c}")
            for c in range(4)]

    def rhs0(k, img, n2):
        return x1fl[k][:, img, 392 * n2:392 * (n2 + 1)]

    def outw0(mo, img, n2, po, b2):
        ov = x1fl[mo][:, img, 392 * n2:392 * (n2 + 1)]
        nc.vector.scalar_tensor_tensor(ov, po[:], b2[:, 0:1], ov, AO.add, AO.add)

    GN = 3
    with tc.tile_pool(name="grd", bufs=1) as grdp, \
         tc.tile_pool(name="dac", bufs=3) as dacp, \
         tc.tile_pool(name="dwt0", bufs=2) as tmp0, \
         tc.tile_pool(name="m0h", bufs=2) as hp0, \
         tc.tile_pool(name="m0r", bufs=4) as rp0, \
         tc.tile_pool(name="m0ps", bufs=4, space="PSUM") as psp0, \
         tc.tile_pool(name="m0po", bufs=2, space="PSUM") as pop0:
        g_set, g2_set = [], []
        for i in range(GN):
            g = grdp.tile([128, 30, 32], bf16, tag=f"g{i}", name=f"g{i}")
            nc.gpsimd.memset(g[:], 0.0)
            g_set.append(g)
            g2 = grdp.tile([128, 30, 32], bf16, tag=f"g2{i}", name=f"g2{i}")
            nc.gpsimd.memset(g2[:], 0.0)
            g2_set.append(g2)
        for img in range(NI):
            for c in range(4):
                u = (4 * img + c) % GN
                g, g2 = g_set[u], g2_set[u]
                nc.scalar.dma_start(
                    out=g[:, 1:29, 1:29],
                    in_=x_d[img, 128 * c:128 * (c + 1), :]
                    .rearrange("p (h w) -> p h w", h=RES))
                nc.scalar.activation(
                    g2[:].rearrange("p h w -> p (h w)")[:, 0:959],
                    g[:].rearrange("p h w -> p (h w)")[:, 1:960], AF.Copy)
                acc = dacp.tile([128, 28, 32], bf16, tag="acc",
                                name=f"a0_{c}_{img}")

                def tmp_fn(i):
                    return tmp0.tile([128, 896], bf16, tag=f"tm{i}",
                                     name=f"tm0_{c}_{img}_{i}")

                def final_fn():
                    nc.vector.tensor_tensor(
                        x1fl[c][:, img, :].rearrange("p (h w) -> p h w", h=RES),
                        acc[:, :, 0:28], g[:, 1:29, 1:29], AO.add)

                emit_conv_dw(nc, dw_w["dw0"][c], dw_b["dw0"][c], g, g2,
                             acc, tmp_fn, final_fn, SPLIT_DW0)
            mlp_img((hp0, rp0, psp0, pop0), img, w1sb0, w2sb0, b1row0, b2sb0,
                    rhs0, outw0, "m0")

    x2fl = x1fl   # trunk now holds x2 (spatial, bf16)

    # ---------------- P3: cascaded attention -> y_sb ------------------------
    # y in window-block layout: y_sb[c][64*h2+d, img, 49*w + pos]
    y_sb = [xwp.tile([128, NI, POS], bf16, tag=f"wm{c}", name=f"y_{c}")
            for c in range(4)]

    def prow(i):
        return 64 * (i % 2)

    with ExitStack() as es:
        spkp = es.enter_context(tc.tile_pool(name="spk", bufs=1))
        spxp = es.enter_context(tc.tile_pool(name="spx", bufs=2))
        spp = es.enter_context(tc.tile_pool(name="sp", bufs=1))
        kqtp = es.enter_context(tc.tile_pool(name="kqt", bufs=1))
        kpkp = es.enter_context(tc.tile_pool(name="kpk", bufs=1))
        vtp = es.enter_context(tc.tile_pool(name="vt", bufs=1))
        qgp = es.enter_context(tc.tile_pool(name="qg", bufs=1))
        qgrp = es.enter_context(tc.tile_pool(name="qgr", bufs=1))
        qgap = es.enter_context(tc.tile_pool(name="qga", bufs=1))
        qtmp = es.enter_context(tc.tile_pool(name="qtm", bufs=1))
        attp = es.enter_context(tc.tile_pool(name="att", bufs=2))
        rsp = es.enter_context(tc.tile_pool(name="rsp", bufs=1))
        spop = es.enter_context(tc.tile_pool(name="spo", bufs=1))
        pkqp = es.enter_context(tc.tile_pool(name="pkq", bufs=1, space="PSUM"))
        pvtp = es.enter_context(tc.tile_pool(name="pvt", bufs=1, space="PSUM"))
        pap = es.enter_context(tc.tile_pool(name="pa", bufs=2, space="PSUM"))
        ps1p = es.enter_context(tc.tile_pool(name="ps1", bufs=1, space="PSUM"))
        pbcp = es.enter_context(tc.tile_pool(name="pbc", bufs=1, space="PSUM"))
        pavp = es.enter_context(tc.tile_pool(name="pav", bufs=2, space="PSUM"))

        spx_tiles = {}

        def fetch_spx(h):
            """Stage pair-packed spatial slice via DMA, then repack to
            window-block with V copies (@4x)."""
            c, h2 = h // 2, h % 2
            t = spkp.tile([128, 4, POS], bf16, tag="spk", name=f"spk{h}")
            xv = x2fl[c][64 * h2:64 * h2 + 64, :, :] \
                .rearrange("p (j t) x -> p t j x", t=2)
            for par in range(2):
                nc.gpsimd.dma_start(out=t[64 * par:64 * par + 64, :, :],
                                    in_=xv[:, par, :, :])
            twb = spxp.tile([128, 4, POS], bf16, tag="spxwb", name=f"spxwb{h}")
            for j in range(4):
                for n2 in range(2):
                    co = 392 * n2
                    for w in range(8):
                        nc.vector.tensor_copy(
                            twb[:, j, co + WN * w:co + WN * (w + 1)]
                            .rearrange("p (x y) -> p x y", x=7),
                            win_ap(t[:, j, co:co + 392], n2, w, spatial=True))
            spx_tiles[h] = twb

        # shared q-conv macro-grid: gutters zeroed once, window interiors
        # overwritten per head by the pack copies
        Gq = qgrp.tile([128, GROWS, GCOLS], bf16, tag="qpad", name="qpad")
        nc.gpsimd.memset(Gq[:], 0.0)

        fetch_spx(0)
        sp_all = spx_tiles[0]
        for h in range(NH):
            c, h2 = h // 2, h % 2
            if h + 1 < NH:
                fetch_spx(h + 1)

            kqt = kqtp.tile([128, 4, POS], bf16, tag="kqt", name=f"kqt{h}")
            k_pk = kpkp.tile([128, 2, POS], bf16, tag="k", name=f"k{h}")
            qstack = qgp.tile([128, POS], bf16, tag="qstack", name=f"qstack{h}")
            qp_pk = kpkp.tile([128, 2, POS], bf16, tag="qp", name=f"qp{h}")
            vt_pk = vtp.tile([128, 4 * 1024], bf16, tag="vt", name=f"vt{h}")

            # ---- A/B: kqv matmuls + evict + repack DMAs ----
            for j in range(4):          # image pairs (2j, 2j+1)
                for n2 in range(2):
                    pkq = pkqp.tile([128, 392], f32, tag="pkq",
                                    name=f"pkq{h}_{j}_{n2}")
                    pvt = pvtp.tile([128, 512], f32, tag="pvt",
                                    name=f"pvt{h}_{j}_{n2}")
                    for t_ in range(2):
                        img = 2 * j + t_
                        ob = 64 * t_
                        rhs_base = prow(img)
                        spi = sp_all[rhs_base:rhs_base + 64, img // 2,
                                     392 * n2:392 * (n2 + 1)]
                        nc.tensor.matmul(
                            pkq[ob:ob + 2 * KD, :],
                            wkq_sb[h][rhs_base:rhs_base + 64, :],
                            spi, start=True, stop=True,
                            tile_position=(rhs_base, ob))
                        for w in range(8):
                            nc.tensor.matmul(
                                pvt[ob:ob + WN, 64 * w:64 * (w + 1)],
                                spi[:, WN * w:WN * (w + 1)],
                                wv_sb[h][rhs_base:rhs_base + 64, :],
                                start=True, stop=True,
                                tile_position=(rhs_base, ob))
                    nc.scalar.activation(kqt[:, j, 392 * n2:392 * (n2 + 1)],
                                         pkq[:], AF.Identity,
                                         bias=bkq_sb[h][:, 0:1])
                    nc.scalar.activation(
                        vt_pk[:, 1024 * j + 512 * n2:1024 * j + 512 * (n2 + 1)],
                        pvt[:], AF.Copy)
                for t_ in range(2):
                    img = 2 * j + t_
                    rb = 64 * t_
                    nc.gpsimd.dma_start(
                        out=k_pk[32 * (img % 4):32 * (img % 4) + KD, img // 4, :],
                        in_=kqt[rb:rb + KD, j, :])
                    nc.gpsimd.dma_start(
                        out=qstack[KD * img:KD * (img + 1), :],
                        in_=kqt[rb + KD:rb + 2 * KD, j, :])

            # ---- C: depthwise conv on stacked q (shared guttered grid) ----
            kk = KS[h]
            qsv = qstack[:].rearrange("p (n s) -> p n s", n=NW)
            for w in range(NW):
                nc.vector.tensor_copy(
                    Gq[:, :, (7 + GUT) * w:(7 + GUT) * w + 7],
                    qsv[:, w, :].rearrange("p (x y) -> p x y", x=7))
            GA = qgap.tile([128, GROWS, GCOLS], bf16, tag="qacc",
                           name=f"qacc{h}")
            warm_ps = pbcp.tile([128, 392], f32, tag="pbc", name=f"warm{h}")

            def tmp_q(i):
                return qtmp.tile([128, GROWS, GCOLS], bf16, tag=f"qtm{i}",
                                 name=f"qtm{h}_{i}")

            def warm_fn(tm, t):
                p_ = kk // 2
                y0, _ = _region(GROWS, t[0] - p_)
                x0, _ = _region(GCOLS, t[1] - p_)
                nc.tensor.matmul(
                    warm_ps[0:2, 0:WN], ones2_sb[:],
                    tm[:, y0, x0:x0 + WN],
                    start=True, stop=True, tile_position=(0, 0))

            emit_conv_grid(nc, kk, dq_w[h], dq_b[h], Gq[:], GA, tmp_q, warm_fn)
            # unpack to window-block layout
            qflat = qgp.tile([128, NW, WN], bf16, tag="qflat", name=f"qflat{h}")
            for w in range(NW):
                nc.vector.tensor_copy(
                    qflat[:, w, :].rearrange("p (x y) -> p x y", x=7),
                    GA[:, :, (7 + GUT) * w:(7 + GUT) * w + 7])
            for img in range(NI):
                nc.gpsimd.dma_start(
                    out=qp_pk[32 * (img % 4):32 * (img % 4) + KD, img // 4, :],
                    in_=qflat[KD * img:KD * (img + 1)].rearrange("q n s -> q (n s)"))

            # ---- D: attention per pair ----
            spn = None
            if h + 1 < NH:
                spn = spp.tile([128, 4, POS], bf16, tag="sp", name=f"sp{h + 1}")
            spo_all = spop.tile([128, 4, POS], bf16, tag="spo", name=f"spo{h}")
            for j in range(4):
                for n2 in range(2):
                    pa = pap.tile([128, 392], f32, tag="pa", name=f"pa{h}_{j}_{n2}")
                    for t_ in range(2):
                        img = 2 * j + t_
                        ob = 64 * t_
                        q0 = 32 * (img % 4)
                        kh = k_pk[q0:q0 + KD, img // 4, :]
                        qh = qp_pk[q0:q0 + KD, img // 4, :]
                        for w in range(8):
                            co_ = 392 * n2 + WN * w
                            nc.tensor.matmul(
                                pa[ob:ob + WN, WN * w:WN * (w + 1)],
                                kh[:, co_:co_ + WN], qh[:, co_:co_ + WN],
                                start=True, stop=False,
                                tile_position=(q0, ob))
                    nc.tensor.matmul(pa[:], iab_sb[:], ab_sb[h][:],
                                     start=False, stop=True,
                                     tile_position=(0, 0))
                    ein = attp.tile([128, 392], bf16, tag="ein",
                                    name=f"ein{h}_{j}_{n2}")
                    nc.scalar.activation(ein[:], pa[:], AF.Exp)
                    ps1 = ps1p.tile([2, 392], f32, tag="ps1",
                                    name=f"ps1{h}_{j}_{n2}")
                    nc.tensor.matmul(ps1[:], ones2_sb[:], ein[:],
                                     start=True, stop=True,
                                     tile_position=(0, 0))
                    rs = rsp.tile([2, 392], f32, tag="rs", name=f"rs{h}_{j}_{n2}")
                    nc.vector.reciprocal_approx_fast(rs[:], ps1[:])
                    pbc = pbcp.tile([128, 392], f32, tag="pbc",
                                    name=f"pbc{h}_{j}_{n2}")
                    nc.tensor.matmul(pbc[:], sel2_sb[:], rs[:],
                                     start=True, stop=True,
                                     tile_position=(0, 0))
                    bc = attp.tile([128, 392], bf16, tag="bc",
                                   name=f"bc{h}_{j}_{n2}")
                    nc.scalar.activation(bc[:], pbc[:], AF.Copy)
                    pav = pavp.tile([128, 392], f32, tag="pav",
                                    name=f"pav{h}_{j}_{n2}")
                    for t_ in range(2):
                        img = 2 * j + t_
                        ob = 64 * t_
                        for w in range(8):
                            wg = 8 * n2 + w
                            nc.tensor.matmul(
                                pav[ob:ob + D, WN * w:WN * (w + 1)],
                                vt_pk[ob:ob + WN,
                                      1024 * j + 64 * wg:1024 * j + 64 * (wg + 1)],
                                ein[ob:ob + WN, WN * w:WN * (w + 1)],
                                start=True, stop=True,
                                tile_position=(ob, ob))
                    co = 392 * n2
                    nc.vector.tensor_tensor(spo_all[:, j, co:co + 392], pav[:],
                                            bc[:], AO.mult)
                    if spn is not None:
                        nc.vector.scalar_tensor_tensor(
                            spn[:, j, co:co + 392],
                            spo_all[:, j, co:co + 392],
                            bv_sb[h][:, 0:1],
                            spx_tiles[h + 1][:, j, co:co + 392],
                            AO.add, AO.add)
            yv = y_sb[c][64 * h2:64 * h2 + 64, :, :] \
                .rearrange("p (j t) x -> p t j x", t=2)
            for t_ in range(2):
                nc.gpsimd.dma_start(out=yv[:, t_, :, :],
                                    in_=spo_all[64 * t_:64 * t_ + 64, :, :])
            sp_all = spn

    # ---------------- P4+P5+P6 fused per image ------------------------------
    # y is window-block; proj output window-block; x3 written spatially (trunk)
    x3fl = x2fl
    x4fl = [xwp.tile([128, NI, POS], bf16, tag=f"wm{c}", name=f"x4_{c}")
            for c in range(4)]
    w1sb1, w2sb1, b1row1 = load_mlp_w(w1T1_d, b1f1_d, w2T1_d, wp)

    def rhs1(k, img, n2):
        return x4fl[k][:, img, 392 * n2:392 * (n2 + 1)]

    with ExitStack() as es:
        hyp = es.enter_context(tc.tile_pool(name="hyp", bufs=2))
        pjrp = es.enter_context(tc.tile_pool(name="pjr", bufs=2))
        dacp = es.enter_context(tc.tile_pool(name="dac1", bufs=3))
        grdp1 = es.enter_context(tc.tile_pool(name="grd1", bufs=1))
        tmp1 = es.enter_context(tc.tile_pool(name="dwt1", bufs=2))
        o5p = es.enter_context(tc.tile_pool(name="o5", bufs=2))
        hp1 = es.enter_context(tc.tile_pool(name="m1h", bufs=2))
        rp1 = es.enter_context(tc.tile_pool(name="m1r", bufs=4))
        ppp = es.enter_context(tc.tile_pool(name="ppp", bufs=2, space="PSUM"))
        psp1 = es.enter_context(tc.tile_pool(name="m1ps", bufs=4, space="PSUM"))
        pop1 = es.enter_context(tc.tile_pool(name="m1po", bufs=2, space="PSUM"))

        g1_set, g12_set = [], []
        for i in range(GN):
            g = grdp1.tile([128, 30, 32], bf16, tag=f"g{i}", name=f"h{i}")
            nc.gpsimd.memset(g[:], 0.0)
            g1_set.append(g)
            g2 = grdp1.tile([128, 30, 32], bf16, tag=f"g2{i}", name=f"h2{i}")
            nc.gpsimd.memset(g2[:], 0.0)
            g12_set.append(g2)

        def outw1(mo, img, n2, po, b2):
            x5 = o5p.tile([128, 392], f32, tag="x5", name=f"x5_{mo}_{img}_{n2}")
            nc.vector.scalar_tensor_tensor(
                x5[:], po[:], b2[:, 0:1],
                x4fl[mo][:, img, 392 * n2:392 * (n2 + 1)], AO.add, AO.add)
            nc.sync.dma_start(
                out=out_d[img, 128 * mo:128 * (mo + 1), 392 * n2:392 * (n2 + 1)],
                in_=x5[:])

        for img in range(NI):
            # P4: hswish(y + yb), proj, x3 = x2 + proj + pjb
            hys = []
            for cb in range(4):
                yv = y_sb[cb][:, img, :]
                nc.vector.tensor_scalar(yv, yv, yb_sb[cb][:, 0:1], None,
                                        AO.add)
                r = pjrp.tile([128, POS], bf16, tag="pr")
                nc.scalar.activation(r[:], yv, AF.Relu,
                                     scale=acts[:, 0:1], bias=acth[:, 0:1])
                hy = hyp.tile([128, POS], bf16, tag=f"hy{cb}", name=f"hy{cb}_{img}")
                nc.vector.scalar_tensor_tensor(hy[:], r[:], 1.0, yv,
                                               AO.min, AO.mult)
                hys.append(hy)
            for mo in range(4):
                for n2 in range(2):
                    pp = ppp.tile([128, 392], f32, tag="pp")
                    for k in range(4):
                        nc.tensor.matmul(pp[:], pj_sb[k][:, 128 * mo:128 * (mo + 1)],
                                         hys[k][:, 392 * n2:392 * (n2 + 1)],
                                         start=(k == 0), stop=(k == 3))
                    ov = x2fl[mo][:, img, 392 * n2:392 * (n2 + 1)]
                    for w in range(8):
                        nc.vector.scalar_tensor_tensor(
                            win_ap(ov, n2, w, spatial=True),
                            pp[:, WN * w:WN * (w + 1)]
                            .rearrange("p (x y) -> p x y", x=7),
                            pjb_sb[mo][:, 0:1],
                            win_ap(ov, n2, w, spatial=True),
                            AO.add, AO.add)
            # P5: dw1 units for this image (spatial trunk -> x4 spatial)
            for cb in range(4):
                u = (4 * img + cb) % GN
                g, g2 = g1_set[u], g12_set[u]
                nc.scalar.activation(
                    g[:, 1:29, 1:29],
                    x3fl[cb][:, img, :].rearrange("p (h w) -> p h w", h=RES),
                    AF.Copy)
                nc.scalar.activation(
                    g2[:].rearrange("p h w -> p (h w)")[:, 0:959],
                    g[:].rearrange("p h w -> p (h w)")[:, 1:960], AF.Copy)
                acc = dacp.tile([128, 28, 32], bf16, tag="acc",
                                name=f"a1_{cb}_{img}")

                def tmp_fn(i):
                    return tmp1.tile([128, 896], bf16, tag=f"tm{i}",
                                     name=f"tm1_{cb}_{img}_{i}")

                def final_fn():
                    nc.vector.tensor_tensor(
                        x4fl[cb][:, img, :].rearrange("p (h w) -> p h w", h=RES),
                        acc[:, :, 0:28], g[:, 1:29, 1:29], AO.add)

                emit_conv_dw(nc, dw_w["dw1"][cb], dw_b["dw1"][cb], g, g2,
                             acc, tmp_fn, final_fn, SPLIT_DW1)
            # P6: MLP1 for this image
            mlp_img((hp1, rp1, psp1, pop1), img, w1sb1, w2sb1, b1row1, b2sb1,
                    rhs1, outw1, "m1")

    xw_cm.__exit__(None, None, None)
    big_cm.__exit__(None, None, None)
    wp_cm.__exit__(None, None, None)


# ---------------------------------------------------------------------------
# host-side input preprocessing
# ---------------------------------------------------------------------------

def prep_weights(inp):
    def taps(w):  # [C,1,k,k] -> [C, k*k]
        return w.reshape(w.shape[0], -1).astype(np.float32)

    m = {}
    dwpk = np.zeros((128, 80), np.float32)
    for ci in range(4):
        dwpk[:, 10 * ci:10 * ci + 9] = taps(inp["dw0_w"])[128 * ci:128 * (ci + 1)]
        dwpk[:, 10 * ci + 9] = inp["dw0_b"][128 * ci:128 * (ci + 1)]
        dwpk[:, 40 + 10 * ci:40 + 10 * ci + 9] = \
            taps(inp["dw1_w"])[128 * ci:128 * (ci + 1)]
        dwpk[:, 40 + 10 * ci + 9] = inp["dw1_b"][128 * ci:128 * (ci + 1)]
    m["dwpk"] = dwpk
    m["w1T0"] = np.ascontiguousarray(inp["ffn0_w1"].T).astype(ml_dtypes.bfloat16)
    m["b1f0"] = inp["ffn0_b1"].astype(ml_dtypes.bfloat16)
    m["w2T0"] = np.ascontiguousarray(inp["ffn0_w2"].T).astype(ml_dtypes.bfloat16)

    qkv_w, qkv_b = inp["qkv_w"], inp["qkv_b"]
    wkqT = np.empty((NH, D, 2 * KD), np.float32)
    bkq = np.empty((NH, 2 * KD), np.float32)
    wvT = np.empty((NH, D, D), np.float32)
    bv = np.empty((NH, D), np.float32)
    for h in range(NH):
        W = qkv_w[h]  # [96, 64]
        wkqT[h, :, 0:KD] = W[KD:2 * KD].T       # k
        wkqT[h, :, KD:2 * KD] = W[0:KD].T       # q
        bkq[h, 0:KD] = qkv_b[h, KD:2 * KD]
        bkq[h, KD:2 * KD] = qkv_b[h, 0:KD]
        wvT[h] = W[2 * KD:].T
        bv[h] = qkv_b[h, 2 * KD:]
    # packed: [128, NH*32] kq weights (row halves duplicated)
    akq = np.empty((128, NH * 2 * KD), np.float32)
    awv = np.empty((128, NH * D), np.float32)
    abias = np.zeros((128, 2 * NH), np.float32)
    for h in range(NH):
        akq[0:64, 32 * h:32 * h + 32] = wkqT[h]
        akq[64:128, 32 * h:32 * h + 32] = wkqT[h]
        awv[0:64, 64 * h:64 * h + 64] = wvT[h]
        awv[64:128, 64 * h:64 * h + 64] = wvT[h]
        abias[0:32, 2 * h] = bkq[h]
        abias[64:96, 2 * h] = bkq[h]
        abias[0:64, 2 * h + 1] = bv[h]
        abias[64:128, 2 * h + 1] = bv[h]
    m["attkq"] = akq.astype(ml_dtypes.bfloat16)
    m["attwv"] = awv.astype(ml_dtypes.bfloat16)
    m["attbias"] = abias

    dwq_ws = [inp["dwq_w7"], inp["dwq_w5"]] + [inp["dwq_w3"][i] for i in range(6)]
    dwq_bs = [inp["dwq_b7"], inp["dwq_b5"]] + [inp["dwq_b3"][i] for i in range(6)]
    dwqw = np.zeros((128, NH * 50), np.float32)
    for h in range(NH):
        t = taps(dwq_ws[h]) * SCALE
        nt = t.shape[1]
        for i in range(NI):
            dwqw[KD * i:KD * (i + 1), 50 * h:50 * h + nt] = t
            dwqw[KD * i:KD * (i + 1), 50 * h + 49] = dwq_bs[h] * SCALE
    m["dwqw"] = dwqw

    ab = inp["attn_bias"][:, BIAS_IDX]       # [NH, 49, 49]
    ab = np.tile(ab, (1, 1, 8))              # [NH, 49, 392]
    m["ab"] = ab.transpose(1, 0, 2).reshape(WN, NH * 392).copy() \
        .astype(ml_dtypes.bfloat16)

    iab = np.zeros((WN, 128), np.float32)
    for i in range(WN):
        iab[i, i] = 1.0
        iab[i, 64 + i] = 1.0
    m["iab"] = iab.astype(ml_dtypes.bfloat16)
    ones2 = np.zeros((128, 2), np.float32)
    ones2[0:WN, 0] = 1.0
    ones2[64:64 + WN, 1] = 1.0
    m["ones2"] = ones2.astype(ml_dtypes.bfloat16)
    sel2 = np.zeros((2, 128), np.float32)
    sel2[0, 0:64] = 1.0
    sel2[1, 64:128] = 1.0
    m["sel2"] = sel2

    m["projT"] = np.ascontiguousarray(inp["proj_w"].T).astype(ml_dtypes.bfloat16)
    bpk = np.zeros((128, 16), np.float32)
    for ci in range(4):
        bpk[:, ci] = inp["proj_b"][128 * ci:128 * (ci + 1)]
        bpk[:, 4 + ci] = inp["ffn0_b2"][128 * ci:128 * (ci + 1)]
        bpk[:, 8 + ci] = inp["ffn1_b2"][128 * ci:128 * (ci + 1)]
        bpk[:, 12 + ci] = bv.reshape(ED)[128 * ci:128 * (ci + 1)]
    m["bpk"] = bpk

    m["w1T1"] = np.ascontiguousarray(inp["ffn1_w1"].T).astype(ml_dtypes.bfloat16)
    m["b1f1"] = inp["ffn1_b1"].astype(ml_dtypes.bfloat16)
    m["w2T1"] = np.ascontiguousarray(inp["ffn1_w2"].T).astype(ml_dtypes.bfloat16)
    return m


@functools.lru_cache(maxsize=1)
def _cached_program():
    return build_program()


def _run(inputs, trace=False, **kw):
    nc = _cached_program()
    wm = prep_weights(inputs)
    x = np.asarray(inputs["x"], dtype=np.float32).reshape(64, ED, POS)
    x = x.astype(ml_dtypes.bfloat16)
    in_maps = []
    for core in range(NCORES):
        im = dict(wm)
        im["x"] = np.ascontiguousarray(x[NI * core:NI * (core + 1)])
        in_maps.append(im)
    res = bass_utils.run_bass_kernel_spmd(nc, in_maps, list(range(NCORES)),
                                          trace=trace, **kw)
    out = np.concatenate([r["out"] for r in res.results], axis=0)
    return out.reshape(64, ED, RES, RES).astype(np.float32), res


def kernel(**inputs):
    out, _ = _run(inputs)
    return out


# revision 74
# speedup vs baseline: 1.0857x; 1.0444x over previous
"""Trainium2 Bass kernel for nn_BasicBlock (EfficientViT-style block), v3.

Data-parallel over 8 NeuronCores: batch 64 -> 8 images/core.
SBUF-resident bf16 spatial trunk, no DRAM intermediates.
Depthwise convs via valid-region shifted views, split across V/S/G engines.
Per-core program: dw0 -> MLP0 -> cascaded window attention -> proj -> dw1 -> MLP1.
"""
import itertools
import functools
from contextlib import ExitStack
import numpy as np
import ml_dtypes

import concourse.bass as bass
import concourse.mybir as mybir
import concourse.tile as tile
from concourse import bacc
from concourse import bass_utils

f32 = mybir.dt.float32
bf16 = mybir.dt.bfloat16
AO = mybir.AluOpType
AF = mybir.ActivationFunctionType

ED, KD, NH, AR = 512, 16, 8, 4
D = AR * KD            # 64
DH = D * NH            # 512
RES, WS = 28, 7
SCALE = KD ** -0.5
KS = [7, 5, 3, 3, 3, 3, 3, 3]
NI = 8                 # images per core
NCORES = 8
POS = RES * RES        # 784
NW = 16                # windows per image
WN = WS * WS           # 49


def _bias_idx(ws):
    pts = list(itertools.product(range(ws), range(ws)))
    offs, idxs = {}, []
    for p1 in pts:
        for p2 in pts:
            o = (abs(p1[0] - p2[0]), abs(p1[1] - p2[1]))
            if o not in offs:
                offs[o] = len(offs)
            idxs.append(offs[o])
    return np.array(idxs, dtype=np.int32).reshape(ws * ws, ws * ws), len(offs)


BIAS_IDX, N_OFFS = _bias_idx(WS)


# ---------------------------------------------------------------------------
# conv planning (engine split)
# ---------------------------------------------------------------------------

def _region(sz, d):
    """1D dst range [y0,y1) for shift d (src index = dst + d)."""
    return max(0, -d), sz - max(0, d)


def _plan_taps1(k, cols, fold_engs, desc, g0):
    """Greedy engine split for k*k taps. cols(t) -> per-tap column count.

    V taps: STT accumulate in place. S taps: ACT into a cycling tmp slot,
    folded into the accumulator by a V tensor_tensor (@2x) or a G
    tensor_tensor (slow but off the critical engines).
    Returns ((v_taps, s_taps), wall); s_taps entries are (tap, fold_engine).
    """
    p = k // 2
    center = (p, p)
    ccols = cols(center)
    busy = {"V": ccols * 0.26 + 105 + ccols * 0.52 + 105, "S": 0.0, "G": g0}
    v_taps, s_taps = [], []
    order = sorted([t for t in itertools.product(range(k), range(k))
                    if t != center], key=lambda t: (-cols(t) if desc else cols(t)))
    for t in order:
        c = cols(t)
        cand = {}
        nb = dict(busy)
        nb["V"] = busy["V"] + c * 1.042 + 105
        cand["V"] = max(nb.values())
        if "V" in fold_engs:
            nb = dict(busy)
            nb["S"] = busy["S"] + c * 0.833 + 217
            nb["V"] = busy["V"] + c * 0.52 + 105
            cand["SV"] = max(nb.values())
        if "G" in fold_engs:
            nb = dict(busy)
            nb["S"] = busy["S"] + c * 0.833 + 217
            nb["G"] = busy["G"] + c * 1.98 + 156
            cand["SG"] = max(nb.values())
        eng = min(cand, key=lambda e: cand[e])
        if eng == "V":
            v_taps.append(t)
            busy["V"] += c * 1.042 + 105
        elif eng == "SV":
            s_taps.append((t, "V"))
            busy["S"] += c * 0.833 + 217
            busy["V"] += c * 0.52 + 105
        else:
            s_taps.append((t, "G"))
            busy["S"] += c * 0.833 + 217
            busy["G"] += c * 1.98 + 156
    return (v_taps, s_taps), max(busy.values())


def plan_taps(k, cols, g0=0.0):
    best = None
    for folds in (("V",), ("G",), ("V", "G")):
        for desc in (True, False):
            plan, wall = _plan_taps1(k, cols, folds, desc, g0)
            if best is None or wall < best[1]:
                best = (plan, wall)
    return best[0]


def _cols_sp(t, k):
    dy, dx = t
    p = k // 2
    return (RES - abs(dy - p)) * (RES - abs(dx - p))


PLAN_DW = plan_taps(3, lambda t: _cols_sp(t, 3), g0=1000.0)

# shared 1x16 guttered macro-grid for the per-head q convs: 7 rows,
# 16 windows of 7 cols separated by 3-col gutters (max pad of any head)
GUT = 3
GCOLS = 16 * 7 + 15 * GUT       # 157
GROWS = 7


def _cols_gq(t, k):
    dy, dx = t
    p = k // 2
    return (GROWS - abs(dy - p)) * (GCOLS - abs(dx - p))


PLAN_DWQ = {_k: plan_taps(_k, lambda t: _cols_gq(t, _k), g0=1000.0)
            for _k in (3, 5, 7)}
NSLOT = 4


# dw conv engine split: V taps (flat STT), S taps (flat ACT tmps),
# folds mostly on V (flat TT @2x), FOLD_G set folded on GpSimd
SPLIT_DW0 = ([(1, 0), (1, 2), (0, 1)],
             [(0, 0), (0, 2), (2, 0), (2, 1), (2, 2)], {(2, 1)})
SPLIT_DW1 = SPLIT_DW0


PLAN_VR = ([(1, 0), (1, 2), (0, 1)],
           [((0, 0), "V"), ((0, 2), "V"), ((2, 0), "V"), ((2, 1), "G"),
            ((2, 2), "V")])


def emit_conv_vr(nc, wt, bt, src784, acc, tmp_fn, final_fn):
    """3x3 valid-region depthwise conv on a spatial [128, 784] tile (v3)."""
    p = 1
    v_taps, s_taps = PLAN_VR

    def dst_v(base, t):
        dy, dx = t
        y0, y1 = _region(RES, dy - p)
        x0, x1 = _region(RES, dx - p)
        return base[:].rearrange("p (h w) -> p h w", h=RES)[:, y0:y1, x0:x1]

    def src_v(t):
        dy, dx = t
        y0, y1 = _region(RES, dy - p)
        x0, x1 = _region(RES, dx - p)
        return src784.rearrange("p (h w) -> p h w", h=RES)[
            :, y0 + dy - p:y1 + dy - p, x0 + dx - p:x1 + dx - p]

    def w_(t):
        return wt[:, (t[0] * 3 + t[1]):(t[0] * 3 + t[1]) + 1]

    nc.vector.tensor_scalar(dst_v(acc, (1, 1)), src_v((1, 1)), w_((1, 1)),
                            bt[:, 0:1], AO.mult, AO.add)
    vq = list(v_taps)
    for i, (t, feng) in enumerate(s_taps):
        tm = tmp_fn(i % NSLOT)
        nc.scalar.activation(dst_v(tm, t), src_v(t), AF.Identity, scale=w_(t))
        av, tv = dst_v(acc, t), dst_v(tm, t)
        if feng == "G":
            nc.gpsimd.tensor_tensor(av, tv, av, AO.add)
        else:
            nc.vector.tensor_tensor(av, tv, av, AO.add)
        if vq:
            t2 = vq.pop(0)
            nc.vector.scalar_tensor_tensor(dst_v(acc, t2), src_v(t2), w_(t2),
                                           dst_v(acc, t2), AO.mult, AO.add)
    for t2 in vq:
        nc.vector.scalar_tensor_tensor(dst_v(acc, t2), src_v(t2), w_(t2),
                                       dst_v(acc, t2), AO.mult, AO.add)
    final_fn()


def emit_conv_dw(nc, wt, bt, g, g2, acc, tmp_fn, final_fn, split):
    """3x3 depthwise conv, flat shifted taps over a [128,30,32] padded grid.

    g2 is g shifted left one column (keeps even element offsets for the
    odd-dx taps). acc is [128, 28, 32]; flat cols 0..891 hold the interior.
    """
    DW_V_TAPS, DW_S_TAPS, DW_FOLD_G = split
    gf = g[:].rearrange("p h w -> p (h w)")
    g2f = g2[:].rearrange("p h w -> p (h w)") if g2 is not None else None
    af = acc[:].rearrange("p h w -> p (h w)")

    def src(t):
        dy, dx = t
        if g2f is not None and dx == 1:
            return g2f[:, 32 * dy:32 * dy + 892]
        return gf[:, 32 * dy + dx:32 * dy + dx + 892]

    def w_(t):
        return wt[:, (t[0] * 3 + t[1]):(t[0] * 3 + t[1]) + 1]

    nc.vector.tensor_scalar(af[:, 0:892], src((1, 1)), w_((1, 1)),
                            bt[:, 0:1], AO.mult, AO.add)
    vq = list(DW_V_TAPS)
    for i, t in enumerate(DW_S_TAPS):
        tm = tmp_fn(i % NSLOT)
        tf = tm[:].rearrange("p h w -> p (h w)") if len(tm.shape) == 3 else tm[:]
        nc.scalar.activation(tf[:, 0:892], src(t), AF.Identity, scale=w_(t))
        if t in DW_FOLD_G:
            nc.gpsimd.tensor_tensor(af[:, 0:892], tf[:, 0:892], af[:, 0:892],
                                    AO.add)
        else:
            nc.vector.tensor_tensor(af[:, 0:892], tf[:, 0:892], af[:, 0:892],
                                    AO.add)
        if vq:
            t2 = vq.pop(0)
            nc.vector.scalar_tensor_tensor(af[:, 0:892], src(t2), w_(t2),
                                           af[:, 0:892], AO.mult, AO.add)
    for t2 in vq:
        nc.vector.scalar_tensor_tensor(af[:, 0:892], src(t2), w_(t2),
                                       af[:, 0:892], AO.mult, AO.add)
    final_fn()


def emit_conv_grid(nc, k, wt, bt, G, GA, tmp_fn, warm_fn):
    """k*k depthwise conv on the shared guttered macro-grid [128, 7, 157]."""
    p = k // 2
    v_taps, s_taps = PLAN_DWQ[k]

    def dst_v(base, t):
        dy, dx = t
        y0, y1 = _region(GROWS, dy - p)
        x0, x1 = _region(GCOLS, dx - p)
        return base[:, y0:y1, x0:x1]

    def src_v(t):
        dy, dx = t
        y0, y1 = _region(GROWS, dy - p)
        x0, x1 = _region(GCOLS, dx - p)
        return G[:, y0 + dy - p:y1 + dy - p, x0 + dx - p:x1 + dx - p]

    def wcol(t):
        return t[0] * k + t[1]

    nc.vector.tensor_scalar(dst_v(GA[:], (p, p)), src_v((p, p)),
                            wt[:, wcol((p, p)):wcol((p, p)) + 1],
                            bt[:, 0:1], AO.mult, AO.add)
    vq = list(v_taps)
    for i, (t, feng) in enumerate(s_taps):
        tm = tmp_fn(i % NSLOT)
        nc.scalar.activation(dst_v(tm[:], t), src_v(t), AF.Identity,
                             scale=wt[:, wcol(t):wcol(t) + 1])
        av, tv = dst_v(GA[:], t), dst_v(tm[:], t)
        if feng == "G":
            nc.gpsimd.tensor_tensor(av, tv, av, AO.add)
        else:
            nc.vector.tensor_tensor(av, tv, av, AO.add)
        if warm_fn is not None and i % 2 == 0:
            warm_fn(tm, t)
        if vq:
            t2 = vq.pop(0)
            nc.vector.scalar_tensor_tensor(dst_v(GA[:], t2), src_v(t2),
                                           wt[:, wcol(t2):wcol(t2) + 1],
                                           dst_v(GA[:], t2), AO.mult, AO.add)
    for t2 in vq:
        nc.vector.scalar_tensor_tensor(dst_v(GA[:], t2), src_v(t2),
                                       wt[:, wcol(t2):wcol(t2) + 1],
                                       dst_v(GA[:], t2), AO.mult, AO.add)


# ---------------------------------------------------------------------------
# program builder
# ---------------------------------------------------------------------------

def build_program():
    nc = bacc.Bacc("TRN2", target_bir_lowering=False, debug=False,
                   enable_asserts=False, num_devices=NCORES)

    def din(name, shape, dt=f32):
        return nc.dram_tensor(name, list(shape), dt, kind="ExternalInput").ap()

    x_d = din("x", [NI, ED, POS], bf16)
    dwpk_d = din("dwpk", [128, 80])
    w1T0_d = din("w1T0", [ED, 2 * ED], bf16)
    b1f0_d = din("b1f0", [2 * ED], bf16)
    w2T0_d = din("w2T0", [2 * ED, ED], bf16)
    attkq_d = din("attkq", [128, NH * 2 * KD], bf16)
    attwv_d = din("attwv", [128, NH * D], bf16)
    attbias_d = din("attbias", [128, 2 * NH])
    dwqw_d = din("dwqw", [128, NH * 50])
    ab_d = din("ab", [WN, NH * 392], bf16)
    iab_d = din("iab", [WN, 128], bf16)
    ones2_d = din("ones2", [128, 2], bf16)
    sel2_d = din("sel2", [2, 128])
    projT_d = din("projT", [DH, ED], bf16)
    bpk_d = din("bpk", [128, 16])
    w1T1_d = din("w1T1", [ED, 2 * ED], bf16)
    b1f1_d = din("b1f1", [2 * ED], bf16)
    w2T1_d = din("w2T1", [2 * ED, ED], bf16)

    out_d = nc.dram_tensor("out", [NI, ED, POS], f32, kind="ExternalOutput").ap()

    with tile.TileContext(nc) as tc:
        _body(tc, nc, x_d, dwpk_d, w1T0_d, b1f0_d, w2T0_d,
              attkq_d, attwv_d, attbias_d, dwqw_d, ab_d,
              iab_d, ones2_d, sel2_d,
              projT_d, bpk_d,
              w1T1_d, b1f1_d, w2T1_d, out_d)

    nc.compile()
    return nc


def win_ap(ap392, n2, w, spatial):
    """Per-window [*, 49] AP from a 392-col half. spatial: 3D 7x7 slice of
    the 14x28 spatial half; else dense 49-block (window-block layout)."""
    if spatial:
        al, b = w // 4, w % 4
        v = ap392.rearrange("p (h x) -> p h x", h=14)
        return v[:, 7 * al:7 * al + 7, 7 * b:7 * b + 7]
    return ap392[:, WN * w:WN * (w + 1)]


def _body(tc, nc, x_d, dwpk_d, w1T0_d, b1f0_d, w2T0_d,
          attkq_d, attwv_d, attbias_d, dwqw_d, ab_d,
          iab_d, ones2_d, sel2_d,
          projT_d, bpk_d,
          w1T1_d, b1f1_d, w2T1_d, out_d):

    # ---------------- persistent pools -------------------------------------
    wp_cm = tc.tile_pool(name="wp", bufs=1)
    wp = wp_cm.__enter__()
    big_cm = tc.tile_pool(name="big", bufs=1)
    big = big_cm.__enter__()
    xw_cm = tc.tile_pool(name="xw", bufs=1)
    xwp = xw_cm.__enter__()

    def load_mlp_w(w1T_dram, b1_dram, w2T_dram, b2_dram, pool):
        w1sb = []
        for k in range(4):
            w = pool.tile([128, 2 * ED], bf16, tag=f"w1_{k}")
            nc.sync.dma_start(out=w, in_=w1T_dram[128 * k:128 * (k + 1), :])
            w1sb.append(w)
        w2sb = []
        for k in range(8):
            w = pool.tile([128, ED], bf16, tag=f"w2_{k}")
            nc.sync.dma_start(out=w, in_=w2T_dram[128 * k:128 * (k + 1), :])
            w2sb.append(w)
        b1row = pool.tile([1, 2 * ED], bf16, tag="b1row")
        nc.sync.dma_start(out=b1row, in_=b1_dram.unsqueeze(0))
        b2sb = []
        for m in range(4):
            b = pool.tile([128, 1], f32, tag=f"b2_{m}")
            nc.sync.dma_start(out=b, in_=b2_dram[128 * m:128 * (m + 1)].unsqueeze(1))
            b2sb.append(b)
        return w1sb, w2sb, b1row, b2sb

    # dw weights
    dw_w, dw_b = {}, {}
    for nm, wd, bd in (("dw0", dw0w_d, dw0b_d), ("dw1", dw1w_d, dw1b_d)):
        ws_, bs_ = [], []
        for c in range(4):
            w = wp.tile([128, 9], f32, tag=f"{nm}w{c}")
            nc.sync.dma_start(out=w, in_=wd[c])
            b = wp.tile([128, 1], f32, tag=f"{nm}b{c}")
            nc.sync.dma_start(out=b, in_=bd[c].unsqueeze(1))
            ws_.append(w)
            bs_.append(b)
        dw_w[nm], dw_b[nm] = ws_, bs_

    ones392 = wp.tile([1, 392], bf16, tag="ones392")
    nc.vector.memset(ones392, 1.0)
    acth = wp.tile([128, 1], f32, tag="acth")
    nc.vector.memset(acth, 0.5)
    acts = wp.tile([128, 1], f32, tag="acts")
    nc.vector.memset(acts, 1.0 / 6.0)

    # attention weights
    wkq_sb, bkq_sb, wv_sb, bv_sb, dq_w, dq_b, ab_sb = [], [], [], [], [], [], []
    for h in range(NH):
        t = wp.tile([128, 2 * KD], bf16, tag=f"wkq{h}")
        nc.sync.dma_start(out=t[0:64, :], in_=wkqT_d[h])
        nc.sync.dma_start(out=t[64:128, :], in_=wkqT_d[h])
        wkq_sb.append(t)
        t = wp.tile([128, 1], f32, tag=f"bkq{h}")
        nc.sync.dma_start(out=t[0:32, :], in_=bkq_d[h].unsqueeze(1))
        nc.sync.dma_start(out=t[64:96, :], in_=bkq_d[h].unsqueeze(1))
        bkq_sb.append(t)
        t = wp.tile([128, D], bf16, tag=f"wv{h}")
        nc.sync.dma_start(out=t[0:64, :], in_=wvT_d[h])
        nc.sync.dma_start(out=t[64:128, :], in_=wvT_d[h])
        wv_sb.append(t)
        t = wp.tile([128, 1], f32, tag=f"bv{h}")
        nc.sync.dma_start(out=t[0:64, :], in_=bv_d[h].unsqueeze(1))
        nc.sync.dma_start(out=t[64:128, :], in_=bv_d[h].unsqueeze(1))
        bv_sb.append(t)

        t = wp.tile([128, 49], f32, tag=f"dqw{h}")
        nc.sync.dma_start(out=t, in_=dwqw_d[h])
        dq_w.append(t)
        t = wp.tile([128, 1], f32, tag=f"dqb{h}")
        nc.sync.dma_start(out=t, in_=dwqb_d[h].unsqueeze(1))
        dq_b.append(t)
        t = wp.tile([WN, 392], bf16, tag=f"ab{h}")
        nc.sync.dma_start(out=t, in_=ab_d[h])
        ab_sb.append(t)
    iab_sb = wp.tile([WN, 128], bf16, tag="iab")
    nc.sync.dma_start(out=iab_sb, in_=iab_d)
    ones2_sb = wp.tile([128, 2], bf16, tag="ones2")
    nc.sync.dma_start(out=ones2_sb, in_=ones2_d)
    sel2_sb = wp.tile([2, 128], f32, tag="sel2")
    nc.sync.dma_start(out=sel2_sb, in_=sel2_d)

    # proj
    pj_sb = []
    for k in range(4):
        w = wp.tile([128, ED], bf16, tag=f"pj{k}")
        nc.sync.dma_start(out=w, in_=projT_d[128 * k:128 * (k + 1), :])
        pj_sb.append(w)
    pjb_sb, yb_sb = [], []
    for m in range(4):
        b = wp.tile([128, 1], f32, tag=f"pjb{m}")
        nc.sync.dma_start(out=b, in_=projb_d[128 * m:128 * (m + 1)].unsqueeze(1))
        pjb_sb.append(b)
        b = wp.tile([128, 1], f32, tag=f"ybt{m}")
        nc.sync.dma_start(out=b, in_=yb_d[128 * m:128 * (m + 1)].unsqueeze(1))
        yb_sb.append(b)

    w1sb0, w2sb0, b1row0, b2sb0 = load_mlp_w(w1T0_d, b1f0_d, w2T0_d, b2f0_d, wp)

    # ---------------- MLP per-image emitter --------------------------------
    def mlp_img(pools, img, w1sb, w2sb, b1row, b2sb, rhs_getter, out_writer, name):
        hp, rp, psp, pop = pools
        hs = []
        for m in range(8):
            h = hp.tile([128, POS], bf16, tag=f"h{m}", name=f"{name}h{m}_{img}")
            hs.append(h)
        for m in range(8):
            for n2 in range(2):
                ph = psp.tile([128, 392], f32, tag="ph")
                for k in range(4):
                    nc.tensor.matmul(
                        ph[:], w1sb[k][:, 128 * m:128 * (m + 1)],
                        rhs_getter(k, img, n2),
                        start=(k == 0), stop=False)
                nc.tensor.matmul(
                    ph[:], b1row[:, 128 * m:128 * (m + 1)],
                    ones392[:], start=False, stop=True)
                r = rp.tile([128, 392], bf16, tag="relu")
                nc.scalar.activation(r[:], ph[:], AF.Relu,
                                     scale=acts[:, 0:1], bias=acth[:, 0:1])
                nc.vector.scalar_tensor_tensor(
                    hs[m][:, 392 * n2:392 * (n2 + 1)], r[:], 1.0,
                    ph[:], AO.min, AO.mult)
        for mo in range(4):
            for n2 in range(2):
                po = pop.tile([128, 392], f32, tag="po")
                for k in range(8):
                    nc.tensor.matmul(
                        po[:], w2sb[k][:, 128 * mo:128 * (mo + 1)],
                        hs[k][:, 392 * n2:392 * (n2 + 1)],
                        start=(k == 0), stop=(k == 7))
                out_writer(mo, img, n2, po, b2sb[mo])

    # ---------------- P0+P1: input DMA + dw0 + residual -> x1 (spatial) -----
    x1fl = [big.tile([128, NI, POS], bf16, tag=f"fl{c}", name=f"x1_{c}")
            for c in range(4)]

    def rhs0(k, img, n2):
        return x1fl[k][:, img, 392 * n2:392 * (n2 + 1)]

    def outw0(mo, img, n2, po, b2):
        ov = x1fl[mo][:, img, 392 * n2:392 * (n2 + 1)]
        nc.vector.scalar_tensor_tensor(ov, po[:], b2[:, 0:1], ov, AO.add, AO.add)

    GN = 3
    with tc.tile_pool(name="stg", bufs=4) as stgp, \
         tc.tile_pool(name="dac", bufs=3) as dacp, \
         tc.tile_pool(name="dwt0", bufs=2) as tmp0, \
         tc.tile_pool(name="m0h", bufs=2) as hp0, \
         tc.tile_pool(name="m0r", bufs=4) as rp0, \
         tc.tile_pool(name="m0ps", bufs=4, space="PSUM") as psp0, \
         tc.tile_pool(name="m0po", bufs=2, space="PSUM") as pop0:
        for img in range(NI):
            for c in range(4):
                stg = stgp.tile([128, POS], bf16, tag="stg",
                                name=f"stg{c}_{img}")
                nc.scalar.dma_start(out=stg,
                                    in_=x_d[img, 128 * c:128 * (c + 1), :])
                acc = dacp.tile([128, POS], bf16, tag="acc",
                                name=f"a0_{c}_{img}")

                def tmp_fn(i):
                    return tmp0.tile([128, POS], bf16, tag=f"tm{i}",
                                     name=f"tm0_{c}_{img}_{i}")

                def final_fn():
                    nc.vector.tensor_tensor(x1fl[c][:, img, :], acc[:],
                                            stg[:], AO.add)

                emit_conv_vr(nc, dw_w["dw0"][c], dw_b["dw0"][c], stg[:],
                             acc, tmp_fn, final_fn)
            mlp_img((hp0, rp0, psp0, pop0), img, w1sb0, w2sb0, b1row0, b2sb0,
                    rhs0, outw0, "m0")

    x2fl = x1fl   # trunk now holds x2 (spatial, bf16)

    # ---------------- P3: cascaded attention -> y_sb ------------------------
    # y in window-block layout: y_sb[c][64*h2+d, img, 49*w + pos]
    y_sb = [xwp.tile([128, NI, POS], bf16, tag=f"wm{c}", name=f"y_{c}")
            for c in range(4)]

    def prow(i):
        return 64 * (i % 2)

    with ExitStack() as es:
        spkp = es.enter_context(tc.tile_pool(name="spk", bufs=1))
        spxp = es.enter_context(tc.tile_pool(name="spx", bufs=2))
        spp = es.enter_context(tc.tile_pool(name="sp", bufs=1))
        kqtp = es.enter_context(tc.tile_pool(name="kqt", bufs=1))
        kpkp = es.enter_context(tc.tile_pool(name="kpk", bufs=1))
        vtp = es.enter_context(tc.tile_pool(name="vt", bufs=1))
        qgp = es.enter_context(tc.tile_pool(name="qg", bufs=1))
        qgrp = es.enter_context(tc.tile_pool(name="qgr", bufs=1))
        qgap = es.enter_context(tc.tile_pool(name="qga", bufs=1))
        qtmp = es.enter_context(tc.tile_pool(name="qtm", bufs=1))
        attp = es.enter_context(tc.tile_pool(name="att", bufs=2))
        rsp = es.enter_context(tc.tile_pool(name="rsp", bufs=1))
        spop = es.enter_context(tc.tile_pool(name="spo", bufs=1))
        pkqp = es.enter_context(tc.tile_pool(name="pkq", bufs=1, space="PSUM"))
        pvtp = es.enter_context(tc.tile_pool(name="pvt", bufs=1, space="PSUM"))
        pap = es.enter_context(tc.tile_pool(name="pa", bufs=2, space="PSUM"))
        ps1p = es.enter_context(tc.tile_pool(name="ps1", bufs=1, space="PSUM"))
        pbcp = es.enter_context(tc.tile_pool(name="pbc", bufs=1, space="PSUM"))
        pavp = es.enter_context(tc.tile_pool(name="pav", bufs=2, space="PSUM"))

        spx_tiles = {}

        def fetch_spx(h):
            """Stage pair-packed spatial slice via DMA, then repack to
            window-block with V copies (@4x)."""
            c, h2 = h // 2, h % 2
            t = spkp.tile([128, 4, POS], bf16, tag="spk", name=f"spk{h}")
            xv = x2fl[c][64 * h2:64 * h2 + 64, :, :] \
                .rearrange("p (j t) x -> p t j x", t=2)
            for par in range(2):
                nc.gpsimd.dma_start(out=t[64 * par:64 * par + 64, :, :],
                                    in_=xv[:, par, :, :])
            twb = spxp.tile([128, 4, POS], bf16, tag="spxwb", name=f"spxwb{h}")
            for j in range(4):
                for n2 in range(2):
                    co = 392 * n2
                    for w in range(8):
                        nc.vector.tensor_copy(
                            twb[:, j, co + WN * w:co + WN * (w + 1)]
                            .rearrange("p (x y) -> p x y", x=7),
                            win_ap(t[:, j, co:co + 392], n2, w, spatial=True))
            spx_tiles[h] = twb

        # shared q-conv macro-grid: gutters zeroed once, window interiors
        # overwritten per head by the pack copies
        Gq = qgrp.tile([128, GROWS, GCOLS], bf16, tag="qpad", name="qpad")
        nc.gpsimd.memset(Gq[:], 0.0)

        fetch_spx(0)
        sp_all = spx_tiles[0]
        for h in range(NH):
            c, h2 = h // 2, h % 2
            if h + 1 < NH:
                fetch_spx(h + 1)

            kqt = kqtp.tile([128, 4, POS], bf16, tag="kqt", name=f"kqt{h}")
            k_pk = kpkp.tile([128, 2, POS], bf16, tag="k", name=f"k{h}")
            qstack = qgp.tile([128, POS], bf16, tag="qstack", name=f"qstack{h}")
            qp_pk = kpkp.tile([128, 2, POS], bf16, tag="qp", name=f"qp{h}")
            vt_pk = vtp.tile([128, 4 * 1024], bf16, tag="vt", name=f"vt{h}")

            # ---- A/B: kqv matmuls + evict + repack DMAs ----
            for j in range(4):          # image pairs (2j, 2j+1)
                for n2 in range(2):
                    pkq = pkqp.tile([128, 392], f32, tag="pkq",
                                    name=f"pkq{h}_{j}_{n2}")
                    pvt = pvtp.tile([128, 512], f32, tag="pvt",
                                    name=f"pvt{h}_{j}_{n2}")
                    for t_ in range(2):
                        img = 2 * j + t_
                        ob = 64 * t_
                        rhs_base = prow(img)
                        spi = sp_all[rhs_base:rhs_base + 64, img // 2,
                                     392 * n2:392 * (n2 + 1)]
                        nc.tensor.matmul(
                            pkq[ob:ob + 2 * KD, :],
                            wkq_sb[h][rhs_base:rhs_base + 64, :],
                            spi, start=True, stop=True,
                            tile_position=(rhs_base, ob))
                        for w in range(8):
                            nc.tensor.matmul(
                                pvt[ob:ob + WN, 64 * w:64 * (w + 1)],
                                spi[:, WN * w:WN * (w + 1)],
                                wv_sb[h][rhs_base:rhs_base + 64, :],
                                start=True, stop=True,
                                tile_position=(rhs_base, ob))
                    nc.scalar.activation(kqt[:, j, 392 * n2:392 * (n2 + 1)],
                                         pkq[:], AF.Identity,
                                         bias=bkq_sb[h][:, 0:1])
                    nc.scalar.activation(
                        vt_pk[:, 1024 * j + 512 * n2:1024 * j + 512 * (n2 + 1)],
                        pvt[:], AF.Copy)
                for t_ in range(2):
                    img = 2 * j + t_
                    rb = 64 * t_
                    nc.gpsimd.dma_start(
                        out=k_pk[32 * (img % 4):32 * (img % 4) + KD, img // 4, :],
                        in_=kqt[rb:rb + KD, j, :])
                    nc.gpsimd.dma_start(
                        out=qstack[KD * img:KD * (img + 1), :],
                        in_=kqt[rb + KD:rb + 2 * KD, j, :])

            # ---- C: depthwise conv on stacked q (shared guttered grid) ----
            kk = KS[h]
            qsv = qstack[:].rearrange("p (n s) -> p n s", n=NW)
            for w in range(NW):
                nc.vector.tensor_copy(
                    Gq[:, :, (7 + GUT) * w:(7 + GUT) * w + 7],
                    qsv[:, w, :].rearrange("p (x y) -> p x y", x=7))
            GA = qgap.tile([128, GROWS, GCOLS], bf16, tag="qacc",
                           name=f"qacc{h}")
            warm_ps = pbcp.tile([128, 392], f32, tag="pbc", name=f"warm{h}")

            def tmp_q(i):
                return qtmp.tile([128, GROWS, GCOLS], bf16, tag=f"qtm{i}",
                                 name=f"qtm{h}_{i}")

            def warm_fn(tm, t):
                p_ = kk // 2
                y0, _ = _region(GROWS, t[0] - p_)
                x0, _ = _region(GCOLS, t[1] - p_)
                nc.tensor.matmul(
                    warm_ps[0:2, 0:WN], ones2_sb[:],
                    tm[:, y0, x0:x0 + WN],
                    start=True, stop=True, tile_position=(0, 0))

            emit_conv_grid(nc, kk, dq_w[h], dq_b[h], Gq[:], GA, tmp_q, warm_fn)
            # unpack to window-block layout
            qflat = qgp.tile([128, NW, WN], bf16, tag="qflat", name=f"qflat{h}")
            for w in range(NW):
                nc.vector.tensor_copy(
                    qflat[:, w, :].rearrange("p (x y) -> p x y", x=7),
                    GA[:, :, (7 + GUT) * w:(7 + GUT) * w + 7])
            for img in range(NI):
                nc.gpsimd.dma_start(
                    out=qp_pk[32 * (img % 4):32 * (img % 4) + KD, img // 4, :],
                    in_=qflat[KD * img:KD * (img + 1)].rearrange("q n s -> q (n s)"))

            # ---- D: attention per pair ----
            spn = None
            if h + 1 < NH:
                spn = spp.tile([128, 4, POS], bf16, tag="sp", name=f"sp{h + 1}")
            spo_all = spop.tile([128, 4, POS], bf16, tag="spo", name=f"spo{h}")
            for j in range(4):
                for n2 in range(2):
                    pa = pap.tile([128, 392], f32, tag="pa", name=f"pa{h}_{j}_{n2}")
                    for t_ in range(2):
                        img = 2 * j + t_
                        ob = 64 * t_
                        q0 = 32 * (img % 4)
                        kh = k_pk[q0:q0 + KD, img // 4, :]
                        qh = qp_pk[q0:q0 + KD, img // 4, :]
                        for w in range(8):
                            co_ = 392 * n2 + WN * w
                            nc.tensor.matmul(
                                pa[ob:ob + WN, WN * w:WN * (w + 1)],
                                kh[:, co_:co_ + WN], qh[:, co_:co_ + WN],
                                start=True, stop=False,
                                tile_position=(q0, ob))
                    nc.tensor.matmul(pa[:], iab_sb[:], ab_sb[h][:],
                                     start=False, stop=True,
                                     tile_position=(0, 0))
                    ein = attp.tile([128, 392], bf16, tag="ein",
                                    name=f"ein{h}_{j}_{n2}")
                    nc.scalar.activation(ein[:], pa[:], AF.Exp)
                    ps1 = ps1p.tile([2, 392], f32, tag="ps1",
                                    name=f"ps1{h}_{j}_{n2}")
                    nc.tensor.matmul(ps1[:], ones2_sb[:], ein[:],
                                     start=True, stop=True,
                                     tile_position=(0, 0))
                    rs = rsp.tile([2, 392], f32, tag="rs", name=f"rs{h}_{j}_{n2}")
                    nc.vector.reciprocal_approx_fast(rs[:], ps1[:])
                    pbc = pbcp.tile([128, 392], f32, tag="pbc",
                                    name=f"pbc{h}_{j}_{n2}")
                    nc.tensor.matmul(pbc[:], sel2_sb[:], rs[:],
                                     start=True, stop=True,
                                     tile_position=(0, 0))
                    bc = attp.tile([128, 392], bf16, tag="bc",
                                   name=f"bc{h}_{j}_{n2}")
                    nc.scalar.activation(bc[:], pbc[:], AF.Copy)
                    pav = pavp.tile([128, 392], f32, tag="pav",
                                    name=f"pav{h}_{j}_{n2}")
                    for t_ in range(2):
                        img = 2 * j + t_
                        ob = 64 * t_
                        for w in range(8):
                            wg = 8 * n2 + w
                            nc.tensor.matmul(
                                pav[ob:ob + D, WN * w:WN * (w + 1)],
                                vt_pk[ob:ob + WN,
                                      1024 * j + 64 * wg:1024 * j + 64 * (wg + 1)],
                                ein[ob:ob + WN, WN * w:WN * (w + 1)],
                                start=True, stop=True,
                                tile_position=(ob, ob))
                    co = 392 * n2
                    nc.vector.tensor_tensor(spo_all[:, j, co:co + 392], pav[:],
                                            bc[:], AO.mult)
                    if spn is not None:
                        nc.vector.scalar_tensor_tensor(
                            spn[:, j, co:co + 392],
                            spo_all[:, j, co:co + 392],
                            bv_sb[h][:, 0:1],
                            spx_tiles[h + 1][:, j, co:co + 392],
                            AO.add, AO.add)
            yv = y_sb[c][64 * h2:64 * h2 + 64, :, :] \
                .rearrange("p (j t) x -> p t j x", t=2)
            for t_ in range(2):
                nc.gpsimd.dma_start(out=yv[:, t_, :, :],
                                    in_=spo_all[64 * t_:64 * t_ + 64, :, :])
            sp_all = spn

    # ---------------- P4+P5+P6 fused per image ------------------------------
    # y is window-block; proj output window-block; x3 written spatially (trunk)
    x3fl = x2fl
    x4fl = [xwp.tile([128, NI, POS], bf16, tag=f"wm{c}", name=f"x4_{c}")
            for c in range(4)]
    w1sb1, w2sb1, b1row1 = load_mlp_w(w1T1_d, b1f1_d, w2T1_d, wp)

    def rhs1(k, img, n2):
        return x4fl[k][:, img, 392 * n2:392 * (n2 + 1)]

    with ExitStack() as es:
        hyp = es.enter_context(tc.tile_pool(name="hyp", bufs=2))
        pjrp = es.enter_context(tc.tile_pool(name="pjr", bufs=2))
        dacp = es.enter_context(tc.tile_pool(name="dac1", bufs=3))
        tmp1 = es.enter_context(tc.tile_pool(name="dwt1", bufs=2))
        o5p = es.enter_context(tc.tile_pool(name="o5", bufs=2))
        hp1 = es.enter_context(tc.tile_pool(name="m1h", bufs=2))
        rp1 = es.enter_context(tc.tile_pool(name="m1r", bufs=4))
        ppp = es.enter_context(tc.tile_pool(name="ppp", bufs=2, space="PSUM"))
        psp1 = es.enter_context(tc.tile_pool(name="m1ps", bufs=4, space="PSUM"))
        pop1 = es.enter_context(tc.tile_pool(name="m1po", bufs=2, space="PSUM"))

        def outw1(mo, img, n2, po, b2):
            x5 = o5p.tile([128, 392], f32, tag="x5", name=f"x5_{mo}_{img}_{n2}")
            nc.vector.scalar_tensor_tensor(
                x5[:], po[:], b2[:, 0:1],
                x4fl[mo][:, img, 392 * n2:392 * (n2 + 1)], AO.add, AO.add)
            nc.sync.dma_start(
                out=out_d[img, 128 * mo:128 * (mo + 1), 392 * n2:392 * (n2 + 1)],
                in_=x5[:])

        for img in range(NI):
            # P4: hswish(y + yb), proj, x3 = x2 + proj + pjb
            hys = []
            for cb in range(4):
                yv = y_sb[cb][:, img, :]
                nc.vector.tensor_scalar(yv, yv, yb_sb[cb][:, 0:1], None,
                                        AO.add)
                r = pjrp.tile([128, POS], bf16, tag="pr")
                nc.scalar.activation(r[:], yv, AF.Relu,
                                     scale=acts[:, 0:1], bias=acth[:, 0:1])
                hy = hyp.tile([128, POS], bf16, tag=f"hy{cb}", name=f"hy{cb}_{img}")
                nc.vector.scalar_tensor_tensor(hy[:], r[:], 1.0, yv,
                                               AO.min, AO.mult)
                hys.append(hy)
            for mo in range(4):
                for n2 in range(2):
                    pp = ppp.tile([128, 392], f32, tag="pp")
                    for k in range(4):
                        nc.tensor.matmul(pp[:], pj_sb[k][:, 128 * mo:128 * (mo + 1)],
                                         hys[k][:, 392 * n2:392 * (n2 + 1)],
                                         start=(k == 0), stop=(k == 3))
                    ov = x2fl[mo][:, img, 392 * n2:392 * (n2 + 1)]
                    for w in range(8):
                        nc.vector.scalar_tensor_tensor(
                            win_ap(ov, n2, w, spatial=True),
                            pp[:, WN * w:WN * (w + 1)]
                            .rearrange("p (x y) -> p x y", x=7),
                            pjb_sb[mo][:, 0:1],
                            win_ap(ov, n2, w, spatial=True),
                            AO.add, AO.add)
            # P5: dw1 units for this image (spatial trunk -> x4 spatial)
            for cb in range(4):
                srcv = x3fl[cb][:, img, :]
                acc = dacp.tile([128, POS], bf16, tag="acc",
                                name=f"a1_{cb}_{img}")

                def tmp_fn(i):
                    return tmp1.tile([128, POS], bf16, tag=f"tm{i}",
                                     name=f"tm1_{cb}_{img}_{i}")

                def final_fn():
                    nc.vector.tensor_tensor(x4fl[cb][:, img, :], acc[:],
                                            srcv, AO.add)

                emit_conv_vr(nc, dw_w["dw1"][cb], dw_b["dw1"][cb], srcv,
                             acc, tmp_fn, final_fn)
            # P6: MLP1 for this image
            mlp_img((hp1, rp1, psp1, pop1), img, w1sb1, w2sb1, b1row1, b2sb1,
                    rhs1, outw1, "m1")

    xw_cm.__exit__(None, None, None)
    big_cm.__exit__(None, None, None)
    wp_cm.__exit__(None, None, None)


# ---------------------------------------------------------------------------
# host-side input preprocessing
# ---------------------------------------------------------------------------

def prep_weights(inp):
    def taps(w):  # [C,1,k,k] -> [C, k*k]
        return w.reshape(w.shape[0], -1).astype(np.float32)

    m = {}
    dwpk = np.zeros((128, 80), np.float32)
    for ci in range(4):
        dwpk[:, 10 * ci:10 * ci + 9] = taps(inp["dw0_w"])[128 * ci:128 * (ci + 1)]
        dwpk[:, 10 * ci + 9] = inp["dw0_b"][128 * ci:128 * (ci + 1)]
        dwpk[:, 40 + 10 * ci:40 + 10 * ci + 9] = \
            taps(inp["dw1_w"])[128 * ci:128 * (ci + 1)]
        dwpk[:, 40 + 10 * ci + 9] = inp["dw1_b"][128 * ci:128 * (ci + 1)]
    m["dwpk"] = dwpk
    m["w1T0"] = np.ascontiguousarray(inp["ffn0_w1"].T).astype(ml_dtypes.bfloat16)
    m["b1f0"] = inp["ffn0_b1"].astype(ml_dtypes.bfloat16)
    m["w2T0"] = np.ascontiguousarray(inp["ffn0_w2"].T).astype(ml_dtypes.bfloat16)

    qkv_w, qkv_b = inp["qkv_w"], inp["qkv_b"]
    wkqT = np.empty((NH, D, 2 * KD), np.float32)
    bkq = np.empty((NH, 2 * KD), np.float32)
    wvT = np.empty((NH, D, D), np.float32)
    bv = np.empty((NH, D), np.float32)
    for h in range(NH):
        W = qkv_w[h]  # [96, 64]
        wkqT[h, :, 0:KD] = W[KD:2 * KD].T       # k
        wkqT[h, :, KD:2 * KD] = W[0:KD].T       # q
        bkq[h, 0:KD] = qkv_b[h, KD:2 * KD]
        bkq[h, KD:2 * KD] = qkv_b[h, 0:KD]
        wvT[h] = W[2 * KD:].T
        bv[h] = qkv_b[h, 2 * KD:]
    # packed: [128, NH*32] kq weights (row halves duplicated)
    akq = np.empty((128, NH * 2 * KD), np.float32)
    awv = np.empty((128, NH * D), np.float32)
    abias = np.zeros((128, 2 * NH), np.float32)
    for h in range(NH):
        akq[0:64, 32 * h:32 * h + 32] = wkqT[h]
        akq[64:128, 32 * h:32 * h + 32] = wkqT[h]
        awv[0:64, 64 * h:64 * h + 64] = wvT[h]
        awv[64:128, 64 * h:64 * h + 64] = wvT[h]
        abias[0:32, 2 * h] = bkq[h]
        abias[64:96, 2 * h] = bkq[h]
        abias[0:64, 2 * h + 1] = bv[h]
        abias[64:128, 2 * h + 1] = bv[h]
    m["attkq"] = akq.astype(ml_dtypes.bfloat16)
    m["attwv"] = awv.astype(ml_dtypes.bfloat16)
    m["attbias"] = abias

    dwq_ws = [inp["dwq_w7"], inp["dwq_w5"]] + [inp["dwq_w3"][i] for i in range(6)]
    dwq_bs = [inp["dwq_b7"], inp["dwq_b5"]] + [inp["dwq_b3"][i] for i in range(6)]
    dwqw = np.zeros((128, NH * 50), np.float32)
    for h in range(NH):
        t = taps(dwq_ws[h]) * SCALE
        nt = t.shape[1]
        for i in range(NI):
            dwqw[KD * i:KD * (i + 1), 50 * h:50 * h + nt] = t
            dwqw[KD * i:KD * (i + 1), 50 * h + 49] = dwq_bs[h] * SCALE
    m["dwqw"] = dwqw

    ab = inp["attn_bias"][:, BIAS_IDX]       # [NH, 49, 49]
    ab = np.tile(ab, (1, 1, 8))              # [NH, 49, 392]
    m["ab"] = ab.transpose(1, 0, 2).reshape(WN, NH * 392).copy() \
        .astype(ml_dtypes.bfloat16)

    iab = np.zeros((WN, 128), np.float32)
    for i in range(WN):
        iab[i, i] = 1.0
        iab[i, 64 + i] = 1.0
    m["iab"] = iab.astype(ml_dtypes.bfloat16)
    ones2 = np.zeros((128, 2), np.float32)
    ones2[0:WN, 0] = 1.0
    ones2[64:64 + WN, 1] = 1.0
    m["ones2"] = ones2.astype(ml_dtypes.bfloat16)
    sel2 = np.zeros((2, 128), np.float32)
    sel2[0, 0:64] = 1.0
    sel2[1, 64:128] = 1.0
    m["sel2"] = sel2

    m["projT"] = np.ascontiguousarray(inp["proj_w"].T).astype(ml_dtypes.bfloat16)
    bpk = np.zeros((128, 16), np.float32)
    for ci in range(4):
        bpk[:, ci] = inp["proj_b"][128 * ci:128 * (ci + 1)]
        bpk[:, 4 + ci] = inp["ffn0_b2"][128 * ci:128 * (ci + 1)]
        bpk[:, 8 + ci] = inp["ffn1_b2"][128 * ci:128 * (ci + 1)]
        bpk[:, 12 + ci] = bv.reshape(ED)[128 * ci:128 * (ci + 1)]
    m["bpk"] = bpk

    m["w1T1"] = np.ascontiguousarray(inp["ffn1_w1"].T).astype(ml_dtypes.bfloat16)
    m["b1f1"] = inp["ffn1_b1"].astype(ml_dtypes.bfloat16)
    m["w2T1"] = np.ascontiguousarray(inp["ffn1_w2"].T).astype(ml_dtypes.bfloat16)
    return m


@functools.lru_cache(maxsize=1)
def _cached_program():
    return build_program()


def _run(inputs, trace=False, **kw):
    nc = _cached_program()
    wm = prep_weights(inputs)
    x = np.asarray(inputs["x"], dtype=np.float32).reshape(64, ED, POS)
    x = x.astype(ml_dtypes.bfloat16)
    in_maps = []
    for core in range(NCORES):
        im = dict(wm)
        im["x"] = np.ascontiguousarray(x[NI * core:NI * (core + 1)])
        in_maps.append(im)
    res = bass_utils.run_bass_kernel_spmd(nc, in_maps, list(range(NCORES)),
                                          trace=trace, **kw)
    out = np.concatenate([r["out"] for r in res.results], axis=0)
    return out.reshape(64, ED, RES, RES).astype(np.float32), res


def kernel(**inputs):
    out, _ = _run(inputs)
    return out


# revision 76
# speedup vs baseline: 1.1384x; 1.0485x over previous
"""Trainium2 Bass kernel for nn_BasicBlock (EfficientViT-style block), v3.

Data-parallel over 8 NeuronCores: batch 64 -> 8 images/core.
SBUF-resident bf16 spatial trunk, no DRAM intermediates.
Depthwise convs via valid-region shifted views, split across V/S/G engines.
Per-core program: dw0 -> MLP0 -> cascaded window attention -> proj -> dw1 -> MLP1.
"""
import itertools
import functools
from contextlib import ExitStack
import numpy as np
import ml_dtypes

import concourse.bass as bass
import concourse.mybir as mybir
import concourse.tile as tile
from concourse import bacc
from concourse import bass_utils

f32 = mybir.dt.float32
bf16 = mybir.dt.bfloat16
AO = mybir.AluOpType
AF = mybir.ActivationFunctionType

ED, KD, NH, AR = 512, 16, 8, 4
D = AR * KD            # 64
DH = D * NH            # 512
RES, WS = 28, 7
SCALE = KD ** -0.5
KS = [7, 5, 3, 3, 3, 3, 3, 3]
NI = 8                 # images per core
NCORES = 8
POS = RES * RES        # 784
NW = 16                # windows per image
WN = WS * WS           # 49


def _bias_idx(ws):
    pts = list(itertools.product(range(ws), range(ws)))
    offs, idxs = {}, []
    for p1 in pts:
        for p2 in pts:
            o = (abs(p1[0] - p2[0]), abs(p1[1] - p2[1]))
            if o not in offs:
                offs[o] = len(offs)
            idxs.append(offs[o])
    return np.array(idxs, dtype=np.int32).reshape(ws * ws, ws * ws), len(offs)


BIAS_IDX, N_OFFS = _bias_idx(WS)


# ---------------------------------------------------------------------------
# conv planning (engine split)
# ---------------------------------------------------------------------------

def _region(sz, d):
    """1D dst range [y0,y1) for shift d (src index = dst + d)."""
    return max(0, -d), sz - max(0, d)


def _plan_taps1(k, cols, fold_engs, desc, g0):
    """Greedy engine split for k*k taps. cols(t) -> per-tap column count.

    V taps: STT accumulate in place. S taps: ACT into a cycling tmp slot,
    folded into the accumulator by a V tensor_tensor (@2x) or a G
    tensor_tensor (slow but off the critical engines).
    Returns ((v_taps, s_taps), wall); s_taps entries are (tap, fold_engine).
    """
    p = k // 2
    center = (p, p)
    ccols = cols(center)
    busy = {"V": ccols * 0.26 + 105 + ccols * 0.52 + 105, "S": 0.0, "G": g0}
    v_taps, s_taps = [], []
    order = sorted([t for t in itertools.product(range(k), range(k))
                    if t != center], key=lambda t: (-cols(t) if desc else cols(t)))
    for t in order:
        c = cols(t)
        cand = {}
        nb = dict(busy)
        nb["V"] = busy["V"] + c * 1.042 + 105
        cand["V"] = max(nb.values())
        if "V" in fold_engs:
            nb = dict(busy)
            nb["S"] = busy["S"] + c * 0.833 + 217
            nb["V"] = busy["V"] + c * 0.52 + 105
            cand["SV"] = max(nb.values())
        if "G" in fold_engs:
            nb = dict(busy)
            nb["S"] = busy["S"] + c * 0.833 + 217
            nb["G"] = busy["G"] + c * 1.98 + 156
            cand["SG"] = max(nb.values())
        eng = min(cand, key=lambda e: cand[e])
        if eng == "V":
            v_taps.append(t)
            busy["V"] += c * 1.042 + 105
        elif eng == "SV":
            s_taps.append((t, "V"))
            busy["S"] += c * 0.833 + 217
            busy["V"] += c * 0.52 + 105
        else:
            s_taps.append((t, "G"))
            busy["S"] += c * 0.833 + 217
            busy["G"] += c * 1.98 + 156
    return (v_taps, s_taps), max(busy.values())


def plan_taps(k, cols, g0=0.0):
    best = None
    for folds in (("V",), ("G",), ("V", "G")):
        for desc in (True, False):
            plan, wall = _plan_taps1(k, cols, folds, desc, g0)
            if best is None or wall < best[1]:
                best = (plan, wall)
    return best[0]


def _cols_sp(t, k):
    dy, dx = t
    p = k // 2
    return (RES - abs(dy - p)) * (RES - abs(dx - p))


PLAN_DW = plan_taps(3, lambda t: _cols_sp(t, 3), g0=1000.0)

# shared 1x16 guttered macro-grid for the per-head q convs: 7 rows,
# 16 windows of 7 cols separated by 3-col gutters (max pad of any head)
GUT = 3
GCOLS = 16 * 7 + 15 * GUT       # 157
GROWS = 7


def _cols_gq(t, k):
    dy, dx = t
    p = k // 2
    return (GROWS - abs(dy - p)) * (GCOLS - abs(dx - p))


PLAN_DWQ = {_k: plan_taps(_k, lambda t: _cols_gq(t, _k), g0=1000.0)
            for _k in (3, 5, 7)}
NSLOT = 4


# dw conv engine split: V taps (flat STT), S taps (flat ACT tmps),
# folds mostly on V (flat TT @2x), FOLD_G set folded on GpSimd
SPLIT_DW0 = ([(1, 0), (1, 2), (0, 1)],
             [(0, 0), (0, 2), (2, 0), (2, 1), (2, 2)], {(2, 1)})
SPLIT_DW1 = SPLIT_DW0


PLAN_VR = ([(1, 0), (1, 2), (0, 1)],
           [((0, 0), "V"), ((0, 2), "V"), ((2, 0), "V"), ((2, 1), "G"),
            ((2, 2), "V")])


def emit_conv_vr(nc, wt, bt, src784, acc, tmp_fn, final_fn):
    """3x3 valid-region depthwise conv on a spatial [128, 784] tile (v3)."""
    p = 1
    v_taps, s_taps = PLAN_VR

    def dst_v(base, t):
        dy, dx = t
        y0, y1 = _region(RES, dy - p)
        x0, x1 = _region(RES, dx - p)
        return base[:].rearrange("p (h w) -> p h w", h=RES)[:, y0:y1, x0:x1]

    def src_v(t):
        dy, dx = t
        y0, y1 = _region(RES, dy - p)
        x0, x1 = _region(RES, dx - p)
        return src784.rearrange("p (h w) -> p h w", h=RES)[
            :, y0 + dy - p:y1 + dy - p, x0 + dx - p:x1 + dx - p]

    def w_(t):
        return wt[:, (t[0] * 3 + t[1]):(t[0] * 3 + t[1]) + 1]

    nc.vector.tensor_scalar(dst_v(acc, (1, 1)), src_v((1, 1)), w_((1, 1)),
                            bt[:, 0:1], AO.mult, AO.add)
    vq = list(v_taps)
    for i, (t, feng) in enumerate(s_taps):
        tm = tmp_fn(i % NSLOT)
        nc.scalar.activation(dst_v(tm, t), src_v(t), AF.Identity, scale=w_(t))
        av, tv = dst_v(acc, t), dst_v(tm, t)
        if feng == "G":
            nc.gpsimd.tensor_tensor(av, tv, av, AO.add)
        else:
            nc.vector.tensor_tensor(av, tv, av, AO.add)
        if vq:
            t2 = vq.pop(0)
            nc.vector.scalar_tensor_tensor(dst_v(acc, t2), src_v(t2), w_(t2),
                                           dst_v(acc, t2), AO.mult, AO.add)
    for t2 in vq:
        nc.vector.scalar_tensor_tensor(dst_v(acc, t2), src_v(t2), w_(t2),
                                       dst_v(acc, t2), AO.mult, AO.add)
    final_fn()


def emit_conv_dw(nc, wt, bt, g, g2, acc, tmp_fn, final_fn, split):
    """3x3 depthwise conv, flat shifted taps over a [128,30,32] padded grid.

    g2 is g shifted left one column (keeps even element offsets for the
    odd-dx taps). acc is [128, 28, 32]; flat cols 0..891 hold the interior.
    """
    DW_V_TAPS, DW_S_TAPS, DW_FOLD_G = split
    gf = g[:].rearrange("p h w -> p (h w)")
    g2f = g2[:].rearrange("p h w -> p (h w)") if g2 is not None else None
    af = acc[:].rearrange("p h w -> p (h w)")

    def src(t):
        dy, dx = t
        if g2f is not None and dx == 1:
            return g2f[:, 32 * dy:32 * dy + 892]
        return gf[:, 32 * dy + dx:32 * dy + dx + 892]

    def w_(t):
        return wt[:, (t[0] * 3 + t[1]):(t[0] * 3 + t[1]) + 1]

    nc.vector.tensor_scalar(af[:, 0:892], src((1, 1)), w_((1, 1)),
                            bt[:, 0:1], AO.mult, AO.add)
    vq = list(DW_V_TAPS)
    for i, t in enumerate(DW_S_TAPS):
        tm = tmp_fn(i % NSLOT)
        tf = tm[:].rearrange("p h w -> p (h w)") if len(tm.shape) == 3 else tm[:]
        nc.scalar.activation(tf[:, 0:892], src(t), AF.Identity, scale=w_(t))
        if t in DW_FOLD_G:
            nc.gpsimd.tensor_tensor(af[:, 0:892], tf[:, 0:892], af[:, 0:892],
                                    AO.add)
        else:
            nc.vector.tensor_tensor(af[:, 0:892], tf[:, 0:892], af[:, 0:892],
                                    AO.add)
        if vq:
            t2 = vq.pop(0)
            nc.vector.scalar_tensor_tensor(af[:, 0:892], src(t2), w_(t2),
                                           af[:, 0:892], AO.mult, AO.add)
    for t2 in vq:
        nc.vector.scalar_tensor_tensor(af[:, 0:892], src(t2), w_(t2),
                                       af[:, 0:892], AO.mult, AO.add)
    final_fn()


def emit_conv_grid(nc, k, wt, bt, G, GA, tmp_fn, warm_fn):
    """k*k depthwise conv on the shared guttered macro-grid [128, 7, 157]."""
    p = k // 2
    v_taps, s_taps = PLAN_DWQ[k]

    def dst_v(base, t):
        dy, dx = t
        y0, y1 = _region(GROWS, dy - p)
        x0, x1 = _region(GCOLS, dx - p)
        return base[:, y0:y1, x0:x1]

    def src_v(t):
        dy, dx = t
        y0, y1 = _region(GROWS, dy - p)
        x0, x1 = _region(GCOLS, dx - p)
        return G[:, y0 + dy - p:y1 + dy - p, x0 + dx - p:x1 + dx - p]

    def wcol(t):
        return t[0] * k + t[1]

    nc.vector.tensor_scalar(dst_v(GA[:], (p, p)), src_v((p, p)),
                            wt[:, wcol((p, p)):wcol((p, p)) + 1],
                            bt[:, 0:1], AO.mult, AO.add)
    vq = list(v_taps)
    for i, (t, feng) in enumerate(s_taps):
        tm = tmp_fn(i % NSLOT)
        nc.scalar.activation(dst_v(tm[:], t), src_v(t), AF.Identity,
                             scale=wt[:, wcol(t):wcol(t) + 1])
        av, tv = dst_v(GA[:], t), dst_v(tm[:], t)
        if feng == "G":
            nc.gpsimd.tensor_tensor(av, tv, av, AO.add)
        else:
            nc.vector.tensor_tensor(av, tv, av, AO.add)
        if warm_fn is not None and i % 2 == 0:
            warm_fn(tm, t)
        if vq:
            t2 = vq.pop(0)
            nc.vector.scalar_tensor_tensor(dst_v(GA[:], t2), src_v(t2),
                                           wt[:, wcol(t2):wcol(t2) + 1],
                                           dst_v(GA[:], t2), AO.mult, AO.add)
    for t2 in vq:
        nc.vector.scalar_tensor_tensor(dst_v(GA[:], t2), src_v(t2),
                                       wt[:, wcol(t2):wcol(t2) + 1],
                                       dst_v(GA[:], t2), AO.mult, AO.add)


# ---------------------------------------------------------------------------
# program builder
# ---------------------------------------------------------------------------

def build_program():
    nc = bacc.Bacc("TRN2", target_bir_lowering=False, debug=False,
                   enable_asserts=False, num_devices=NCORES)

    def din(name, shape, dt=f32):
        return nc.dram_tensor(name, list(shape), dt, kind="ExternalInput").ap()

    x_d = din("x", [NI, ED, POS], bf16)
    dwpk_d = din("dwpk", [128, 80])
    w1T0_d = din("w1T0", [ED, 2 * ED], bf16)
    b1f0_d = din("b1f0", [2 * ED], bf16)
    w2T0_d = din("w2T0", [2 * ED, ED], bf16)
    attkq_d = din("attkq", [128, NH * 2 * KD], bf16)
    attwv_d = din("attwv", [128, NH * D], bf16)
    attbias_d = din("attbias", [128, 2 * NH])
    dwqw_d = din("dwqw", [128, NH * 50])
    ab_d = din("ab", [WN, NH * 392], bf16)
    iab_d = din("iab", [WN, 128], bf16)
    ones2_d = din("ones2", [128, 2], bf16)
    sel2_d = din("sel2", [2, 128])
    projT_d = din("projT", [DH, ED], bf16)
    bpk_d = din("bpk", [128, 16])
    w1T1_d = din("w1T1", [ED, 2 * ED], bf16)
    b1f1_d = din("b1f1", [2 * ED], bf16)
    w2T1_d = din("w2T1", [2 * ED, ED], bf16)

    out_d = nc.dram_tensor("out", [NI, ED, POS], f32, kind="ExternalOutput").ap()

    with tile.TileContext(nc) as tc:
        _body(tc, nc, x_d, dwpk_d, w1T0_d, b1f0_d, w2T0_d,
              attkq_d, attwv_d, attbias_d, dwqw_d, ab_d,
              iab_d, ones2_d, sel2_d,
              projT_d, bpk_d,
              w1T1_d, b1f1_d, w2T1_d, out_d)

    nc.compile()
    return nc


def win_ap(ap392, n2, w, spatial):
    """Per-window [*, 49] AP from a 392-col half. spatial: 3D 7x7 slice of
    the 14x28 spatial half; else dense 49-block (window-block layout)."""
    if spatial:
        al, b = w // 4, w % 4
        v = ap392.rearrange("p (h x) -> p h x", h=14)
        return v[:, 7 * al:7 * al + 7, 7 * b:7 * b + 7]
    return ap392[:, WN * w:WN * (w + 1)]


def _body(tc, nc, x_d, dwpk_d, w1T0_d, b1f0_d, w2T0_d,
          attkq_d, attwv_d, attbias_d, dwqw_d, ab_d,
          iab_d, ones2_d, sel2_d,
          projT_d, bpk_d,
          w1T1_d, b1f1_d, w2T1_d, out_d):

    # ---------------- persistent pools -------------------------------------
    wp_cm = tc.tile_pool(name="wp", bufs=1)
    wp = wp_cm.__enter__()
    big_cm = tc.tile_pool(name="big", bufs=1)
    big = big_cm.__enter__()
    xw_cm = tc.tile_pool(name="xw", bufs=1)
    xwp = xw_cm.__enter__()

    def load_mlp_w(w1T_dram, b1_dram, w2T_dram, b2_dram, pool):
        w1sb = []
        for k in range(4):
            w = pool.tile([128, 2 * ED], bf16, tag=f"w1_{k}")
            nc.sync.dma_start(out=w, in_=w1T_dram[128 * k:128 * (k + 1), :])
            w1sb.append(w)
        w2sb = []
        for k in range(8):
            w = pool.tile([128, ED], bf16, tag=f"w2_{k}")
            nc.sync.dma_start(out=w, in_=w2T_dram[128 * k:128 * (k + 1), :])
            w2sb.append(w)
        b1row = pool.tile([1, 2 * ED], bf16, tag="b1row")
        nc.sync.dma_start(out=b1row, in_=b1_dram.unsqueeze(0))
        b2sb = []
        for m in range(4):
            b = pool.tile([128, 1], f32, tag=f"b2_{m}")
            nc.sync.dma_start(out=b, in_=b2_dram[128 * m:128 * (m + 1)].unsqueeze(1))
            b2sb.append(b)
        return w1sb, w2sb, b1row, b2sb

    # dw weights
    dw_w, dw_b = {}, {}
    for nm, wd, bd in (("dw0", dw0w_d, dw0b_d), ("dw1", dw1w_d, dw1b_d)):
        ws_, bs_ = [], []
        for c in range(4):
            w = wp.tile([128, 9], f32, tag=f"{nm}w{c}")
            nc.sync.dma_start(out=w, in_=wd[c])
            b = wp.tile([128, 1], f32, tag=f"{nm}b{c}")
            nc.sync.dma_start(out=b, in_=bd[c].unsqueeze(1))
            ws_.append(w)
            bs_.append(b)
        dw_w[nm], dw_b[nm] = ws_, bs_

    ones392 = wp.tile([1, 392], bf16, tag="ones392")
    nc.vector.memset(ones392, 1.0)
    acth = wp.tile([128, 1], f32, tag="acth")
    nc.vector.memset(acth, 0.5)
    acts = wp.tile([128, 1], f32, tag="acts")
    nc.vector.memset(acts, 1.0 / 6.0)

    # attention weights
    wkq_sb, bkq_sb, wv_sb, bv_sb, dq_w, dq_b, ab_sb = [], [], [], [], [], [], []
    for h in range(NH):
        t = wp.tile([128, 2 * KD], bf16, tag=f"wkq{h}")
        nc.sync.dma_start(out=t[0:64, :], in_=wkqT_d[h])
        nc.sync.dma_start(out=t[64:128, :], in_=wkqT_d[h])
        wkq_sb.append(t)
        t = wp.tile([128, 1], f32, tag=f"bkq{h}")
        nc.sync.dma_start(out=t[0:32, :], in_=bkq_d[h].unsqueeze(1))
        nc.sync.dma_start(out=t[64:96, :], in_=bkq_d[h].unsqueeze(1))
        bkq_sb.append(t)
        t = wp.tile([128, D], bf16, tag=f"wv{h}")
        nc.sync.dma_start(out=t[0:64, :], in_=wvT_d[h])
        nc.sync.dma_start(out=t[64:128, :], in_=wvT_d[h])
        wv_sb.append(t)
        t = wp.tile([128, 1], f32, tag=f"bv{h}")
        nc.sync.dma_start(out=t[0:64, :], in_=bv_d[h].unsqueeze(1))
        nc.sync.dma_start(out=t[64:128, :], in_=bv_d[h].unsqueeze(1))
        bv_sb.append(t)

        t = wp.tile([128, 49], f32, tag=f"dqw{h}")
        nc.sync.dma_start(out=t, in_=dwqw_d[h])
        dq_w.append(t)
        t = wp.tile([128, 1], f32, tag=f"dqb{h}")
        nc.sync.dma_start(out=t, in_=dwqb_d[h].unsqueeze(1))
        dq_b.append(t)
        t = wp.tile([WN, 392], bf16, tag=f"ab{h}")
        nc.sync.dma_start(out=t, in_=ab_d[h])
        ab_sb.append(t)
    iab_sb = wp.tile([WN, 128], bf16, tag="iab")
    nc.sync.dma_start(out=iab_sb, in_=iab_d)
    ones2_sb = wp.tile([128, 2], bf16, tag="ones2")
    nc.sync.dma_start(out=ones2_sb, in_=ones2_d)
    sel2_sb = wp.tile([2, 128], f32, tag="sel2")
    nc.sync.dma_start(out=sel2_sb, in_=sel2_d)

    # proj
    pj_sb = []
    for k in range(4):
        w = wp.tile([128, ED], bf16, tag=f"pj{k}")
        nc.sync.dma_start(out=w, in_=projT_d[128 * k:128 * (k + 1), :])
        pj_sb.append(w)
    pjb_sb, yb_sb = [], []
    for m in range(4):
        b = wp.tile([128, 1], f32, tag=f"pjb{m}")
        nc.sync.dma_start(out=b, in_=projb_d[128 * m:128 * (m + 1)].unsqueeze(1))
        pjb_sb.append(b)
        b = wp.tile([128, 1], f32, tag=f"ybt{m}")
        nc.sync.dma_start(out=b, in_=yb_d[128 * m:128 * (m + 1)].unsqueeze(1))
        yb_sb.append(b)

    w1sb0, w2sb0, b1row0, b2sb0 = load_mlp_w(w1T0_d, b1f0_d, w2T0_d, b2f0_d, wp)

    # ---------------- MLP per-image emitter --------------------------------
    def mlp_img(pools, img, w1sb, w2sb, b1row, b2sb, rhs_getter, out_writer, name):
        hp, rp, psp, pop = pools
        hs = []
        for m in range(8):
            h = hp.tile([128, POS], bf16, tag=f"h{m}", name=f"{name}h{m}_{img}")
            hs.append(h)
        for m in range(8):
            for n2 in range(2):
                ph = psp.tile([128, 392], f32, tag="ph")
                for k in range(4):
                    nc.tensor.matmul(
                        ph[:], w1sb[k][:, 128 * m:128 * (m + 1)],
                        rhs_getter(k, img, n2),
                        start=(k == 0), stop=False)
                nc.tensor.matmul(
                    ph[:], b1row[:, 128 * m:128 * (m + 1)],
                    ones392[:], start=False, stop=True)
                r = rp.tile([128, 392], bf16, tag="relu")
                nc.scalar.activation(r[:], ph[:], AF.Relu,
                                     scale=acts[:, 0:1], bias=acth[:, 0:1])
                nc.vector.scalar_tensor_tensor(
                    hs[m][:, 392 * n2:392 * (n2 + 1)], r[:], 1.0,
                    ph[:], AO.min, AO.mult)
        for mo in range(4):
            for n2 in range(2):
                po = pop.tile([128, 392], f32, tag="po")
                for k in range(8):
                    nc.tensor.matmul(
                        po[:], w2sb[k][:, 128 * mo:128 * (mo + 1)],
                        hs[k][:, 392 * n2:392 * (n2 + 1)],
                        start=(k == 0), stop=(k == 7))
                out_writer(mo, img, n2, po, b2sb[mo])

    # ---------------- P0+P1: input DMA + dw0 + residual -> x1 (spatial) -----
    x1fl = [big.tile([128, NI, POS], bf16, tag=f"fl{c}", name=f"x1_{c}")
            for c in range(4)]

    def rhs0(k, img, n2):
        return x1fl[k][:, img, 392 * n2:392 * (n2 + 1)]

    def outw0(mo, img, n2, po, b2):
        ov = x1fl[mo][:, img, 392 * n2:392 * (n2 + 1)]
        nc.vector.scalar_tensor_tensor(ov, po[:], b2[:, 0:1], ov, AO.add, AO.add)

    GN = 3
    with tc.tile_pool(name="stg", bufs=4) as stgp, \
         tc.tile_pool(name="dac", bufs=3) as dacp, \
         tc.tile_pool(name="dwt0", bufs=2) as tmp0, \
         tc.tile_pool(name="m0h", bufs=2) as hp0, \
         tc.tile_pool(name="m0r", bufs=4) as rp0, \
         tc.tile_pool(name="m0ps", bufs=4, space="PSUM") as psp0, \
         tc.tile_pool(name="m0po", bufs=2, space="PSUM") as pop0:
        for img in range(NI):
            for c in range(4):
                stg = stgp.tile([128, POS], bf16, tag="stg",
                                name=f"stg{c}_{img}")
                nc.scalar.dma_start(out=stg,
                                    in_=x_d[img, 128 * c:128 * (c + 1), :])
                acc = dacp.tile([128, POS], bf16, tag="acc",
                                name=f"a0_{c}_{img}")

                def tmp_fn(i):
                    return tmp0.tile([128, POS], bf16, tag=f"tm{i}",
                                     name=f"tm0_{c}_{img}_{i}")

                def final_fn():
                    nc.vector.tensor_tensor(x1fl[c][:, img, :], acc[:],
                                            stg[:], AO.add)

                emit_conv_vr(nc, dw_w["dw0"][c], dw_b["dw0"][c], stg[:],
                             acc, tmp_fn, final_fn)
            mlp_img((hp0, rp0, psp0, pop0), img, w1sb0, w2sb0, b1row0, b2sb0,
                    rhs0, outw0, "m0")

    x2fl = x1fl   # trunk now holds x2 (spatial, bf16)

    # ---------------- P3: cascaded attention -> y_sb ------------------------
    # y in window-block layout: y_sb[c][64*h2+d, img, 49*w + pos]
    y_sb = [xwp.tile([128, NI, POS], bf16, tag=f"wm{c}", name=f"y_{c}")
            for c in range(4)]

    def prow(i):
        return 64 * (i % 2)

    with ExitStack() as es:
        spkp = es.enter_context(tc.tile_pool(name="spk", bufs=1))
        spxp = es.enter_context(tc.tile_pool(name="spx", bufs=2))
        spp = es.enter_context(tc.tile_pool(name="sp", bufs=1))
        kqtp = es.enter_context(tc.tile_pool(name="kqt", bufs=1))
        kpkp = es.enter_context(tc.tile_pool(name="kpk", bufs=1))
        vtp = es.enter_context(tc.tile_pool(name="vt", bufs=1))
        qgp = es.enter_context(tc.tile_pool(name="qg", bufs=1))
        qgrp = es.enter_context(tc.tile_pool(name="qgr", bufs=1))
        qgap = es.enter_context(tc.tile_pool(name="qga", bufs=1))
        qtmp = es.enter_context(tc.tile_pool(name="qtm", bufs=1))
        attp = es.enter_context(tc.tile_pool(name="att", bufs=2))
        rsp = es.enter_context(tc.tile_pool(name="rsp", bufs=1))
        spop = es.enter_context(tc.tile_pool(name="spo", bufs=1))
        pkqp = es.enter_context(tc.tile_pool(name="pkq", bufs=1, space="PSUM"))
        pvtp = es.enter_context(tc.tile_pool(name="pvt", bufs=1, space="PSUM"))
        pap = es.enter_context(tc.tile_pool(name="pa", bufs=2, space="PSUM"))
        ps1p = es.enter_context(tc.tile_pool(name="ps1", bufs=1, space="PSUM"))
        pbcp = es.enter_context(tc.tile_pool(name="pbc", bufs=1, space="PSUM"))
        pavp = es.enter_context(tc.tile_pool(name="pav", bufs=2, space="PSUM"))

        spx_tiles = {}

        def fetch_spx(h):
            """Stage pair-packed spatial slice via DMA, then repack to
            window-block with V copies (@4x)."""
            c, h2 = h // 2, h % 2
            t = spkp.tile([128, 4, POS], bf16, tag="spk", name=f"spk{h}")
            xv = x2fl[c][64 * h2:64 * h2 + 64, :, :] \
                .rearrange("p (j t) x -> p t j x", t=2)
            for par in range(2):
                nc.gpsimd.dma_start(out=t[64 * par:64 * par + 64, :, :],
                                    in_=xv[:, par, :, :])
            twb = spxp.tile([128, 4, POS], bf16, tag="spxwb", name=f"spxwb{h}")
            for j in range(4):
                for n2 in range(2):
                    co = 392 * n2
                    for w in range(8):
                        nc.vector.tensor_copy(
                            twb[:, j, co + WN * w:co + WN * (w + 1)]
                            .rearrange("p (x y) -> p x y", x=7),
                            win_ap(t[:, j, co:co + 392], n2, w, spatial=True))
            spx_tiles[h] = twb

        # shared q-conv macro-grid: gutters zeroed once, window interiors
        # overwritten per head by the pack copies
        Gq = qgrp.tile([128, GROWS, GCOLS], bf16, tag="qpad", name="qpad")
        nc.gpsimd.memset(Gq[:], 0.0)

        fetch_spx(0)
        sp_all = spx_tiles[0]
        for h in range(NH):
            c, h2 = h // 2, h % 2
            if h + 1 < NH:
                fetch_spx(h + 1)

            kqt = kqtp.tile([128, 4, POS], bf16, tag="kqt", name=f"kqt{h}")
            k_pk = kpkp.tile([128, 2, POS], bf16, tag="k", name=f"k{h}")
            qstack = qgp.tile([128, POS], bf16, tag="qstack", name=f"qstack{h}")
            qp_pk = kpkp.tile([128, 2, POS], bf16, tag="qp", name=f"qp{h}")
            vt_pk = vtp.tile([128, 4 * 1024], bf16, tag="vt", name=f"vt{h}")

            # ---- A/B: kqv matmuls + evict + repack DMAs ----
            for j in range(4):          # image pairs (2j, 2j+1)
                for n2 in range(2):
                    pkq = pkqp.tile([128, 392], f32, tag="pkq",
                                    name=f"pkq{h}_{j}_{n2}")
                    pvt = pvtp.tile([128, 512], f32, tag="pvt",
                                    name=f"pvt{h}_{j}_{n2}")
                    for t_ in range(2):
                        img = 2 * j + t_
                        ob = 64 * t_
                        rhs_base = prow(img)
                        spi = sp_all[rhs_base:rhs_base + 64, img // 2,
                                     392 * n2:392 * (n2 + 1)]
                        nc.tensor.matmul(
                            pkq[ob:ob + 2 * KD, :],
                            wkq_sb[h][rhs_base:rhs_base + 64, :],
                            spi, start=True, stop=True,
                            tile_position=(rhs_base, ob))
                        for w in range(8):
                            nc.tensor.matmul(
                                pvt[ob:ob + WN, 64 * w:64 * (w + 1)],
                                spi[:, WN * w:WN * (w + 1)],
                                wv_sb[h][rhs_base:rhs_base + 64, :],
                                start=True, stop=True,
                                tile_position=(rhs_base, ob))
                    nc.scalar.activation(kqt[:, j, 392 * n2:392 * (n2 + 1)],
                                         pkq[:], AF.Identity,
                                         bias=bkq_sb[h][:, 0:1])
                    nc.scalar.activation(
                        vt_pk[:, 1024 * j + 512 * n2:1024 * j + 512 * (n2 + 1)],
                        pvt[:], AF.Copy)
                for t_ in range(2):
                    img = 2 * j + t_
                    rb = 64 * t_
                    nc.gpsimd.dma_start(
                        out=k_pk[32 * (img % 4):32 * (img % 4) + KD, img // 4, :],
                        in_=kqt[rb:rb + KD, j, :])
                    nc.gpsimd.dma_start(
                        out=qstack[KD * img:KD * (img + 1), :],
                        in_=kqt[rb + KD:rb + 2 * KD, j, :])

            # ---- C: depthwise conv on stacked q (shared guttered grid) ----
            kk = KS[h]
            qsv = qstack[:].rearrange("p (n s) -> p n s", n=NW)
            for w in range(NW):
                nc.vector.tensor_copy(
                    Gq[:, :, (7 + GUT) * w:(7 + GUT) * w + 7],
                    qsv[:, w, :].rearrange("p (x y) -> p x y", x=7))
            GA = qgap.tile([128, GROWS, GCOLS], bf16, tag="qacc",
                           name=f"qacc{h}")
            warm_ps = pbcp.tile([128, 392], f32, tag="pbc", name=f"warm{h}")

            def tmp_q(i):
                return qtmp.tile([128, GROWS, GCOLS], bf16, tag=f"qtm{i}",
                                 name=f"qtm{h}_{i}")

            def warm_fn(tm, t):
                p_ = kk // 2
                y0, _ = _region(GROWS, t[0] - p_)
                x0, _ = _region(GCOLS, t[1] - p_)
                nc.tensor.matmul(
                    warm_ps[0:2, 0:WN], ones2_sb[:],
                    tm[:, y0, x0:x0 + WN],
                    start=True, stop=True, tile_position=(0, 0))

            emit_conv_grid(nc, kk, dq_w[h], dq_b[h], Gq[:], GA, tmp_q, warm_fn)
            # unpack to window-block layout
            qflat = qgp.tile([128, NW, WN], bf16, tag="qflat", name=f"qflat{h}")
            for w in range(NW):
                nc.vector.tensor_copy(
                    qflat[:, w, :].rearrange("p (x y) -> p x y", x=7),
                    GA[:, :, (7 + GUT) * w:(7 + GUT) * w + 7])
            for img in range(NI):
                nc.gpsimd.dma_start(
                    out=qp_pk[32 * (img % 4):32 * (img % 4) + KD, img // 4, :],
                    in_=qflat[KD * img:KD * (img + 1)].rearrange("q n s -> q (n s)"))

            # ---- D: attention per pair ----
            spn = None
            if h + 1 < NH:
                spn = spp.tile([128, 4, POS], bf16, tag="sp", name=f"sp{h + 1}")
            spo_all = spop.tile([128, 4, POS], bf16, tag="spo", name=f"spo{h}")
            for j in range(4):
                for n2 in range(2):
                    pa = pap.tile([128, 392], f32, tag="pa", name=f"pa{h}_{j}_{n2}")
                    for t_ in range(2):
                        img = 2 * j + t_
                        ob = 64 * t_
                        q0 = 32 * (img % 4)
                        kh = k_pk[q0:q0 + KD, img // 4, :]
                        qh = qp_pk[q0:q0 + KD, img // 4, :]
                        for w in range(8):
                            co_ = 392 * n2 + WN * w
                            nc.tensor.matmul(
                                pa[ob:ob + WN, WN * w:WN * (w + 1)],
                                kh[:, co_:co_ + WN], qh[:, co_:co_ + WN],
                                start=True, stop=False,
                                tile_position=(q0, ob))
                    nc.tensor.matmul(pa[:], iab_sb[:], ab_sb[h][:],
                                     start=False, stop=True,
                                     tile_position=(0, 0))
                    ein = attp.tile([128, 392], bf16, tag="ein",
                                    name=f"ein{h}_{j}_{n2}")
                    nc.scalar.activation(ein[:], pa[:], AF.Exp)
                    ps1 = ps1p.tile([2, 392], f32, tag="ps1",
                                    name=f"ps1{h}_{j}_{n2}")
                    nc.tensor.matmul(ps1[:], ones2_sb[:], ein[:],
                                     start=True, stop=True,
                                     tile_position=(0, 0))
                    rs = rsp.tile([2, 392], f32, tag="rs", name=f"rs{h}_{j}_{n2}")
                    nc.vector.reciprocal_approx_fast(rs[:], ps1[:])
                    pbc = pbcp.tile([128, 392], f32, tag="pbc",
                                    name=f"pbc{h}_{j}_{n2}")
                    nc.tensor.matmul(pbc[:], sel2_sb[:], rs[:],
                                     start=True, stop=True,
                                     tile_position=(0, 0))
                    bc = attp.tile([128, 392], bf16, tag="bc",
                                   name=f"bc{h}_{j}_{n2}")
                    nc.scalar.activation(bc[:], pbc[:], AF.Copy)
                    pav = pavp.tile([128, 392], f32, tag="pav",
                                    name=f"pav{h}_{j}_{n2}")
                    for t_ in range(2):
                        img = 2 * j + t_
                        ob = 64 * t_
                        for w in range(8):
                            wg = 8 * n2 + w
                            nc.tensor.matmul(
                                pav[ob:ob + D, WN * w:WN * (w + 1)],
                                vt_pk[ob:ob + WN,
                                      1024 * j + 64 * wg:1024 * j + 64 * (wg + 1)],
                                ein[ob:ob + WN, WN * w:WN * (w + 1)],
                                start=True, stop=True,
                                tile_position=(ob, ob))
                    co = 392 * n2
                    nc.vector.tensor_tensor(spo_all[:, j, co:co + 392], pav[:],
                                            bc[:], AO.mult)
                    if spn is not None:
                        nc.vector.scalar_tensor_tensor(
                            spn[:, j, co:co + 392],
                            spo_all[:, j, co:co + 392],
                            bv_sb[h][:, 0:1],
                            spx_tiles[h + 1][:, j, co:co + 392],
                            AO.add, AO.add)
            yv = y_sb[c][64 * h2:64 * h2 + 64, :, :] \
                .rearrange("p (j t) x -> p t j x", t=2)
            for t_ in range(2):
                nc.gpsimd.dma_start(out=yv[:, t_, :, :],
                                    in_=spo_all[64 * t_:64 * t_ + 64, :, :])
            sp_all = spn

    # ---------------- P4+P5+P6 fused per image ------------------------------
    # y is window-block; proj output window-block; x3 written spatially (trunk)
    x3fl = x2fl
    x4fl = [xwp.tile([128, NI, POS], bf16, tag=f"wm{c}", name=f"x4_{c}")
            for c in range(4)]
    w1sb1, w2sb1, b1row1 = load_mlp_w(w1T1_d, b1f1_d, w2T1_d, wp)

    def rhs1(k, img, n2):
        return x4fl[k][:, img, 392 * n2:392 * (n2 + 1)]

    with ExitStack() as es:
        hyp = es.enter_context(tc.tile_pool(name="hyp", bufs=2))
        pjrp = es.enter_context(tc.tile_pool(name="pjr", bufs=2))
        dacp = es.enter_context(tc.tile_pool(name="dac1", bufs=3))
        tmp1 = es.enter_context(tc.tile_pool(name="dwt1", bufs=2))
        o5p = es.enter_context(tc.tile_pool(name="o5", bufs=2))
        hp1 = es.enter_context(tc.tile_pool(name="m1h", bufs=2))
        rp1 = es.enter_context(tc.tile_pool(name="m1r", bufs=4))
        ppp = es.enter_context(tc.tile_pool(name="ppp", bufs=2, space="PSUM"))
        psp1 = es.enter_context(tc.tile_pool(name="m1ps", bufs=4, space="PSUM"))
        pop1 = es.enter_context(tc.tile_pool(name="m1po", bufs=2, space="PSUM"))

        def outw1(mo, img, n2, po, b2):
            x5 = o5p.tile([128, 392], f32, tag="x5", name=f"x5_{mo}_{img}_{n2}")
            nc.vector.scalar_tensor_tensor(
                x5[:], po[:], b2[:, 0:1],
                x4fl[mo][:, img, 392 * n2:392 * (n2 + 1)], AO.add, AO.add)
            nc.sync.dma_start(
                out=out_d[img, 128 * mo:128 * (mo + 1), 392 * n2:392 * (n2 + 1)],
                in_=x5[:])

        for img in range(NI):
            # P4: hswish(y + yb), proj, x3 = x2 + proj + pjb
            hys = []
            for cb in range(4):
                yv = y_sb[cb][:, img, :]
                nc.vector.tensor_scalar(yv, yv, yb_sb[cb][:, 0:1], None,
                                        AO.add)
                r = pjrp.tile([128, POS], bf16, tag="pr")
                nc.scalar.activation(r[:], yv, AF.Relu,
                                     scale=acts[:, 0:1], bias=acth[:, 0:1])
                hy = hyp.tile([128, POS], bf16, tag=f"hy{cb}", name=f"hy{cb}_{img}")
                nc.vector.scalar_tensor_tensor(hy[:], r[:], 1.0, yv,
                                               AO.min, AO.mult)
                hys.append(hy)
            for mo in range(4):
                for n2 in range(2):
                    pp = ppp.tile([128, 392], f32, tag="pp")
                    for k in range(4):
                        nc.tensor.matmul(pp[:], pj_sb[k][:, 128 * mo:128 * (mo + 1)],
                                         hys[k][:, 392 * n2:392 * (n2 + 1)],
                                         start=(k == 0), stop=(k == 3))
                    ppe = pjrp.tile([128, 392], bf16, tag="ppe",
                                    name=f"ppe{mo}_{img}_{n2}")
                    nc.scalar.activation(ppe[:], pp[:], AF.Identity,
                                         bias=pjb_sb[mo][:, 0:1])
                    ov = x2fl[mo][:, img, 392 * n2:392 * (n2 + 1)]
                    for w in range(8):
                        nc.vector.tensor_tensor(
                            win_ap(ov, n2, w, spatial=True),
                            ppe[:, WN * w:WN * (w + 1)]
                            .rearrange("p (x y) -> p x y", x=7),
                            win_ap(ov, n2, w, spatial=True),
                            AO.add)
            # P5: dw1 units for this image (spatial trunk -> x4 spatial)
            for cb in range(4):
                srcv = x3fl[cb][:, img, :]
                acc = dacp.tile([128, POS], bf16, tag="acc",
                                name=f"a1_{cb}_{img}")

                def tmp_fn(i):
                    return tmp1.tile([128, POS], bf16, tag=f"tm{i}",
                                     name=f"tm1_{cb}_{img}_{i}")

                def final_fn():
                    nc.vector.tensor_tensor(x4fl[cb][:, img, :], acc[:],
                                            srcv, AO.add)

                emit_conv_vr(nc, dw_w["dw1"][cb], dw_b["dw1"][cb], srcv,
                             acc, tmp_fn, final_fn)
            # P6: MLP1 for this image
            mlp_img((hp1, rp1, psp1, pop1), img, w1sb1, w2sb1, b1row1, b2sb1,
                    rhs1, outw1, "m1")

    xw_cm.__exit__(None, None, None)
    big_cm.__exit__(None, None, None)
    wp_cm.__exit__(None, None, None)


# ---------------------------------------------------------------------------
# host-side input preprocessing
# ---------------------------------------------------------------------------

def prep_weights(inp):
    def taps(w):  # [C,1,k,k] -> [C, k*k]
        return w.reshape(w.shape[0], -1).astype(np.float32)

    m = {}
    dwpk = np.zeros((128, 80), np.float32)
    for ci in range(4):
        dwpk[:, 10 * ci:10 * ci + 9] = taps(inp["dw0_w"])[128 * ci:128 * (ci + 1)]
        dwpk[:, 10 * ci + 9] = inp["dw0_b"][128 * ci:128 * (ci + 1)]
        dwpk[:, 40 + 10 * ci:40 + 10 * ci + 9] = \
            taps(inp["dw1_w"])[128 * ci:128 * (ci + 1)]
        dwpk[:, 40 + 10 * ci + 9] = inp["dw1_b"][128 * ci:128 * (ci + 1)]
    m["dwpk"] = dwpk
    m["w1T0"] = np.ascontiguousarray(inp["ffn0_w1"].T).astype(ml_dtypes.bfloat16)
    m["b1f0"] = inp["ffn0_b1"].astype(ml_dtypes.bfloat16)
    m["w2T0"] = np.ascontiguousarray(inp["ffn0_w2"].T).astype(ml_dtypes.bfloat16)

    qkv_w, qkv_b = inp["qkv_w"], inp["qkv_b"]
    wkqT = np.empty((NH, D, 2 * KD), np.float32)
    bkq = np.empty((NH, 2 * KD), np.float32)
    wvT = np.empty((NH, D, D), np.float32)
    bv = np.empty((NH, D), np.float32)
    for h in range(NH):
        W = qkv_w[h]  # [96, 64]
        wkqT[h, :, 0:KD] = W[KD:2 * KD].T       # k
        wkqT[h, :, KD:2 * KD] = W[0:KD].T       # q
        bkq[h, 0:KD] = qkv_b[h, KD:2 * KD]
        bkq[h, KD:2 * KD] = qkv_b[h, 0:KD]
        wvT[h] = W[2 * KD:].T
        bv[h] = qkv_b[h, 2 * KD:]
    # packed: [128, NH*32] kq weights (row halves duplicated)
    akq = np.empty((128, NH * 2 * KD), np.float32)
    awv = np.empty((128, NH * D), np.float32)
    abias = np.zeros((128, 2 * NH), np.float32)
    for h in range(NH):
        akq[0:64, 32 * h:32 * h + 32] = wkqT[h]
        akq[64:128, 32 * h:32 * h + 32] = wkqT[h]
        awv[0:64, 64 * h:64 * h + 64] = wvT[h]
        awv[64:128, 64 * h:64 * h + 64] = wvT[h]
        abias[0:32, 2 * h] = bkq[h]
        abias[64:96, 2 * h] = bkq[h]
        abias[0:64, 2 * h + 1] = bv[h]
        abias[64:128, 2 * h + 1] = bv[h]
    m["attkq"] = akq.astype(ml_dtypes.bfloat16)
    m["attwv"] = awv.astype(ml_dtypes.bfloat16)
    m["attbias"] = abias

    dwq_ws = [inp["dwq_w7"], inp["dwq_w5"]] + [inp["dwq_w3"][i] for i in range(6)]
    dwq_bs = [inp["dwq_b7"], inp["dwq_b5"]] + [inp["dwq_b3"][i] for i in range(6)]
    dwqw = np.zeros((128, NH * 50), np.float32)
    for h in range(NH):
        t = taps(dwq_ws[h]) * SCALE
        nt = t.shape[1]
        for i in range(NI):
            dwqw[KD * i:KD * (i + 1), 50 * h:50 * h + nt] = t
            dwqw[KD * i:KD * (i + 1), 50 * h + 49] = dwq_bs[h] * SCALE
    m["dwqw"] = dwqw

    ab = inp["attn_bias"][:, BIAS_IDX]       # [NH, 49, 49]
    ab = np.tile(ab, (1, 1, 8))              # [NH, 49, 392]
    m["ab"] = ab.transpose(1, 0, 2).reshape(WN, NH * 392).copy() \
        .astype(ml_dtypes.bfloat16)

    iab = np.zeros((WN, 128), np.float32)
    for i in range(WN):
        iab[i, i] = 1.0
        iab[i, 64 + i] = 1.0
    m["iab"] = iab.astype(ml_dtypes.bfloat16)
    ones2 = np.zeros((128, 2), np.float32)
    ones2[0:WN, 0] = 1.0
    ones2[64:64 + WN, 1] = 1.0
    m["ones2"] = ones2.astype(ml_dtypes.bfloat16)
    sel2 = np.zeros((2, 128), np.float32)
    sel2[0, 0:64] = 1.0
    sel2[1, 64:128] = 1.0
    m["sel2"] = sel2

    m["projT"] = np.ascontiguousarray(inp["proj_w"].T).astype(ml_dtypes.bfloat16)
    bpk = np.zeros((128, 16), np.float32)
    for ci in range(4):
        bpk[:, ci] = inp["proj_b"][128 * ci:128 * (ci + 1)]
        bpk[:, 4 + ci] = inp["ffn0_b2"][128 * ci:128 * (ci + 1)]
        bpk[:, 8 + ci] = inp["ffn1_b2"][128 * ci:128 * (ci + 1)]
        bpk[:, 12 + ci] = bv.reshape(ED)[128 * ci:128 * (ci + 1)]
    m["bpk"] = bpk

    m["w1T1"] = np.ascontiguousarray(inp["ffn1_w1"].T).astype(ml_dtypes.bfloat16)
    m["b1f1"] = inp["ffn1_b1"].astype(ml_dtypes.bfloat16)
    m["w2T1"] = np.ascontiguousarray(inp["ffn1_w2"].T).astype(ml_dtypes.bfloat16)
    return m


@functools.lru_cache(maxsize=1)
def _cached_program():
    return build_program()


def _run(inputs, trace=False, **kw):
    nc = _cached_program()
    wm = prep_weights(inputs)
    x = np.asarray(inputs["x"], dtype=np.float32).reshape(64, ED, POS)
    x = x.astype(ml_dtypes.bfloat16)
    in_maps = []
    for core in range(NCORES):
        im = dict(wm)
        im["x"] = np.ascontiguousarray(x[NI * core:NI * (core + 1)])
        in_maps.append(im)
    res = bass_utils.run_bass_kernel_spmd(nc, in_maps, list(range(NCORES)),
                                          trace=trace, **kw)
    out = np.concatenate([r["out"] for r in res.results], axis=0)
    return out.reshape(64, ED, RES, RES).astype(np.float32), res


def kernel(**inputs):
    out, _ = _run(inputs)
    return out
